# revision 1
# baseline (speedup 1.0000x reference)
"""MoD (mixture-of-depths) MLP wrapper kernel for Trainium2, 8 NeuronCores.

Sharding: core c handles batch row b = c//2 and the half of that row's
top-K tokens with global selection ranks in [h*1024, (h+1)*1024), h = c%2.
Each core computes the full row's router scores + top-K threshold locally
(no collectives), gathers exactly 1024 token rows by rank via indirect DMA,
runs the FFN in bf16 (fp32 accumulation), and scatters results back into a
zero-filled per-core output buffer.  Host sums the two buffers of each row.
"""

import sys, os

sys.path.insert(0, "/opt/trn_rl_repo")

from contextlib import ExitStack

import numpy as np

from concourse import bass, mybir
from concourse import bacc
import concourse.tile as tile
from concourse.bass import IndirectOffsetOnAxis

B, L, D = 4, 4096, 1024
DFF = 4 * D
K = L // 2              # 2048 selected tokens per row
NCORES = 8
P = 128
NT = L // P             # 32 token tiles per row
SEL = K // 2            # 1024 selected tokens per core
NSJ = SEL // P          # 8 selected-token blocks
ND = D // P             # 8 d chunks
NM = DFF // P           # 32 dff tiles
NKGRP = 4               # w2 k-chunks per streamed tile
RADIX_PASSES = 4
OOB_SENTINEL = 2 * L    # > bounds_check => skipped by indirect DMA

F32 = mybir.dt.float32
BF16 = mybir.dt.bfloat16
I32 = mybir.dt.int32
Alu = mybir.AluOpType
Act = mybir.ActivationFunctionType


def build_program():
    nc = bacc.Bacc(
        "TRN2",
        target_bir_lowering=False,
        debug=False,
        enable_asserts=False,
        num_devices=NCORES,
    )

    x_row = nc.dram_tensor("x_row", [L, D], F32, kind="ExternalInput").ap()
    w1 = nc.dram_tensor("w1", [D, DFF], F32, kind="ExternalInput").ap()
    w2 = nc.dram_tensor("w2", [DFF, D], F32, kind="ExternalInput").ap()
    wr = nc.dram_tensor("wr", [1, D], F32, kind="ExternalInput").ap()
    br = nc.dram_tensor("br", [1, 1], F32, kind="ExternalInput").ap()
    b1t = nc.dram_tensor("b1t", [P, NM], F32, kind="ExternalInput").ap()
    b2 = nc.dram_tensor("b2", [1, D], F32, kind="ExternalInput").ap()
    hbase = nc.dram_tensor("hbase", [1, 1], F32, kind="ExternalInput").ap()
    ident = nc.dram_tensor("ident128", [P, P], F32, kind="ExternalInput").ap()
    ltri = nc.dram_tensor("ltri128", [P, P], F32, kind="ExternalInput").ap()
    slt32 = nc.dram_tensor("slt32", [NT, NT], F32, kind="ExternalInput").ap()
    id32 = nc.dram_tensor("id32", [NT, NT], F32, kind="ExternalInput").ap()
    ones_1x128 = nc.dram_tensor("ones_1x128", [1, P], F32, kind="ExternalInput").ap()
    ones_1x128b = nc.dram_tensor("ones_1x128b", [1, P], BF16, kind="ExternalInput").ap()
    ones_128x1 = nc.dram_tensor("ones_128x1", [P, 1], F32, kind="ExternalInput").ap()
    ones_32x128 = nc.dram_tensor("ones_32x128", [NT, P], F32, kind="ExternalInput").ap()

    out_row = nc.dram_tensor("out_row", [L, D], F32, kind="ExternalOutput").ap()

    scores_d = nc.dram_tensor("scores_d", [P, NT], F32).ap()
    selidx2_d = nc.dram_tensor("selidx2_d", [SEL, 1], F32).ap()

    with tile.TileContext(nc) as tc, ExitStack() as S0:
        const = S0.enter_context(tc.tile_pool(name="const", bufs=1))
        w1_pool = S0.enter_context(tc.tile_pool(name="w1bf", bufs=1))

        # ---- small constant loads ------------------------------------------------
        def cload(pool, ap, shape, dtype=F32, name=None):
            t = pool.tile(shape, dtype, name=name)
            nc.sync.dma_start(out=t[:], in_=ap)
            return t

        br_sb = cload(const, br, [1, 1], name="c_br")
        hb_sb = cload(const, hbase, [1, 1], name="c_hb")
        b1t_sb = cload(const, b1t, [P, NM], name="c_b1t")
        ident_sb = cload(const, ident, [P, P], name="c_id")
        ltri_sb = cload(const, ltri, [P, P], name="c_lt")
        slt32_sb = cload(const, slt32, [NT, NT], name="c_sl")
        id32_sb = cload(const, id32, [NT, NT], name="c_id32")
        o1x128_sb = cload(const, ones_1x128, [1, P], name="c_o1")
        o1x128b_sb = cload(const, ones_1x128b, [1, P], BF16, name="c_o1b")
        o128x1_sb = cload(const, ones_128x1, [P, 1], name="c_oc")
        o32x128_sb = cload(const, ones_32x128, [NT, P], name="c_o32")
        b2bf_sb = const.tile([1, D], BF16)
        nc.gpsimd.dma_start(out=b2bf_sb[:], in_=b2)  # cast f32 -> bf16

        br_col = const.tile([P, 1], F32)
        nc.gpsimd.partition_broadcast(br_col[:], br_sb[:])
        hb_col = const.tile([P, 1], F32)
        nc.gpsimd.partition_broadcast(hb_col[:], hb_sb[:])

        iota_i = const.tile([P, 1], I32)
        nc.gpsimd.iota(iota_i[:], pattern=[[1, 1]], base=0, channel_multiplier=1)
        iota_f = const.tile([P, 1], F32)
        nc.vector.tensor_copy(out=iota_f[:], in_=iota_i[:])

        tokid = const.tile([P, NT], I32)
        nc.gpsimd.iota(tokid[:], pattern=[[P, NT]], base=0, channel_multiplier=1)
        iota512 = const.tile([P, 512], I32)
        nc.gpsimd.iota(iota512[:], pattern=[[1, 512]], base=0, channel_multiplier=0)
        iota512f = const.tile([P, 512], F32)
        nc.vector.tensor_copy(out=iota512f[:], in_=iota512[:])
        tokidf = const.tile([P, NT], F32)
        nc.vector.tensor_copy(out=tokidf[:], in_=tokid[:])

        scores_sb = const.tile([P, NT], F32)
        selidx_sb = const.tile([P, NSJ], I32)
        offf_c = const.tile([P, NT], F32)

        # ---- w1 resident loads (cast f32->bf16 during DMA), overlap prefix ------
        w1bf = []
        for kd in range(ND):
            t_ = w1_pool.tile([P, DFF], BF16, name=f"w1bf_{kd}")
            nc.gpsimd.dma_start(out=t_[:], in_=w1[kd * P:(kd + 1) * P, :])
            w1bf.append(t_)

        if os.environ.get("KVAR") == "noprefix":
            # diagnostic variant: synthetic selection (first 1024 tokens)
            offi_tmp = const.tile([P, NT], I32)
            nc.vector.memset(offi_tmp[:], OOB_SENTINEL)
            nc.gpsimd.iota(offi_tmp[:, :NSJ], pattern=[[P, NSJ]], base=0,
                           channel_multiplier=1)
            nc.vector.tensor_copy(out=offf_c[:], in_=offi_tmp[:])
        if os.environ.get("KVAR") != "noprefix":
          with ExitStack() as SPM:
            misc_psum = SPM.enter_context(tc.tile_pool(name="misc_psum", bufs=2, space="PSUM"))

            # ---- phase A: router scores (fp32, exact) ----------------------------
            with ExitStack() as SA:
                apool = SA.enter_context(tc.tile_pool(name="apool", bufs=1))
                xs_pool = SA.enter_context(tc.tile_pool(name="xs", bufs=5))
                junk_pool = SA.enter_context(tc.tile_pool(name="junk", bufs=2))

                wr_sb = cload(apool, wr, [1, D], name="c_wr")
                wrb = apool.tile([P, D], F32)
                for n in range(D // 512):
                    pt = misc_psum.tile([P, 512], F32, name="mp")
                    nc.tensor.matmul(out=pt[:], lhsT=o1x128_sb[:],
                                     rhs=wr_sb[:, n * 512:(n + 1) * 512],
                                     start=True, stop=True)
                    nc.vector.tensor_copy(out=wrb[:, n * 512:(n + 1) * 512], in_=pt[:])

                for t in range(NT):
                    x_t = xs_pool.tile([P, D], F32)
                    nc.sync.dma_start(out=x_t[:], in_=x_row[t * P:(t + 1) * P, :])
                    prod = junk_pool.tile([P, D], F32, name="prod")
                    nc.vector.tensor_tensor(out=prod[:], in0=x_t[:], in1=wrb[:],
                                            op=Alu.mult)
                    sink = junk_pool.tile([P, D], BF16, name="sink")
                    nc.scalar.activation(out=sink[:], in_=prod[:], func=Act.Identity,
                                         bias=0.0, scale=1.0,
                                         accum_out=scores_sb[:, t:t + 1])
                # add router bias once
                scores2 = apool.tile([P, NT], F32, name="scores2")
                nc.vector.tensor_tensor(out=scores2[:], in0=scores_sb[:],
                                        in1=br_col[:, :1].to_broadcast([P, NT]),
                                        op=Alu.add)
                nc.vector.tensor_copy(out=scores_sb[:], in_=scores2[:])

            # ---- phase C: top-K threshold via 128-way bisection ------------------
            with ExitStack() as SC:
                radix = SC.enter_context(tc.tile_pool(name="radix", bufs=2))
                rjunk = SC.enter_context(tc.tile_pool(name="rjunk", bufs=1))
                rep_pool = SC.enter_context(tc.tile_pool(name="rep", bufs=1))

                nc.sync.dma_start(out=scores_d, in_=scores_sb[:])
                scores_row = rep_pool.tile([1, L], F32)
                nc.sync.dma_start(out=scores_row[:], in_=scores_d.rearrange("p c -> () (p c)"))
                scores_rep = rep_pool.tile([P, L], F32)
                for n in range(L // 512):
                    pt = misc_psum.tile([P, 512], F32, name="mp")
                    nc.tensor.matmul(out=pt[:], lhsT=o1x128_sb[:],
                                     rhs=scores_row[:, n * 512:(n + 1) * 512],
                                     start=True, stop=True)
                    nc.vector.tensor_copy(out=scores_rep[:, n * 512:(n + 1) * 512], in_=pt[:])

                lo = radix.tile([1, 1], F32, name="lo")
                nc.vector.memset(lo[:], -16.0)
                w_ = radix.tile([1, 1], F32, name="w")
                nc.vector.memset(w_[:], 32.0 / P)
                thrb = radix.tile([P, 1], F32, name="thrb")
                nc.vector.tensor_scalar(out=thrb[:], in0=iota_f[:], scalar1=32.0 / P,
                                        scalar2=None, op0=Alu.mult)
                for _pass in range(RADIX_PASSES):
                    lo_c = radix.tile([P, 1], F32, name="lo_c")
                    nc.gpsimd.partition_broadcast(lo_c[:], lo[:])
                    thr2 = radix.tile([P, 1], F32, name="thr2")
                    nc.vector.tensor_tensor(out=thr2[:], in0=thrb[:], in1=lo_c[:], op=Alu.add)
                    cnt = radix.tile([P, 1], F32, name="cnt")
                    junk2 = rjunk.tile([P, L], F32, name="junk2")
                    nc.vector.tensor_tensor(out=junk2[:], in0=scores_rep[:],
                                            in1=thr2[:, :1].to_broadcast([P, L]),
                                            op=Alu.is_ge)
                    sink2 = rjunk.tile([P, L], BF16, name="sink2")
                    nc.scalar.activation(out=sink2[:], in_=junk2[:], func=Act.Identity,
                                         bias=0.0, scale=1.0, accum_out=cnt[:])
                    sel = radix.tile([P, 1], F32, name="sel")
                    nc.vector.tensor_scalar(out=sel[:], in0=cnt[:], scalar1=float(K),
                                            scalar2=None, op0=Alu.is_ge)
                    ssum_p = misc_psum.tile([1, 1], F32, name="mp")
                    nc.tensor.matmul(out=ssum_p[:], lhsT=sel[:], rhs=o128x1_sb[:],
                                     start=True, stop=True)
                    s_sb = radix.tile([1, 1], F32, name="s_sb")
                    nc.vector.tensor_copy(out=s_sb[:], in_=ssum_p[:])
                    ps = radix.tile([1, 1], F32, name="ps")
                    nc.vector.tensor_scalar(out=ps[:], in0=s_sb[:], scalar1=-1.0,
                                            scalar2=None, op0=Alu.add)
                    d_ = radix.tile([1, 1], F32, name="d_")
                    nc.vector.tensor_tensor(out=d_[:], in0=ps[:], in1=w_[:], op=Alu.mult)
                    lo2 = radix.tile([1, 1], F32, name="lo2")
                    nc.vector.tensor_tensor(out=lo2[:], in0=lo[:], in1=d_[:], op=Alu.add)
                    w2_ = radix.tile([1, 1], F32, name="w2_")
                    nc.vector.tensor_scalar(out=w2_[:], in0=w_[:], scalar1=1.0 / P,
                                            scalar2=None, op0=Alu.mult)
                    thrb2 = radix.tile([P, 1], F32, name="thrb")
                    nc.vector.tensor_scalar(out=thrb2[:], in0=thrb[:], scalar1=1.0 / P,
                                            scalar2=None, op0=Alu.mult)
                    lo, w_, thrb = lo2, w2_, thrb2

                # ---- mask, global rank, rank-window compaction -------------------
                T_col = radix.tile([P, 1], F32, name="T_col")
                nc.gpsimd.partition_broadcast(T_col[:], lo[:])
                maskf = radix.tile([P, NT], F32, name="maskf")
                nc.vector.tensor_tensor(out=maskf[:], in0=scores_sb[:],
                                        in1=T_col[:, :1].to_broadcast([P, NT]), op=Alu.is_ge)

                colsum_p = misc_psum.tile([NT, 1], F32, name="mp")
                nc.tensor.matmul(out=colsum_p[:], lhsT=maskf[:], rhs=o128x1_sb[:],
                                 start=True, stop=True)
                colsum = radix.tile([NT, 1], F32, name="colsum")
                nc.vector.tensor_copy(out=colsum[:], in_=colsum_p[:])
                excl_p = misc_psum.tile([NT, 1], F32, name="mp")
                nc.tensor.matmul(out=excl_p[:], lhsT=slt32_sb[:], rhs=colsum[:],
                                 start=True, stop=True)
                excl = radix.tile([NT, 1], F32, name="excl")
                nc.vector.tensor_copy(out=excl[:], in_=excl_p[:])
                diag = radix.tile([NT, NT], F32, name="diag")
                nc.vector.tensor_tensor(out=diag[:], in0=id32_sb[:],
                                        in1=excl[:, :1].to_broadcast([NT, NT]), op=Alu.mult)
                rank_p = misc_psum.tile([P, NT], F32, name="mp")
                nc.tensor.matmul(out=rank_p[:], lhsT=ltri_sb[:], rhs=maskf[:],
                                 start=True, stop=False, skip_group_check=True)
                nc.tensor.matmul(out=rank_p[:], lhsT=o32x128_sb[:], rhs=diag[:],
                                 start=False, stop=True, skip_group_check=True)
                rank = radix.tile([P, NT], F32, name="rank")
                nc.vector.tensor_copy(out=rank[:], in_=rank_p[:])

                off = radix.tile([P, NT], F32, name="off")
                nc.vector.tensor_tensor(out=off[:], in0=rank[:],
                                        in1=hb_col[:, :1].to_broadcast([P, NT]),
                                        op=Alu.subtract)
                w0 = radix.tile([P, NT], F32, name="w0")
                nc.vector.tensor_scalar(out=w0[:], in0=off[:], scalar1=0.0, scalar2=None,
                                        op0=Alu.is_ge)
                w1m = radix.tile([P, NT], F32, name="w1m")
                nc.vector.tensor_scalar(out=w1m[:], in0=off[:], scalar1=float(SEL),
                                        scalar2=None, op0=Alu.is_lt)
                m2 = radix.tile([P, NT], F32, name="m2")
                nc.vector.tensor_tensor(out=m2[:], in0=w0[:], in1=w1m[:], op=Alu.mult)
                m3 = radix.tile([P, NT], F32, name="m3")
                nc.vector.tensor_tensor(out=m3[:], in0=m2[:], in1=maskf[:], op=Alu.mult)
                t1 = radix.tile([P, NT], F32, name="t1")
                nc.vector.tensor_scalar(out=t1[:], in0=off[:],
                                        scalar1=-float(OOB_SENTINEL),
                                        scalar2=None, op0=Alu.add)
                t2 = radix.tile([P, NT], F32, name="t2")
                nc.vector.tensor_tensor(out=t2[:], in0=t1[:], in1=m3[:], op=Alu.mult)
                offf = radix.tile([P, NT], F32, name="offf")
                nc.vector.tensor_scalar(out=offf[:], in0=t2[:],
                                        scalar1=float(OOB_SENTINEL),
                                        scalar2=None, op0=Alu.add)
                nc.vector.tensor_copy(out=offf_c[:], in_=offf[:])

        # ---- compaction: sel_idx[r] = token id with rank r, via selection matmul -
        # S[p, r] = (offf[p, c] == r) is one-hot per rank; tokid_col^T @ S
        # accumulated over the 32 token chunks yields the compacted index row.
        # Output DRAM buffers arrive pre-zeroed (run_bass_via_pjrt donates
        # np.zeros buffers; native run_neff pre-zeros out_maps), so unselected
        # out_row rows stay zero without an explicit fill.
        with ExitStack() as SG:
            sg_pool = SG.enter_context(tc.tile_pool(name="sg", bufs=3))
            sg_psum = SG.enter_context(tc.tile_pool(name="sg_psum", bufs=2, space="PSUM"))
            sel_ps = [sg_psum.tile([1, 512], F32, name="selps") for _ in range(2)]
            for c in range(NT):
                offc = sg_pool.tile([P, 1], F32, name="offc")
                nc.vector.tensor_copy(out=offc[:], in_=offf_c[:, c:c + 1])
                for n in range(2):
                    on = sg_pool.tile([P, 1], F32, name="on")
                    nc.vector.tensor_scalar(out=on[:], in0=offc[:],
                                            scalar1=-float(n * 512), scalar2=None,
                                            op0=Alu.add)
                    smat = sg_pool.tile([P, 512], F32, name="smat")
                    nc.vector.tensor_tensor(out=smat[:], in0=iota512f[:],
                                            in1=on[:, :1].to_broadcast([P, 512]),
                                            op=Alu.is_equal)
                    nc.tensor.matmul(out=sel_ps[n][:], lhsT=tokidf[:, c:c + 1],
                                     rhs=smat[:], start=(c == 0), stop=(c == NT - 1),
                                     skip_group_check=True)
            selrow = sg_pool.tile([1, SEL], F32, name="selrow")
            for n in range(2):
                nc.vector.tensor_copy(out=selrow[:, n * 512:(n + 1) * 512],
                                      in_=sel_ps[n][:])
            nc.sync.dma_start(out=selidx2_d, in_=selrow[:])
            # reload as [P, NSJ] with (p, j) <- rank j*128 + p, cast to int32
            self_sb = sg_pool.tile([P, NSJ], F32, name="self_sb")
            nc.sync.dma_start(
                out=self_sb[:],
                in_=selidx2_d.rearrange("(j p) one -> p (j one)", p=P))
            nc.vector.tensor_copy(out=selidx_sb[:], in_=self_sb[:])

        # ---- gather + transpose + MLP --------------------------------------------
        with ExitStack() as SM:
            ht_pool = SM.enter_context(tc.tile_pool(name="ht", bufs=1))
            ht = ht_pool.tile([P, NM, SEL], BF16)

            with ExitStack() as SB:
                xt_pool = SB.enter_context(tc.tile_pool(name="xt", bufs=1))
                xsel_pool = SB.enter_context(tc.tile_pool(name="xsel", bufs=4))
                tp_psum = SB.enter_context(tc.tile_pool(name="tp_psum", bufs=2, space="PSUM"))
                mm1_psum = SB.enter_context(tc.tile_pool(name="mm1_psum", bufs=6, space="PSUM"))

                xt = []
                for kd in range(ND):
                    xt.append(xt_pool.tile([P, SEL], BF16, name=f"xt_{kd}"))
                for j in range(NSJ):
                    xs = xsel_pool.tile([P, D], F32, name="xsel")
                    nc.gpsimd.indirect_dma_start(
                        out=xs[:], out_offset=None, in_=x_row,
                        in_offset=IndirectOffsetOnAxis(ap=selidx_sb[:, j:j + 1],
                                                       axis=0))
                    for kd in range(ND):
                        tp = tp_psum.tile([P, P], F32, name="tp")
                        nc.tensor.transpose(out=tp[:], in_=xs[:, kd * P:(kd + 1) * P],
                                            identity=ident_sb[:])
                        nc.vector.tensor_copy(out=xt[kd][:, j * P:(j + 1) * P], in_=tp[:])

                # ---- mm1: ht[m, sel] = gelu(w1^T x_sel^T + b1) -------------------
                for n in range(SEL // 512):
                    for m in range(NM):
                        ph = mm1_psum.tile([P, 512], F32, name="ph")
                        for kd in range(ND):
                            nc.tensor.matmul(
                                out=ph[:],
                                lhsT=w1bf[kd][:, m * P:(m + 1) * P],
                                rhs=xt[kd][:, n * 512:(n + 1) * 512],
                                start=(kd == 0), stop=(kd == ND - 1),
                            )
                        nc.scalar.activation(
                            out=ht[:, m, n * 512:(n + 1) * 512], in_=ph[:],
                            func=Act.Gelu_apprx_tanh, bias=b1t_sb[:, m:m + 1], scale=1.0,
                        )

            # ---- mm2: y[sel, D] = ht^T @ w2 + b2 ---------------------------------
            if os.environ.get("KVAR") == "mm1stop":
                SM.close()
                nc.compile()
                return nc
            with ExitStack() as SY:
                y_pool = SY.enter_context(tc.tile_pool(name="y", bufs=1))
                w2_pool = SY.enter_context(tc.tile_pool(name="w2s", bufs=5))
                mm2_psum = SY.enter_context(tc.tile_pool(name="mm2_psum", bufs=8, space="PSUM"))
                y_sb = y_pool.tile([P, NSJ, D], F32)
                for n in range(D // 512):
                    pys = [mm2_psum.tile([P, 512], F32, name="py") for _ in range(NSJ)]
                    for s in range(NSJ):
                        nc.tensor.matmul(
                            out=pys[s][:], lhsT=o1x128b_sb[:],
                            rhs=b2bf_sb[:, n * 512:(n + 1) * 512],
                            start=True, stop=False, skip_group_check=True,
                        )
                    for kg in range(NM // NKGRP):
                        w2t = w2_pool.tile([P, NKGRP, 512], BF16, name="w2t")
                        src = w2[:, n * 512:(n + 1) * 512].rearrange(
                            "(g p) f -> p g f", p=P)[:, kg * NKGRP:(kg + 1) * NKGRP, :]
                        nc.gpsimd.dma_start(out=w2t[:], in_=src)
                        for ki in range(NKGRP):
                            kk = kg * NKGRP + ki
                            for s in range(NSJ):
                                nc.tensor.matmul(
                                    out=pys[s][:],
                                    lhsT=ht[:, kk, s * P:(s + 1) * P],
                                    rhs=w2t[:, ki, :],
                                    start=False, stop=(kk == NM - 1),
                                    skip_group_check=True,
                                )
                    for s in range(NSJ):
                        nc.vector.tensor_copy(
                            out=y_sb[:, s, n * 512:(n + 1) * 512], in_=pys[s][:])

                # ---- scatter y rows into zeroed output ---------------------------
                for j in range(NSJ):
                    nc.gpsimd.indirect_dma_start(
                        out=out_row, out_offset=IndirectOffsetOnAxis(
                            ap=selidx_sb[:, j:j + 1], axis=0),
                        in_=y_sb[:, j, :], in_offset=None,
                    )

    nc.compile()
    return nc


def make_consts():
    q = np.arange(P)
    consts = {
        "ident128": np.eye(P, dtype=np.float32),
        "ltri128": (q[:, None] < q[None, :]).astype(np.float32),  # [q, p] = q < p
        "slt32": (np.arange(NT)[:, None] < np.arange(NT)[None, :]).astype(np.float32),
        "id32": np.eye(NT, dtype=np.float32),
        "ones_1x128": np.ones((1, P), np.float32),
        "ones_128x1": np.ones((P, 1), np.float32),
        "ones_32x128": np.ones((NT, P), np.float32),
    }
    import ml_dtypes
    consts["ones_1x128b"] = np.ones((1, P), ml_dtypes.bfloat16)
    return consts


def make_in_maps(x, W1, b1, W2, b2, wr, br):
    consts = make_consts()
    x = np.ascontiguousarray(np.asarray(x, np.float32))
    in_maps = []
    for c in range(NCORES):
        b, h = divmod(c, 2)
        m = {
            "x_row": x[b],
            "w1": np.asarray(W1, np.float32),
            "w2": np.asarray(W2, np.float32),
            "wr": np.asarray(wr, np.float32).reshape(1, D),
            "br": np.asarray(br, np.float32).reshape(1, 1),
            "b1t": np.ascontiguousarray(np.asarray(b1, np.float32).reshape(NM, P).T),
            "b2": np.asarray(b2, np.float32).reshape(1, D),
            "hbase": np.array([[h * SEL]], np.float32),
        }
        m.update(consts)
        in_maps.append(m)
    return in_maps


_NC_CACHE = None


def _get_program():
    global _NC_CACHE
    if _NC_CACHE is None:
        _NC_CACHE = build_program()
    return _NC_CACHE


def kernel(x, W1, b1, W2, b2, wr, br):
    from concourse.bass_utils import run_bass_kernel_spmd

    nc = _get_program()
    in_maps = make_in_maps(x, W1, b1, W2, b2, wr, br)
    res = run_bass_kernel_spmd(nc, in_maps, list(range(NCORES))).results
    out = np.stack(
        [res[2 * b]["out_row"] + res[2 * b + 1]["out_row"] for b in range(B)]
    )
    return out.astype(np.float32)



# revision 9
# speedup vs baseline: 2.1921x; 2.1921x over previous
"""MoD (mixture-of-depths) MLP wrapper kernel for Trainium2, 8 NeuronCores.

Sharding: core c handles batch row b = c//2 and the half of that row's
top-K tokens with global selection ranks in [h*1024, (h+1)*1024), h = c%2.
Each core computes the full row's router scores + top-K threshold locally
(no collectives), gathers exactly 1024 token rows by rank via indirect DMA,
runs the FFN in bf16 (fp32 accumulation), and scatters results back into the
pre-zeroed per-core output buffer with dma_scatter_add.  Host sums the two
buffers of each row.

v2 schedule: x loads get DMA priority, the top-K threshold search runs as
fused Sign-activation counts + partition_all_reduce, rank compaction uses a
digit-decomposed one-hot matmul (also producing the int16 scatter index
layout directly), and the output scatter is dma_scatter_add (per-index DMA
descriptors) instead of whole-tensor indirect DMA.
"""

import sys

sys.path.insert(0, "/opt/trn_rl_repo")

from contextlib import ExitStack

import numpy as np

from concourse import bass, bass_isa, mybir
from concourse import bacc
import concourse.tile as tile
from concourse.bass import IndirectOffsetOnAxis

B, L, D = 4, 4096, 1024
DFF = 4 * D
K = L // 2              # 2048 selected tokens per row
NCORES = 8
P = 128
NT = L // P             # 32 token tiles per row
SEL = K // 2            # 1024 selected tokens per core
NSJ = SEL // P          # 8 selected-token blocks
ND = D // P             # 8 d chunks
NM = DFF // P           # 32 dff tiles
NKGRP = 4               # w2 k-chunks per streamed tile
RADIX_PASSES = 4

F32 = mybir.dt.float32
BF16 = mybir.dt.bfloat16
I32 = mybir.dt.int32
I16 = mybir.dt.int16
Alu = mybir.AluOpType
Act = mybir.ActivationFunctionType
Red = bass_isa.ReduceOp


def build_program():
    nc = bacc.Bacc(
        "TRN2",
        target_bir_lowering=False,
        debug=False,
        enable_asserts=False,
        num_devices=NCORES,
    )

    x_row = nc.dram_tensor("x_row", [L, D], F32, kind="ExternalInput").ap()
    w1 = nc.dram_tensor("w1", [D, DFF], F32, kind="ExternalInput").ap()
    w2 = nc.dram_tensor("w2", [DFF, D], F32, kind="ExternalInput").ap()
    wr = nc.dram_tensor("wr", [1, D], F32, kind="ExternalInput").ap()
    b1t = nc.dram_tensor("b1t", [P, NM], F32, kind="ExternalInput").ap()
    b2 = nc.dram_tensor("b2", [1, D], F32, kind="ExternalInput").ap()
    hbase = nc.dram_tensor("hbase", [1, 1], F32, kind="ExternalInput").ap()
    identb = nc.dram_tensor("identb", [P, P], BF16, kind="ExternalInput").ap()
    ltri = nc.dram_tensor("ltri128", [P, P], F32, kind="ExternalInput").ap()
    slt32 = nc.dram_tensor("slt32", [NT, NT], F32, kind="ExternalInput").ap()
    id32 = nc.dram_tensor("id32", [NT, NT], F32, kind="ExternalInput").ap()
    ones_1x128 = nc.dram_tensor("ones_1x128", [1, P], F32, kind="ExternalInput").ap()
    ones_1x128b = nc.dram_tensor("ones_1x128b", [1, P], BF16, kind="ExternalInput").ap()
    ones_128x1 = nc.dram_tensor("ones_128x1", [P, 1], F32, kind="ExternalInput").ap()
    ones_32x128 = nc.dram_tensor("ones_32x128", [NT, P], F32, kind="ExternalInput").ap()
    rep16 = nc.dram_tensor("rep16", [16, P], F32, kind="ExternalInput").ap()

    out_row = nc.dram_tensor("out_row", [L, D], F32, kind="ExternalOutput").ap()

    scores_d = nc.dram_tensor("scores_d", [P, NT], F32).ap()

    with tile.TileContext(nc) as tc, ExitStack() as S0:
        const = S0.enter_context(tc.tile_pool(name="const", bufs=1))
        w1_pool = S0.enter_context(tc.tile_pool(name="w1bf", bufs=1))

        def cload(pool, ap, shape, dtype=F32, name=None):
            t = pool.tile(shape, dtype, name=name)
            nc.sync.dma_start(out=t[:], in_=ap)
            return t

        # ---- SP-queue order: wr, o1, hbase FIRST (phase A needs them) -------
        wr_sb = cload(const, wr, [1, D], name="c_wr")
        o1x128_sb = cload(const, ones_1x128, [1, P], name="c_o1")
        hb_sb = cload(const, hbase, [1, 1], name="c_hb")

        # ---- Pool-queue iotas (independent of SP queue) ---------------------
        # big digit-decomposition iota tables live only through phase E
        dig_ctx = tc.tile_pool(name="dig", bufs=1)
        dig = dig_ctx.__enter__()

        iota_i = const.tile([P, 1], I32)
        nc.gpsimd.iota(iota_i[:], pattern=[[1, 1]], base=0, channel_multiplier=1)
        tokid = const.tile([P, NT], I32)
        nc.gpsimd.iota(tokid[:], pattern=[[P, NT]], base=0, channel_multiplier=1)
        iJ128_i = dig.tile([P, NT, 128], I32)
        nc.gpsimd.iota(iJ128_i[:], pattern=[[0, NT], [1, 128]], base=0,
                       channel_multiplier=0)
        iK64_i = dig.tile([P, NT, 64], I32)
        nc.gpsimd.iota(iK64_i[:], pattern=[[0, NT], [1, 64]], base=0,
                       channel_multiplier=0)
        iJ16_i = dig.tile([P, NT, 16], I32)
        nc.gpsimd.iota(iJ16_i[:], pattern=[[0, NT], [1, 16]], base=0,
                       channel_multiplier=0)
        iK8_i = dig.tile([P, NT, 8], I32)
        nc.gpsimd.iota(iK8_i[:], pattern=[[0, NT], [1, 8]], base=0,
                       channel_multiplier=0)
        i7_i = const.tile([P, 7], I32)
        nc.gpsimd.iota(i7_i[:], pattern=[[1, 7]], base=1, channel_multiplier=0)

        iota_f = const.tile([P, 1], F32)
        nc.vector.tensor_copy(out=iota_f[:], in_=iota_i[:])
        tokidf = const.tile([P, NT], F32)
        nc.vector.tensor_copy(out=tokidf[:], in_=tokid[:])
        iJ128f = dig.tile([P, NT, 128], F32)
        nc.vector.tensor_copy(out=iJ128f[:], in_=iJ128_i[:])
        iK64f = dig.tile([P, NT, 64], F32)
        nc.vector.tensor_copy(out=iK64f[:], in_=iK64_i[:])
        iJ16f = dig.tile([P, NT, 16], F32)
        nc.vector.tensor_copy(out=iJ16f[:], in_=iJ16_i[:])
        iK8f = dig.tile([P, NT, 8], F32)
        nc.vector.tensor_copy(out=iK8f[:], in_=iK8_i[:])
        i7f = const.tile([P, 7], F32)
        nc.vector.tensor_copy(out=i7f[:], in_=i7_i[:])
        thr128 = const.tile([P, 7], F32)
        nc.vector.tensor_scalar(out=thr128[:], in0=i7f[:], scalar1=128.0,
                                scalar2=None, op0=Alu.mult)
        thr16 = const.tile([P, 7], F32)
        nc.vector.tensor_scalar(out=thr16[:], in0=i7f[:], scalar1=16.0,
                                scalar2=None, op0=Alu.mult)
        hb_col = const.tile([P, 1], F32)
        nc.gpsimd.partition_broadcast(hb_col[:], hb_sb[:])

        scores_sb = const.tile([P, NT], F32)
        selidx_sb = const.tile([P, NSJ], I32)
        idx16_sb = const.tile([P, SEL // 16], I16)

        misc_psum_ctx = tc.tile_pool(name="misc_psum", bufs=2, space="PSUM")
        misc_psum = misc_psum_ctx.__enter__()

        # ---- phase A: router scores (fp32, exact; router bias dropped — it
        # shifts every score equally so the top-K set is unchanged) ----------
        with ExitStack() as SA:
            apool = SA.enter_context(tc.tile_pool(name="apool", bufs=1))
            xs_pool = SA.enter_context(tc.tile_pool(name="xs", bufs=8))
            junk_pool = SA.enter_context(tc.tile_pool(name="junk", bufs=3))

            wrb = apool.tile([P, D], F32)
            for n in range(D // 512):
                pt = misc_psum.tile([P, 512], F32, name="mp")
                nc.tensor.matmul(out=pt[:], lhsT=o1x128_sb[:],
                                 rhs=wr_sb[:, n * 512:(n + 1) * 512],
                                 start=True, stop=True)
                nc.vector.tensor_copy(out=wrb[:, n * 512:(n + 1) * 512], in_=pt[:])

            for t in range(NT):
                x_t = xs_pool.tile([P, D], F32)
                nc.sync.dma_start(out=x_t[:], in_=x_row[t * P:(t + 1) * P, :])
                prod = junk_pool.tile([P, D], F32, name="prod")
                nc.vector.tensor_tensor(out=prod[:], in0=x_t[:], in1=wrb[:],
                                        op=Alu.mult)
                sink = junk_pool.tile([P, D], BF16, name="sink")
                nc.scalar.activation(out=sink[:], in_=prod[:], func=Act.Identity,
                                     bias=0.0, scale=1.0,
                                     accum_out=scores_sb[:, t:t + 1])

        # ---- remaining small consts, then w1, on the SP queue ---------------
        b1t_sb = cload(const, b1t, [P, NM], name="c_b1t")
        identb_sb = cload(const, identb, [P, P], BF16, name="c_id")
        ltri_sb = cload(const, ltri, [P, P], name="c_lt")
        slt32_sb = cload(const, slt32, [NT, NT], name="c_sl")
        id32_sb = cload(const, id32, [NT, NT], name="c_id32")
        o1x128b_sb = cload(const, ones_1x128b, [1, P], BF16, name="c_o1b")
        o128x1_sb = cload(const, ones_128x1, [P, 1], name="c_oc")
        o32x128_sb = cload(const, ones_32x128, [NT, P], name="c_o32")
        rep16_sb = cload(const, rep16, [16, P], name="c_rep16")
        b2bf_sb = const.tile([1, D], BF16)
        nc.gpsimd.dma_start(out=b2bf_sb[:], in_=b2)  # cast f32 -> bf16

        offf_c = const.tile([P, NT], F32)
        maskf_c = const.tile([P, NT], F32)

        # ---- phases B+C+D: replicate scores, radix threshold, rank ----------
        with ExitStack() as SC:
            radix = SC.enter_context(tc.tile_pool(name="radix", bufs=2))
            rep_pool = SC.enter_context(tc.tile_pool(name="rep", bufs=1))

            nc.sync.dma_start(out=scores_d, in_=scores_sb[:])
            scores_row = rep_pool.tile([1, L], F32)
            nc.sync.dma_start(out=scores_row[:],
                              in_=scores_d.rearrange("p c -> () (p c)"))
            scores_rep = rep_pool.tile([P, L], F32)
            for n in range(L // 512):
                pt = misc_psum.tile([P, 512], F32, name="mp")
                nc.tensor.matmul(out=pt[:], lhsT=o1x128_sb[:],
                                 rhs=scores_row[:, n * 512:(n + 1) * 512],
                                 start=True, stop=True)
                nc.vector.tensor_copy(out=scores_rep[:, n * 512:(n + 1) * 512],
                                      in_=pt[:])

            lo = radix.tile([P, 1], F32, name="lo")
            nc.vector.memset(lo[:], -16.0)
            w_ = radix.tile([P, 1], F32, name="w")
            nc.vector.memset(w_[:], 32.0 / P)
            thrb = radix.tile([P, 1], F32, name="thrb")
            nc.vector.tensor_scalar(out=thrb[:], in0=iota_f[:], scalar1=32.0 / P,
                                    scalar2=None, op0=Alu.mult)
            sjunk = rep_pool.tile([P, L], BF16, name="sjunk")
            for _pass in range(RADIX_PASSES):
                thr = radix.tile([P, 1], F32, name="thr")
                nc.vector.tensor_tensor(out=thr[:], in0=thrb[:], in1=lo[:],
                                        op=Alu.add)
                nthr = radix.tile([P, 1], F32, name="nthr")
                nc.vector.tensor_scalar(out=nthr[:], in0=thr[:], scalar1=-1.0,
                                        scalar2=None, op0=Alu.mult)
                sgn = radix.tile([P, 1], F32, name="sgn")
                nc.scalar.activation(out=sjunk[:], in_=scores_rep[:],
                                     func=Act.Sign, bias=nthr[:, :1], scale=1.0,
                                     accum_out=sgn[:])
                sel = radix.tile([P, 1], F32, name="sel")
                nc.vector.tensor_scalar(out=sel[:], in0=sgn[:], scalar1=0.0,
                                        scalar2=None, op0=Alu.is_ge)
                s_all = radix.tile([P, 1], F32, name="s_all")
                nc.gpsimd.partition_all_reduce(s_all[:], sel[:], channels=P,
                                               reduce_op=Red.add)
                ps = radix.tile([P, 1], F32, name="ps")
                nc.vector.tensor_scalar(out=ps[:], in0=s_all[:], scalar1=-1.0,
                                        scalar2=None, op0=Alu.add)
                d_ = radix.tile([P, 1], F32, name="d_")
                nc.vector.tensor_tensor(out=d_[:], in0=ps[:], in1=w_[:],
                                        op=Alu.mult)
                lo2 = radix.tile([P, 1], F32, name="lo2")
                nc.vector.tensor_tensor(out=lo2[:], in0=lo[:], in1=d_[:],
                                        op=Alu.add)
                w2_ = radix.tile([P, 1], F32, name="w2_")
                nc.vector.tensor_scalar(out=w2_[:], in0=w_[:], scalar1=1.0 / P,
                                        scalar2=None, op0=Alu.mult)
                thrb2 = radix.tile([P, 1], F32, name="thrb2")
                nc.vector.tensor_scalar(out=thrb2[:], in0=thrb[:], scalar1=1.0 / P,
                                        scalar2=None, op0=Alu.mult)
                lo, w_, thrb = lo2, w2_, thrb2

            # ---- mask + global rank (exclusive prefix of mask) --------------
            maskf = radix.tile([P, NT], F32, name="maskf")
            nc.vector.tensor_tensor(out=maskf[:], in0=scores_sb[:],
                                    in1=lo[:, :1].to_broadcast([P, NT]),
                                    op=Alu.is_ge)
            colsum_p = misc_psum.tile([NT, 1], F32, name="mp")
            nc.tensor.matmul(out=colsum_p[:], lhsT=maskf[:], rhs=o128x1_sb[:],
                             start=True, stop=True)
            colsum = radix.tile([NT, 1], F32, name="colsum")
            nc.vector.tensor_copy(out=colsum[:], in_=colsum_p[:])
            excl_p = misc_psum.tile([NT, 1], F32, name="mp")
            nc.tensor.matmul(out=excl_p[:], lhsT=slt32_sb[:], rhs=colsum[:],
                             start=True, stop=True)
            excl = radix.tile([NT, 1], F32, name="excl")
            nc.vector.tensor_copy(out=excl[:], in_=excl_p[:])
            diag = radix.tile([NT, NT], F32, name="diag")
            nc.vector.tensor_tensor(out=diag[:], in0=id32_sb[:],
                                    in1=excl[:, :1].to_broadcast([NT, NT]),
                                    op=Alu.mult)
            rank_p = misc_psum.tile([P, NT], F32, name="mp")
            nc.tensor.matmul(out=rank_p[:], lhsT=ltri_sb[:], rhs=maskf[:],
                             start=True, stop=False, skip_group_check=True)
            nc.tensor.matmul(out=rank_p[:], lhsT=o32x128_sb[:], rhs=diag[:],
                             start=False, stop=True, skip_group_check=True)
            rank = radix.tile([P, NT], F32, name="rank")
            nc.vector.tensor_copy(out=rank[:], in_=rank_p[:])
            off = radix.tile([P, NT], F32, name="off")
            nc.vector.tensor_tensor(out=off[:], in0=rank[:],
                                    in1=hb_col[:, :1].to_broadcast([P, NT]),
                                    op=Alu.subtract)
            nc.vector.tensor_copy(out=offf_c[:], in_=off[:])
            nc.vector.tensor_copy(out=maskf_c[:], in_=maskf[:])

        misc_psum_ctx.__exit__(None, None, None)

        # ---- w1 cast-loads on the Pool queue.  Positioned after the radix
        # all_reduces so the in-order queue starts them only ~70us in, after
        # the x-tile DMAs have drained (they'd otherwise steal DMA bandwidth
        # from the critical-path score loads). ---------------------------------
        w1bf = []
        for kd in range(ND):
            t_ = w1_pool.tile([P, DFF], BF16, name=f"w1bf_{kd}")
            nc.gpsimd.dma_start(out=t_[:], in_=w1[kd * P:(kd + 1) * P, :])
            w1bf.append(t_)

        # ---- phase E: digit split + one-hot compaction matmuls --------------
        # off in [0, SEL) for in-window selected tokens; any other off value
        # (negative rank-window miss, >=SEL, or collision of an unselected
        # token) produces no match in the lo-digit equality below, and
        # unselected tokens are additionally zeroed via tokid*mask weights.
        with ExitStack() as SE:
            ep = SE.enter_context(tc.tile_pool(name="epool", bufs=1))
            e_psum = SE.enter_context(tc.tile_pool(name="e_psum", bufs=2,
                                                   space="PSUM"))
            off = offf_c
            eq7a = ep.tile([P, NT, 7], F32, name="eq7a")
            nc.vector.tensor_tensor(
                out=eq7a[:], in0=off[:, :, None].to_broadcast([P, NT, 7]),
                in1=thr128[:, None, :].to_broadcast([P, NT, 7]), op=Alu.is_ge)
            hi128 = ep.tile([P, NT], F32, name="hi128")
            nc.vector.tensor_reduce(out=hi128[:], in_=eq7a[:],
                                    axis=mybir.AxisListType.X, op=Alu.add)
            hm = ep.tile([P, NT], F32, name="hm")
            nc.vector.tensor_scalar(out=hm[:], in0=hi128[:], scalar1=-128.0,
                                    scalar2=None, op0=Alu.mult)
            lo128 = ep.tile([P, NT], F32, name="lo128")
            nc.vector.tensor_tensor(out=lo128[:], in0=off[:], in1=hm[:],
                                    op=Alu.add)
            eq7b = ep.tile([P, NT, 7], F32, name="eq7b")
            nc.vector.tensor_tensor(
                out=eq7b[:], in0=lo128[:, :, None].to_broadcast([P, NT, 7]),
                in1=thr16[:, None, :].to_broadcast([P, NT, 7]), op=Alu.is_ge)
            mid = ep.tile([P, NT], F32, name="mid")
            nc.vector.tensor_reduce(out=mid[:], in_=eq7b[:],
                                    axis=mybir.AxisListType.X, op=Alu.add)
            hm2 = ep.tile([P, NT], F32, name="hm2")
            nc.vector.tensor_scalar(out=hm2[:], in0=mid[:], scalar1=-16.0,
                                    scalar2=None, op0=Alu.mult)
            lo16 = ep.tile([P, NT], F32, name="lo16")
            nc.vector.tensor_tensor(out=lo16[:], in0=lo128[:], in1=hm2[:],
                                    op=Alu.add)
            h8 = ep.tile([P, NT], F32, name="h8")
            nc.vector.tensor_scalar(out=h8[:], in0=hi128[:], scalar1=8.0,
                                    scalar2=None, op0=Alu.mult)
            hi16 = ep.tile([P, NT], F32, name="hi16")
            nc.vector.tensor_tensor(out=hi16[:], in0=h8[:], in1=mid[:],
                                    op=Alu.add)
            tokw = ep.tile([P, NT], F32, name="tokw")
            nc.vector.tensor_tensor(out=tokw[:], in0=tokidf[:], in1=maskf_c[:],
                                    op=Alu.mult)

            eq128 = ep.tile([P, NT, 128], F32, name="eq128")
            nc.vector.tensor_tensor(
                out=eq128[:], in0=iJ128f[:],
                in1=lo128[:, :, None].to_broadcast([P, NT, 128]),
                op=Alu.is_equal)
            w128 = ep.tile([P, NT, 128], F32, name="w128")
            nc.vector.tensor_tensor(
                out=w128[:], in0=eq128[:],
                in1=tokw[:, :, None].to_broadcast([P, NT, 128]), op=Alu.mult)
            eq8 = ep.tile([P, NT, 8], F32, name="eq8")
            nc.vector.tensor_tensor(
                out=eq8[:], in0=iK8f[:],
                in1=hi128[:, :, None].to_broadcast([P, NT, 8]), op=Alu.is_equal)
            eq16 = ep.tile([P, NT, 16], F32, name="eq16")
            nc.vector.tensor_tensor(
                out=eq16[:], in0=iJ16f[:],
                in1=lo16[:, :, None].to_broadcast([P, NT, 16]), op=Alu.is_equal)
            w16 = ep.tile([P, NT, 16], F32, name="w16")
            nc.vector.tensor_tensor(
                out=w16[:], in0=eq16[:],
                in1=tokw[:, :, None].to_broadcast([P, NT, 16]), op=Alu.mult)
            eq64 = ep.tile([P, NT, 64], F32, name="eq64")
            nc.vector.tensor_tensor(
                out=eq64[:], in0=iK64f[:],
                in1=hi16[:, :, None].to_broadcast([P, NT, 64]), op=Alu.is_equal)

            selps = e_psum.tile([P, NSJ], F32, name="selps")
            sel16ps = e_psum.tile([16, 64], F32, name="sel16ps")
            for c in range(NT):
                nc.tensor.matmul(out=selps[:], lhsT=w128[:, c, :],
                                 rhs=eq8[:, c, :], start=(c == 0),
                                 stop=(c == NT - 1), skip_group_check=True)
            for c in range(NT):
                nc.tensor.matmul(out=sel16ps[:], lhsT=w16[:, c, :],
                                 rhs=eq64[:, c, :], start=(c == 0),
                                 stop=(c == NT - 1), skip_group_check=True)

            nc.vector.tensor_copy(out=selidx_sb[:], in_=selps[:])  # f32->i32
            sel16_sb = ep.tile([16, 64], F32, name="sel16_sb")
            nc.vector.tensor_copy(out=sel16_sb[:], in_=sel16ps[:])
            rep_ps = e_psum.tile([P, 64], F32, name="rep_ps")
            nc.tensor.matmul(out=rep_ps[:], lhsT=rep16_sb[:], rhs=sel16_sb[:],
                             start=True, stop=True)
            nc.vector.tensor_copy(out=idx16_sb[:], in_=rep_ps[:])  # f32->i16

        dig_ctx.__exit__(None, None, None)

        # ---- gather + transpose + MLP ---------------------------------------
        with ExitStack() as SM:
            ht_pool = SM.enter_context(tc.tile_pool(name="ht", bufs=1))
            ht = ht_pool.tile([P, NM, SEL], BF16)

            with ExitStack() as SB:
                xt_pool = SB.enter_context(tc.tile_pool(name="xt", bufs=1))
                xsel_pool = SB.enter_context(tc.tile_pool(name="xsel", bufs=3))
                tp_psum = SB.enter_context(tc.tile_pool(name="tp_psum", bufs=2,
                                                        space="PSUM"))
                mm1_psum = SB.enter_context(tc.tile_pool(name="mm1_psum", bufs=6,
                                                         space="PSUM"))

                xt = []
                for kd in range(ND):
                    xt.append(xt_pool.tile([P, SEL], BF16, name=f"xt_{kd}"))
                for j in range(NSJ):
                    xs = xsel_pool.tile([P, D], BF16, name="xsel")
                    nc.gpsimd.indirect_dma_start(
                        out=xs[:], out_offset=None, in_=x_row,
                        in_offset=IndirectOffsetOnAxis(ap=selidx_sb[:, j:j + 1],
                                                       axis=0))
                    for kd in range(ND):
                        tp = tp_psum.tile([P, P], BF16, name="tp")
                        nc.tensor.transpose(out=tp[:],
                                            in_=xs[:, kd * P:(kd + 1) * P],
                                            identity=identb_sb[:])
                        if (j + kd) % 2 == 0:
                            nc.vector.tensor_copy(
                                out=xt[kd][:, j * P:(j + 1) * P], in_=tp[:])
                        else:
                            nc.scalar.activation(
                                out=xt[kd][:, j * P:(j + 1) * P], in_=tp[:],
                                func=Act.Copy, bias=0.0, scale=1.0)

                # ---- mm1: ht[m, sel] = gelu(w1^T x_sel^T + b1) ---------------
                for n in range(SEL // 512):
                    for m in range(NM):
                        ph = mm1_psum.tile([P, 512], F32, name="ph")
                        for kd in range(ND):
                            nc.tensor.matmul(
                                out=ph[:],
                                lhsT=w1bf[kd][:, m * P:(m + 1) * P],
                                rhs=xt[kd][:, n * 512:(n + 1) * 512],
                                start=(kd == 0), stop=(kd == ND - 1),
                            )
                        nc.scalar.activation(
                            out=ht[:, m, n * 512:(n + 1) * 512], in_=ph[:],
                            func=Act.Gelu_apprx_tanh, bias=b1t_sb[:, m:m + 1],
                            scale=1.0,
                        )

            # ---- mm2: y[sel, D] = ht^T @ w2 + b2, then scatter-add ----------
            with ExitStack() as SY:
                y_pool = SY.enter_context(tc.tile_pool(name="y", bufs=1))
                w2_pool = SY.enter_context(tc.tile_pool(name="w2s", bufs=6))
                mm2_psum = SY.enter_context(tc.tile_pool(name="mm2_psum", bufs=8,
                                                         space="PSUM"))
                for n in range(D // 512):
                    y_n = y_pool.tile([P, NSJ, 512], F32, name=f"y{n}")
                    pys = [mm2_psum.tile([P, 512], F32, name="py")
                           for _ in range(NSJ)]
                    for s in range(NSJ):
                        nc.tensor.matmul(
                            out=pys[s][:], lhsT=o1x128b_sb[:],
                            rhs=b2bf_sb[:, n * 512:(n + 1) * 512],
                            start=True, stop=False, skip_group_check=True,
                        )
                    for kg in range(NM // NKGRP):
                        w2t = w2_pool.tile([P, NKGRP, 512], BF16, name="w2t")
                        src = w2[:, n * 512:(n + 1) * 512].rearrange(
                            "(g p) f -> p g f", p=P)[:, kg * NKGRP:(kg + 1) * NKGRP, :]
                        nc.gpsimd.dma_start(out=w2t[:], in_=src)
                        for ki in range(NKGRP):
                            kk = kg * NKGRP + ki
                            for s in range(NSJ):
                                nc.tensor.matmul(
                                    out=pys[s][:],
                                    lhsT=ht[:, kk, s * P:(s + 1) * P],
                                    rhs=w2t[:, ki, :],
                                    start=False, stop=(kk == NM - 1),
                                    skip_group_check=True,
                                )
                    for s in range(NSJ):
                        nc.scalar.activation(out=y_n[:, s, :], in_=pys[s][:],
                                             func=Act.Copy, bias=0.0, scale=1.0)
                    # scatter this d-half: out_row[idx[r], n*512:(n+1)*512] += y_n[r]
                    nc.gpsimd.dma_scatter_add(
                        out_row[:, n * 512:(n + 1) * 512],
                        y_n[:],
                        idx16_sb[:],
                        SEL,
                        SEL,
                        512,
                        elem_step=D,
                    )

    nc.compile()
    return nc


def make_consts():
    q = np.arange(P)
    import ml_dtypes
    consts = {
        "identb": np.eye(P, dtype=ml_dtypes.bfloat16),
        "ltri128": (q[:, None] < q[None, :]).astype(np.float32),  # [q, p] = q < p
        "slt32": (np.arange(NT)[:, None] < np.arange(NT)[None, :]).astype(np.float32),
        "id32": np.eye(NT, dtype=np.float32),
        "ones_1x128": np.ones((1, P), np.float32),
        "ones_1x128b": np.ones((1, P), ml_dtypes.bfloat16),
        "ones_128x1": np.ones((P, 1), np.float32),
        "ones_32x128": np.ones((NT, P), np.float32),
        "rep16": (np.arange(16)[:, None] == (np.arange(P)[None, :] % 16)
                  ).astype(np.float32),
    }
    return consts


def make_in_maps(x, W1, b1, W2, b2, wr, br):
    consts = make_consts()
    x = np.ascontiguousarray(np.asarray(x, np.float32))
    in_maps = []
    for c in range(NCORES):
        b, h = divmod(c, 2)
        m = {
            "x_row": x[b],
            "w1": np.asarray(W1, np.float32),
            "w2": np.asarray(W2, np.float32),
            "wr": np.asarray(wr, np.float32).reshape(1, D),
            "b1t": np.ascontiguousarray(np.asarray(b1, np.float32).reshape(NM, P).T),
            "b2": np.asarray(b2, np.float32).reshape(1, D),
            "hbase": np.array([[h * SEL]], np.float32),
        }
        m.update(consts)
        in_maps.append(m)
    return in_maps


_NC_CACHE = None


def _get_program():
    global _NC_CACHE
    if _NC_CACHE is None:
        _NC_CACHE = build_program()
    return _NC_CACHE


def kernel(x, W1, b1, W2, b2, wr, br):
    from concourse.bass_utils import run_bass_kernel_spmd

    nc = _get_program()
    in_maps = make_in_maps(x, W1, b1, W2, b2, wr, br)
    res = run_bass_kernel_spmd(nc, in_maps, list(range(NCORES))).results
    out = np.stack(
        [res[2 * b]["out_row"] + res[2 * b + 1]["out_row"] for b in range(B)]
    )
    return out.astype(np.float32)


# revision 35
# speedup vs baseline: 2.3953x; 1.0927x over previous
"""MoD (mixture-of-depths) MLP wrapper kernel for Trainium2, 8 NeuronCores.

Sharding: core c handles batch row b = c//2 and the half of that row's
top-K tokens with global selection ranks in [h*1024, (h+1)*1024), h = c%2.
Each core computes the full row's router scores + top-K threshold locally
(no collectives), gathers exactly 1024 token rows by rank via indirect DMA,
runs the FFN in bf16 (fp32 accumulation), and scatters results back into the
pre-zeroed per-core output buffer with dma_scatter_add.  Host sums the two
buffers of each row.

v2 schedule: x loads get DMA priority, the top-K threshold search runs as
fused Sign-activation counts + partition_all_reduce, rank compaction uses a
digit-decomposed one-hot matmul (also producing the int16 scatter index
layout directly), and the output scatter is dma_scatter_add (per-index DMA
descriptors) instead of whole-tensor indirect DMA.
"""

import sys

sys.path.insert(0, "/opt/trn_rl_repo")

from contextlib import ExitStack

import numpy as np

from concourse import bass, bass_isa, mybir
from concourse import bacc
import concourse.tile as tile
from concourse.bass import IndirectOffsetOnAxis

B, L, D = 4, 4096, 1024
DFF = 4 * D
K = L // 2              # 2048 selected tokens per row
NCORES = 8
P = 128
NT = L // P             # 32 token tiles per row
SEL = K // 2            # 1024 selected tokens per core
NSJ = SEL // P          # 8 selected-token blocks
ND = D // P             # 8 d chunks
NM = DFF // P           # 32 dff tiles
NKGRP = 4               # w2 k-chunks per streamed tile
RADIX_PASSES = 4

F32 = mybir.dt.float32
BF16 = mybir.dt.bfloat16
I32 = mybir.dt.int32
I16 = mybir.dt.int16
Alu = mybir.AluOpType
Act = mybir.ActivationFunctionType
Red = bass_isa.ReduceOp


def build_program():
    nc = bacc.Bacc(
        "TRN2",
        target_bir_lowering=False,
        debug=False,
        enable_asserts=False,
        num_devices=NCORES,
    )

    x_row = nc.dram_tensor("x_row", [L, D], F32, kind="ExternalInput").ap()
    w1 = nc.dram_tensor("w1", [D, DFF], F32, kind="ExternalInput").ap()
    w2 = nc.dram_tensor("w2", [DFF, D], F32, kind="ExternalInput").ap()
    wr = nc.dram_tensor("wr", [1, D], F32, kind="ExternalInput").ap()
    b1t = nc.dram_tensor("b1t", [P, NM], F32, kind="ExternalInput").ap()
    b2 = nc.dram_tensor("b2", [1, D], F32, kind="ExternalInput").ap()
    hbase = nc.dram_tensor("hbase", [1, 1], F32, kind="ExternalInput").ap()
    identb = nc.dram_tensor("identb", [P, P], BF16, kind="ExternalInput").ap()
    ltri = nc.dram_tensor("ltri128", [P, P], F32, kind="ExternalInput").ap()
    slt32 = nc.dram_tensor("slt32", [NT, NT], F32, kind="ExternalInput").ap()
    id32 = nc.dram_tensor("id32", [NT, NT], F32, kind="ExternalInput").ap()
    ones_1x128 = nc.dram_tensor("ones_1x128", [1, P], F32, kind="ExternalInput").ap()
    ones_1x128b = nc.dram_tensor("ones_1x128b", [1, P], BF16, kind="ExternalInput").ap()
    ones_128x1 = nc.dram_tensor("ones_128x1", [P, 1], F32, kind="ExternalInput").ap()
    ones_32x128 = nc.dram_tensor("ones_32x128", [NT, P], F32, kind="ExternalInput").ap()
    rep16 = nc.dram_tensor("rep16", [16, P], F32, kind="ExternalInput").ap()
    ewrap = nc.dram_tensor("ewrap", [16, 8 * P], F32, kind="ExternalInput").ap()

    out_row = nc.dram_tensor("out_row", [L, D], F32, kind="ExternalOutput").ap()

    scores_d2 = nc.dram_tensor("scores_d2", [NT, P], F32).ap()

    with tile.TileContext(nc) as tc, ExitStack() as S0:
        const = S0.enter_context(tc.tile_pool(name="const", bufs=1))
        # pool stack (LIFO): const | ht | w1 | dig | ...phases
        ht_ctx = tc.tile_pool(name="ht", bufs=1)
        ht_pool = ht_ctx.__enter__()
        ht = ht_pool.tile([P, NM, SEL], BF16)
        w1_ctx = tc.tile_pool(name="w1bf", bufs=1)
        w1_pool = w1_ctx.__enter__()

        def cload(pool, ap, shape, dtype=F32, name=None):
            t = pool.tile(shape, dtype, name=name)
            nc.sync.dma_start(out=t[:], in_=ap)
            return t

        # ---- SP-queue order: wr, o1, oc, hbase FIRST (phase A needs them) ---
        wr_sb = cload(const, wr, [1, D], name="c_wr")
        o1x128_sb = cload(const, ones_1x128, [1, P], name="c_o1")
        o128x1_sb = cload(const, ones_128x1, [P, 1], name="c_oc")
        hb_sb = cload(const, hbase, [1, 1], name="c_hb")

        # w1 tiles exist from the start (loads are issued after the radix)
        w1bf = [w1_pool.tile([P, DFF], BF16, name=f"w1bf_{kd}")
                for kd in range(ND)]

        # ---- Pool-queue iotas (independent of SP queue) ---------------------
        # big digit-decomposition iota tables live only through phase E
        dig_ctx = tc.tile_pool(name="dig", bufs=1)
        dig = dig_ctx.__enter__()

        iota_i = const.tile([P, 1], I32)
        nc.gpsimd.iota(iota_i[:], pattern=[[1, 1]], base=0, channel_multiplier=1)
        tokid = const.tile([P, NT], I32)
        nc.gpsimd.iota(tokid[:], pattern=[[P, NT]], base=0, channel_multiplier=1)
        iC_i = const.tile([P, NT], I32)
        nc.gpsimd.iota(iC_i[:], pattern=[[1, NT]], base=0, channel_multiplier=0)
        iQ_i = const.tile([P, 128], I32)
        nc.gpsimd.iota(iQ_i[:], pattern=[[1, 128]], base=0, channel_multiplier=0)
        iK64_i = dig.tile([P, NT, 64], I16)
        nc.gpsimd.iota(iK64_i[:], pattern=[[0, NT], [1, 64]], base=0,
                       channel_multiplier=0)
        iJ16_i = dig.tile([P, NT, 16], I16)
        nc.gpsimd.iota(iJ16_i[:], pattern=[[0, NT], [1, 16]], base=0,
                       channel_multiplier=0)
        i7_i = const.tile([P, 7], I32)
        nc.gpsimd.iota(i7_i[:], pattern=[[1, 7]], base=1, channel_multiplier=0)

        iota_f = const.tile([P, 1], F32)
        nc.vector.tensor_copy(out=iota_f[:], in_=iota_i[:])
        tokidf = const.tile([P, NT], F32)
        nc.vector.tensor_copy(out=tokidf[:], in_=tokid[:])
        cvalf = const.tile([P, NT], F32)
        nc.vector.tensor_copy(out=cvalf[:], in_=iC_i[:])
        iK64f = dig.tile([P, NT, 64], F32)
        nc.vector.tensor_copy(out=iK64f[:], in_=iK64_i[:])
        iJ16f = dig.tile([P, NT, 16], F32)
        nc.vector.tensor_copy(out=iJ16f[:], in_=iJ16_i[:])
        i7f = const.tile([P, 7], F32)
        nc.vector.tensor_copy(out=i7f[:], in_=i7_i[:])
        thr128 = const.tile([P, 7], F32)
        nc.vector.tensor_scalar(out=thr128[:], in0=i7f[:], scalar1=128.0,
                                scalar2=None, op0=Alu.mult)
        thr16 = const.tile([P, 7], F32)
        nc.vector.tensor_scalar(out=thr16[:], in0=i7f[:], scalar1=16.0,
                                scalar2=None, op0=Alu.mult)
        # radix pass-1 threshold grid (build-time constants: lo=-16, w=0.25)
        iQf = const.tile([P, 128], F32)
        nc.vector.tensor_copy(out=iQf[:], in_=iQ_i[:])
        thr1row = const.tile([P, 128], F32)
        nc.vector.tensor_scalar(out=thr1row[:], in0=iQf[:], scalar1=32.0 / P,
                                scalar2=-16.0, op0=Alu.mult, op1=Alu.add)
        # negated per-pass threshold offsets for radix passes 2..4
        W1P = 32.0 / P
        nthrbs = []
        for p_ in range(1, RADIX_PASSES):
            w_p = W1P / (P ** p_)
            t_ = const.tile([P, 1], F32, name=f"nthrb{p_}")
            nc.vector.tensor_scalar(out=t_[:], in0=iota_f[:], scalar1=-w_p,
                                    scalar2=None, op0=Alu.mult)
            nthrbs.append((w_p, t_))
        hb_col = const.tile([P, 1], F32)
        nc.gpsimd.partition_broadcast(hb_col[:], hb_sb[:])

        scores_sb = const.tile([P, NT], F32)
        selidx_sb = const.tile([P, NSJ], I32)
        idx16_sb = const.tile([P, SEL // 16], I16)

        misc_psum_ctx = tc.tile_pool(name="misc_psum", bufs=2, space="PSUM")
        misc_psum = misc_psum_ctx.__enter__()

        # ---- phase A: router scores (fp32, exact; router bias dropped — it
        # shifts every score equally so the top-K set is unchanged).  The
        # first radix pass uses a build-time-constant threshold grid, so its
        # per-tile compare + count-matmul accumulation is folded in here. -----
        c1_psum_ctx = tc.tile_pool(name="c1_psum", bufs=1, space="PSUM")
        c1_psum = c1_psum_ctx.__enter__()
        cnt1_ps = c1_psum.tile([1, 128], F32, name="cnt1")
        nlo = const.tile([P, 1], F32, name="nlo")
        with ExitStack() as SA:
            apool = SA.enter_context(tc.tile_pool(name="apool", bufs=1))
            xs_pool = SA.enter_context(tc.tile_pool(name="xs", bufs=6))
            junk_pool = SA.enter_context(tc.tile_pool(name="junk", bufs=2))
            cmp_pool = SA.enter_context(tc.tile_pool(name="cmp", bufs=3))

            wrb = apool.tile([P, D], F32)
            for n in range(D // 512):
                pt = misc_psum.tile([P, 512], F32, name="mp")
                nc.tensor.matmul(out=pt[:], lhsT=o1x128_sb[:],
                                 rhs=wr_sb[:, n * 512:(n + 1) * 512],
                                 start=True, stop=True)
                nc.vector.tensor_copy(out=wrb[:, n * 512:(n + 1) * 512], in_=pt[:])

            x_last = None
            for t in range(NT):
                x_t = xs_pool.tile([P, D], F32)
                nc.sync.dma_start(out=x_t[:], in_=x_row[t * P:(t + 1) * P, :])
                x_last = x_t
                prod = junk_pool.tile([P, D], F32, name="prod")
                nc.vector.tensor_tensor(out=prod[:], in0=x_t[:], in1=wrb[:],
                                        op=Alu.mult)
                sink = junk_pool.tile([P, D], BF16, name="sink")
                nc.scalar.activation(out=sink[:], in_=prod[:], func=Act.Identity,
                                     bias=0.0, scale=1.0,
                                     accum_out=scores_sb[:, t:t + 1])
                cmp_t = cmp_pool.tile([P, 128], F32, name="cmp")
                nc.vector.tensor_tensor(
                    out=cmp_t[:],
                    in0=scores_sb[:, t:t + 1].to_broadcast([P, 128]),
                    in1=thr1row[:], op=Alu.is_ge)
                nc.tensor.matmul(out=cnt1_ps[:], lhsT=o128x1_sb[:], rhs=cmp_t[:],
                                 start=(t == 0), stop=(t == NT - 1),
                                 skip_group_check=True)
                # stagger the token-major score spill per 8-tile group (on the
                # Act queue so it slots between accums without stalling x DMAs)
                if t % 8 == 7:
                    g = t // 8
                    nc.scalar.dma_start(
                        out=scores_d2[g * 8:(g + 1) * 8, :].rearrange("c p -> p c"),
                        in_=scores_sb[:, g * 8:(g + 1) * 8])

            # pass-1 finalize: nlo = -(lo1) = 16 - (sum(cnt>=K) - 1)*0.25
            cnt1 = apool.tile([1, 128], F32, name="cnt1_sb")
            nc.vector.tensor_copy(out=cnt1[:], in_=cnt1_ps[:])
            selr = apool.tile([1, 128], F32, name="selr")
            nc.vector.tensor_scalar(out=selr[:], in0=cnt1[:], scalar1=float(K),
                                    scalar2=None, op0=Alu.is_ge)
            s1 = apool.tile([1, 1], F32, name="s1")
            nc.vector.tensor_reduce(out=s1[:], in_=selr[:],
                                    axis=mybir.AxisListType.X, op=Alu.add)
            q1 = apool.tile([1, 1], F32, name="q1")
            nc.vector.tensor_scalar(out=q1[:], in0=s1[:], scalar1=-1.0,
                                    scalar2=-W1P, op0=Alu.add, op1=Alu.mult)
            nlo11 = apool.tile([1, 1], F32, name="nlo11")
            nc.vector.tensor_scalar(out=nlo11[:], in0=q1[:], scalar1=16.0,
                                    scalar2=None, op0=Alu.add)
            nc.gpsimd.partition_broadcast(nlo[:], nlo11[:])
        c1_psum_ctx.__exit__(None, None, None)

        offf_c = const.tile([P, NT], F32)
        maskf_c = const.tile([P, NT], F32)

        # ---- phases B+C+D: replicate scores, radix threshold, rank ----------
        with ExitStack() as SC:
            radix = SC.enter_context(tc.tile_pool(name="radix", bufs=2))
            rep_pool = SC.enter_context(tc.tile_pool(name="rep", bufs=1))

            # broadcast-read the spilled scores, one DMA per 1024-token group,
            # FIRST on the in-order SP queue right after the x loads (the
            # remaining const loads queue behind, they aren't needed till later)
            scores_rep = rep_pool.tile([P, L], F32)
            for g in range(4):
                nc.sync.dma_start(
                    out=scores_rep[:, g * 1024:(g + 1) * 1024],
                    in_=scores_d2.rearrange("c p -> () (c p)")
                    [:, g * 1024:(g + 1) * 1024].to_broadcast([P, 1024]))

            # gate the w1 cast-loads behind the score broadcast so their DMAs
            # cannot delay it (WAW edge: the w1 DMA overwrites the gate byte)
            for kd in range(ND):
                nc.vector.tensor_copy(out=w1bf[kd][0:1, 0:1],
                                      in_=scores_rep[0:1, kd:kd + 1])

            # ---- remaining small consts on the SP queue ---------------------
            b1t_sb = cload(const, b1t, [P, NM], name="c_b1t")
            identb_sb = cload(const, identb, [P, P], BF16, name="c_id")
            ltri_sb = cload(const, ltri, [P, P], name="c_lt")
            slt32_sb = cload(const, slt32, [NT, NT], name="c_sl")
            id32_sb = cload(const, id32, [NT, NT], name="c_id32")
            o1x128b_sb = cload(const, ones_1x128b, [1, P], BF16, name="c_o1b")
            o32x128_sb = cload(const, ones_32x128, [NT, P], name="c_o32")
            rep16_sb = cload(const, rep16, [16, P], name="c_rep16")
            ewrap_sb = cload(const, ewrap, [16, 8 * P], name="c_ew")
            b2bf_sb = const.tile([1, D], BF16)
            nc.gpsimd.dma_start(out=b2bf_sb[:], in_=b2)  # cast f32 -> bf16

            sjunk = rep_pool.tile([P, L], BF16, name="sjunk")
            for w_p, nthrb_p in nthrbs:
                nthr = radix.tile([P, 1], F32, name="nthr")
                nc.vector.tensor_tensor(out=nthr[:], in0=nlo[:], in1=nthrb_p[:],
                                        op=Alu.add)
                sgn = radix.tile([P, 1], F32, name="sgn")
                nc.scalar.activation(out=sjunk[:], in_=scores_rep[:],
                                     func=Act.Sign, bias=nthr[:, :1], scale=1.0,
                                     accum_out=sgn[:])
                sel = radix.tile([P, 1], F32, name="sel")
                nc.vector.tensor_scalar(out=sel[:], in0=sgn[:], scalar1=0.0,
                                        scalar2=None, op0=Alu.is_ge)
                s_all = radix.tile([P, 1], F32, name="s_all")
                nc.gpsimd.partition_all_reduce(s_all[:], sel[:], channels=P,
                                               reduce_op=Red.add)
                nd = radix.tile([P, 1], F32, name="nd")
                nc.vector.tensor_scalar(out=nd[:], in0=s_all[:], scalar1=-1.0,
                                        scalar2=-w_p, op0=Alu.add, op1=Alu.mult)
                nlo2 = radix.tile([P, 1], F32, name="nlo2")
                nc.vector.tensor_tensor(out=nlo2[:], in0=nlo[:], in1=nd[:],
                                        op=Alu.add)
                nlo = nlo2

            # ---- mask + global rank (exclusive prefix of mask) --------------
            m0 = radix.tile([P, NT], F32, name="m0")
            nc.vector.tensor_tensor(out=m0[:], in0=scores_sb[:],
                                    in1=nlo[:, :1].to_broadcast([P, NT]),
                                    op=Alu.add)
            maskf = radix.tile([P, NT], F32, name="maskf")
            nc.vector.tensor_scalar(out=maskf[:], in0=m0[:], scalar1=0.0,
                                    scalar2=None, op0=Alu.is_ge)
            colsum_p = misc_psum.tile([NT, 1], F32, name="mp")
            nc.tensor.matmul(out=colsum_p[:], lhsT=maskf[:], rhs=o128x1_sb[:],
                             start=True, stop=True)
            colsum = radix.tile([NT, 1], F32, name="colsum")
            nc.vector.tensor_copy(out=colsum[:], in_=colsum_p[:])
            excl_p = misc_psum.tile([NT, 1], F32, name="mp")
            nc.tensor.matmul(out=excl_p[:], lhsT=slt32_sb[:], rhs=colsum[:],
                             start=True, stop=True)
            excl = radix.tile([NT, 1], F32, name="excl")
            nc.vector.tensor_copy(out=excl[:], in_=excl_p[:])
            diag = radix.tile([NT, NT], F32, name="diag")
            nc.vector.tensor_tensor(out=diag[:], in0=id32_sb[:],
                                    in1=excl[:, :1].to_broadcast([NT, NT]),
                                    op=Alu.mult)
            rank_p = misc_psum.tile([P, NT], F32, name="mp")
            nc.tensor.matmul(out=rank_p[:], lhsT=ltri_sb[:], rhs=maskf[:],
                             start=True, stop=False, skip_group_check=True)
            nc.tensor.matmul(out=rank_p[:], lhsT=o32x128_sb[:], rhs=diag[:],
                             start=False, stop=True, skip_group_check=True)
            rank = radix.tile([P, NT], F32, name="rank")
            nc.vector.tensor_copy(out=rank[:], in_=rank_p[:])
            off = radix.tile([P, NT], F32, name="off")
            nc.vector.tensor_tensor(out=off[:], in0=rank[:],
                                    in1=hb_col[:, :1].to_broadcast([P, NT]),
                                    op=Alu.subtract)
            nc.vector.tensor_copy(out=offf_c[:], in_=off[:])
            nc.vector.tensor_copy(out=maskf_c[:], in_=maskf[:])

        misc_psum_ctx.__exit__(None, None, None)

        # ---- w1 cast-loads on the Pool queue.  Positioned after the radix
        # all_reduces so the in-order queue starts them only ~70us in, after
        # the x-tile DMAs have drained (they'd otherwise steal DMA bandwidth
        # from the critical-path score loads). ---------------------------------
        w1bf = []
        for kd in range(ND):
            t_ = w1_pool.tile([P, DFF], BF16, name=f"w1bf_{kd}")
            nc.gpsimd.dma_start(out=t_[:], in_=w1[kd * P:(kd + 1) * P, :])
            w1bf.append(t_)

        # ---- phase E: digit split + one-hot compaction matmuls --------------
        # off in [0, SEL) for in-window selected tokens; any other off value
        # (negative rank-window miss, >=SEL, or collision of an unselected
        # token) produces no match in the lo-digit equality below, and
        # unselected tokens are additionally zeroed via tokid*mask weights.
        with ExitStack() as SE:
            ep = SE.enter_context(tc.tile_pool(name="epool", bufs=1))
            e_psum = SE.enter_context(tc.tile_pool(name="e_psum", bufs=2,
                                                   space="PSUM"))
            off = offf_c
            eq7a = ep.tile([P, NT, 7], F32, name="eq7a")
            nc.vector.tensor_tensor(
                out=eq7a[:], in0=off[:, :, None].to_broadcast([P, NT, 7]),
                in1=thr128[:, None, :].to_broadcast([P, NT, 7]), op=Alu.is_ge)
            hi128 = ep.tile([P, NT], F32, name="hi128")
            nc.vector.tensor_reduce(out=hi128[:], in_=eq7a[:],
                                    axis=mybir.AxisListType.X, op=Alu.add)
            hm = ep.tile([P, NT], F32, name="hm")
            nc.vector.tensor_scalar(out=hm[:], in0=hi128[:], scalar1=-128.0,
                                    scalar2=None, op0=Alu.mult)
            lo128 = ep.tile([P, NT], F32, name="lo128")
            nc.vector.tensor_tensor(out=lo128[:], in0=off[:], in1=hm[:],
                                    op=Alu.add)
            eq7b = ep.tile([P, NT, 7], F32, name="eq7b")
            nc.vector.tensor_tensor(
                out=eq7b[:], in0=lo128[:, :, None].to_broadcast([P, NT, 7]),
                in1=thr16[:, None, :].to_broadcast([P, NT, 7]), op=Alu.is_ge)
            mid = ep.tile([P, NT], F32, name="mid")
            nc.vector.tensor_reduce(out=mid[:], in_=eq7b[:],
                                    axis=mybir.AxisListType.X, op=Alu.add)
            hm2 = ep.tile([P, NT], F32, name="hm2")
            nc.vector.tensor_scalar(out=hm2[:], in0=mid[:], scalar1=-16.0,
                                    scalar2=None, op0=Alu.mult)
            lo16 = ep.tile([P, NT], F32, name="lo16")
            nc.vector.tensor_tensor(out=lo16[:], in0=lo128[:], in1=hm2[:],
                                    op=Alu.add)
            h8 = ep.tile([P, NT], F32, name="h8")
            nc.vector.tensor_scalar(out=h8[:], in0=hi128[:], scalar1=8.0,
                                    scalar2=None, op0=Alu.mult)
            hi16 = ep.tile([P, NT], F32, name="hi16")
            nc.vector.tensor_tensor(out=hi16[:], in0=h8[:], in1=mid[:],
                                    op=Alu.add)
            # token id = c*128 + p; weight the SMALL equality factors by
            # c*mask (chain C) and p*mask (chain D), then sel16 = 128*C + D.
            cwm = ep.tile([P, NT], F32, name="cwm")
            nc.vector.tensor_tensor(out=cwm[:], in0=cvalf[:], in1=maskf_c[:],
                                    op=Alu.mult)
            pwm = ep.tile([P, NT], F32, name="pwm")
            nc.vector.tensor_tensor(out=pwm[:], in0=maskf_c[:],
                                    in1=iota_f[:, :1].to_broadcast([P, NT]),
                                    op=Alu.mult)

            eq16 = ep.tile([P, NT, 16], F32, name="eq16")
            nc.vector.tensor_tensor(
                out=eq16[:], in0=iJ16f[:],
                in1=lo16[:, :, None].to_broadcast([P, NT, 16]), op=Alu.is_equal)
            eqc16 = ep.tile([P, NT, 16], F32, name="eqc16")
            nc.vector.tensor_tensor(
                out=eqc16[:], in0=eq16[:],
                in1=cwm[:, :, None].to_broadcast([P, NT, 16]), op=Alu.mult)
            eqp16 = ep.tile([P, NT, 16], F32, name="eqp16")
            nc.vector.tensor_tensor(
                out=eqp16[:], in0=eq16[:],
                in1=pwm[:, :, None].to_broadcast([P, NT, 16]), op=Alu.mult)
            eq64 = ep.tile([P, NT, 64], F32, name="eq64")
            nc.vector.tensor_tensor(
                out=eq64[:], in0=iK64f[:],
                in1=hi16[:, :, None].to_broadcast([P, NT, 64]), op=Alu.is_equal)

            pC = e_psum.tile([16, 64], F32, name="pC")
            pD = e_psum.tile([16, 64], F32, name="pD")
            for c in range(NT):
                nc.tensor.matmul(out=pC[:], lhsT=eqc16[:, c, :],
                                 rhs=eq64[:, c, :], start=(c == 0),
                                 stop=(c == NT - 1), skip_group_check=True)
            for c in range(NT):
                nc.tensor.matmul(out=pD[:], lhsT=eqp16[:, c, :],
                                 rhs=eq64[:, c, :], start=(c == 0),
                                 stop=(c == NT - 1), skip_group_check=True)

            sC = ep.tile([16, 64], F32, name="sC")
            nc.vector.tensor_copy(out=sC[:], in_=pC[:])
            sD = ep.tile([16, 64], F32, name="sD")
            nc.vector.tensor_copy(out=sD[:], in_=pD[:])
            uC = ep.tile([16, 64], F32, name="uC")
            nc.vector.tensor_scalar(out=uC[:], in0=sC[:], scalar1=128.0,
                                    scalar2=None, op0=Alu.mult)
            sel16v = ep.tile([16, 64], F32, name="sel16v")
            nc.vector.tensor_tensor(out=sel16v[:], in0=uC[:], in1=sD[:],
                                    op=Alu.add)

            # scatter index layout [128, 64] (16-wrap replicated to 128)
            rep_ps = e_psum.tile([P, 64], F32, name="rep_ps")
            nc.tensor.matmul(out=rep_ps[:], lhsT=rep16_sb[:], rhs=sel16v[:],
                             start=True, stop=True)
            nc.vector.tensor_copy(out=idx16_sb[:], in_=rep_ps[:])  # f32->i16

            # gather index layout [128, 8]: selidx[p, k] = sel16v[p%16, 8k+p//16]
            selps = e_psum.tile([P, NSJ], F32, name="selps")
            for g in range(8):
                nc.tensor.matmul(out=selps[:], lhsT=ewrap_sb[:, g * P:(g + 1) * P],
                                 rhs=sel16v[:, g::8], start=(g == 0),
                                 stop=(g == 7), skip_group_check=True)
            nc.vector.tensor_copy(out=selidx_sb[:], in_=selps[:])  # f32->i32

        dig_ctx.__exit__(None, None, None)

        # ---- gather + transpose + MLP ---------------------------------------
        if True:
            with ExitStack() as SB:
                xt_pool = SB.enter_context(tc.tile_pool(name="xt", bufs=1))
                xsel_pool = SB.enter_context(tc.tile_pool(name="xsel", bufs=3))
                tp_psum = SB.enter_context(tc.tile_pool(name="tp_psum", bufs=2,
                                                        space="PSUM"))
                mm1_psum = SB.enter_context(tc.tile_pool(name="mm1_psum", bufs=6,
                                                         space="PSUM"))

                xt = []
                for kd in range(ND):
                    xt.append(xt_pool.tile([P, SEL], BF16, name=f"xt_{kd}"))
                for j in range(NSJ):
                    xs = xsel_pool.tile([P, D], BF16, name="xsel")
                    nc.gpsimd.indirect_dma_start(
                        out=xs[:], out_offset=None, in_=x_row,
                        in_offset=IndirectOffsetOnAxis(ap=selidx_sb[:, j:j + 1],
                                                       axis=0))
                    for kd in range(ND):
                        tp = tp_psum.tile([P, P], BF16, name="tp")
                        nc.tensor.transpose(out=tp[:],
                                            in_=xs[:, kd * P:(kd + 1) * P],
                                            identity=identb_sb[:])
                        if (j + kd) % 2 == 0:
                            nc.vector.tensor_copy(
                                out=xt[kd][:, j * P:(j + 1) * P], in_=tp[:])
                        else:
                            nc.scalar.activation(
                                out=xt[kd][:, j * P:(j + 1) * P], in_=tp[:],
                                func=Act.Copy, bias=0.0, scale=1.0)

                # ---- mm1: ht[m, sel] = gelu(w1^T x_sel^T + b1) ---------------
                for n in range(SEL // 512):
                    for m in range(NM):
                        ph = mm1_psum.tile([P, 512], F32, name="ph")
                        for kd in range(ND):
                            nc.tensor.matmul(
                                out=ph[:],
                                lhsT=w1bf[kd][:, m * P:(m + 1) * P],
                                rhs=xt[kd][:, n * 512:(n + 1) * 512],
                                start=(kd == 0), stop=(kd == ND - 1),
                            )
                        nc.scalar.activation(
                            out=ht[:, m, n * 512:(n + 1) * 512], in_=ph[:],
                            func=Act.Gelu_apprx_tanh, bias=b1t_sb[:, m:m + 1],
                            scale=1.0,
                        )

            w1_ctx.__exit__(None, None, None)  # free w1 region for w2 stream

            # ---- mm2: y[sel, D] = ht^T @ w2 + b2, then scatter-add ----------
            with ExitStack() as SY:
                y_pool = SY.enter_context(tc.tile_pool(name="y", bufs=1))
                w2_pool = SY.enter_context(tc.tile_pool(name="w2s", bufs=12))
                mm2_psum = SY.enter_context(tc.tile_pool(name="mm2_psum", bufs=8,
                                                         space="PSUM"))
                for n in range(D // 512):
                    y_n = y_pool.tile([P, NSJ, 512], F32, name=f"y{n}")
                    pys = [mm2_psum.tile([P, 512], F32, name="py")
                           for _ in range(NSJ)]
                    for s in range(NSJ):
                        nc.tensor.matmul(
                            out=pys[s][:], lhsT=o1x128b_sb[:],
                            rhs=b2bf_sb[:, n * 512:(n + 1) * 512],
                            start=True, stop=False, skip_group_check=True,
                        )
                    for kg in range(NM // NKGRP):
                        w2t = w2_pool.tile([P, NKGRP, 512], BF16, name="w2t")
                        src = w2[:, n * 512:(n + 1) * 512].rearrange(
                            "(g p) f -> p g f", p=P)[:, kg * NKGRP:(kg + 1) * NKGRP, :]
                        nc.gpsimd.dma_start(out=w2t[:], in_=src)
                        for ki in range(NKGRP):
                            kk = kg * NKGRP + ki
                            for s in range(NSJ):
                                nc.tensor.matmul(
                                    out=pys[s][:],
                                    lhsT=ht[:, kk, s * P:(s + 1) * P],
                                    rhs=w2t[:, ki, :],
                                    start=False, stop=(kk == NM - 1),
                                    skip_group_check=True,
                                )
                    # copy out of PSUM, scattering each 512-token half as soon
                    # as its copies land: out_row[idx[r], n*512:(n+1)*512] += y
                    for s in range(NSJ):
                        nc.scalar.activation(out=y_n[:, s, :], in_=pys[s][:],
                                             func=Act.Copy, bias=0.0, scale=1.0)
                        if s % 4 == 3:
                            h = s // 4
                            nc.gpsimd.dma_scatter_add(
                                out_row[:, n * 512:(n + 1) * 512],
                                y_n[:, h * 4:(h + 1) * 4, :],
                                idx16_sb[:, h * 32:(h + 1) * 32],
                                SEL // 2,
                                SEL // 2,
                                512,
                                elem_step=D,
                            )

        ht_ctx.__exit__(None, None, None)

    nc.compile()
    return nc


def make_consts():
    q = np.arange(P)
    import ml_dtypes
    consts = {
        "identb": np.eye(P, dtype=ml_dtypes.bfloat16),
        "ltri128": (q[:, None] < q[None, :]).astype(np.float32),  # [q, p] = q < p
        "slt32": (np.arange(NT)[:, None] < np.arange(NT)[None, :]).astype(np.float32),
        "id32": np.eye(NT, dtype=np.float32),
        "ones_1x128": np.ones((1, P), np.float32),
        "ones_1x128b": np.ones((1, P), ml_dtypes.bfloat16),
        "ones_128x1": np.ones((P, 1), np.float32),
        "ones_32x128": np.ones((NT, P), np.float32),
        "rep16": (np.arange(16)[:, None] == (np.arange(P)[None, :] % 16)
                  ).astype(np.float32),
    }
    # ewrap[i, g*128 + p] = 1 iff p == g*16 + i  (16-wrap -> 128-wrap expand)
    ew = np.zeros((16, 8 * P), np.float32)
    for i in range(16):
        for g in range(8):
            ew[i, g * P + g * 16 + i] = 1.0
    consts["ewrap"] = ew
    return consts


def make_in_maps(x, W1, b1, W2, b2, wr, br):
    consts = make_consts()
    x = np.ascontiguousarray(np.asarray(x, np.float32))
    in_maps = []
    for c in range(NCORES):
        b, h = divmod(c, 2)
        m = {
            "x_row": x[b],
            "w1": np.asarray(W1, np.float32),
            "w2": np.asarray(W2, np.float32),
            "wr": np.asarray(wr, np.float32).reshape(1, D),
            "b1t": np.ascontiguousarray(np.asarray(b1, np.float32).reshape(NM, P).T),
            "b2": np.asarray(b2, np.float32).reshape(1, D),
            "hbase": np.array([[h * SEL]], np.float32),
        }
        m.update(consts)
        in_maps.append(m)
    return in_maps


_NC_CACHE = None


def _get_program():
    global _NC_CACHE
    if _NC_CACHE is None:
        _NC_CACHE = build_program()
    return _NC_CACHE


def kernel(x, W1, b1, W2, b2, wr, br):
    from concourse.bass_utils import run_bass_kernel_spmd

    nc = _get_program()
    in_maps = make_in_maps(x, W1, b1, W2, b2, wr, br)
    res = run_bass_kernel_spmd(nc, in_maps, list(range(NCORES))).results
    out = np.stack(
        [res[2 * b]["out_row"] + res[2 * b + 1]["out_row"] for b in range(B)]
    )
    return out.astype(np.float32)


# revision 57
# speedup vs baseline: 2.4180x; 1.0095x over previous
"""MoD (mixture-of-depths) MLP wrapper kernel for Trainium2, 8 NeuronCores.

Sharding: core c handles batch row b = c//2 and the half of that row's
top-K tokens with global selection ranks in [h*1024, (h+1)*1024), h = c%2.
Each core computes the full row's router scores + top-K threshold locally
(no collectives), gathers exactly 1024 token rows by rank via indirect DMA,
runs the FFN in bf16 (fp32 accumulation), and scatters results back into the
pre-zeroed per-core output buffer with dma_scatter_add.  Host sums the two
buffers of each row.

v2 schedule: x loads get DMA priority, the top-K threshold search runs as
fused Sign-activation counts + partition_all_reduce, rank compaction uses a
digit-decomposed one-hot matmul (also producing the int16 scatter index
layout directly), and the output scatter is dma_scatter_add (per-index DMA
descriptors) instead of whole-tensor indirect DMA.
"""

import sys

sys.path.insert(0, "/opt/trn_rl_repo")

from contextlib import ExitStack

import numpy as np

from concourse import bass, bass_isa, mybir
from concourse import bacc
import concourse.tile as tile
from concourse.bass import IndirectOffsetOnAxis

B, L, D = 4, 4096, 1024
DFF = 4 * D
K = L // 2              # 2048 selected tokens per row
NCORES = 8
P = 128
NT = L // P             # 32 token tiles per row
SEL = K // 2            # 1024 selected tokens per core
NSJ = SEL // P          # 8 selected-token blocks
ND = D // P             # 8 d chunks
NM = DFF // P           # 32 dff tiles
NKGRP = 4               # w2 k-chunks per streamed tile
RADIX_PASSES = 4

F32 = mybir.dt.float32
BF16 = mybir.dt.bfloat16
I32 = mybir.dt.int32
I16 = mybir.dt.int16
Alu = mybir.AluOpType
Act = mybir.ActivationFunctionType
Red = bass_isa.ReduceOp


def build_program():
    nc = bacc.Bacc(
        "TRN2",
        target_bir_lowering=False,
        debug=False,
        enable_asserts=False,
        num_devices=NCORES,
    )

    x_row = nc.dram_tensor("x_row", [L, D], F32, kind="ExternalInput").ap()
    w1 = nc.dram_tensor("w1", [D, DFF], F32, kind="ExternalInput").ap()
    w2 = nc.dram_tensor("w2", [DFF, D], F32, kind="ExternalInput").ap()
    wr = nc.dram_tensor("wr", [1, D], F32, kind="ExternalInput").ap()
    b1t = nc.dram_tensor("b1t", [P, NM], F32, kind="ExternalInput").ap()
    b2 = nc.dram_tensor("b2", [1, D], F32, kind="ExternalInput").ap()
    hbase = nc.dram_tensor("hbase", [1, 1], F32, kind="ExternalInput").ap()
    ltri = nc.dram_tensor("ltri128", [P, P], F32, kind="ExternalInput").ap()
    slt32 = nc.dram_tensor("slt32", [NT, NT], F32, kind="ExternalInput").ap()
    id32 = nc.dram_tensor("id32", [NT, NT], F32, kind="ExternalInput").ap()
    ones_1x128 = nc.dram_tensor("ones_1x128", [1, P], F32, kind="ExternalInput").ap()
    ones_1x128b = nc.dram_tensor("ones_1x128b", [1, P], BF16, kind="ExternalInput").ap()
    ones_128x1 = nc.dram_tensor("ones_128x1", [P, 1], F32, kind="ExternalInput").ap()
    ones128 = nc.dram_tensor("ones128", [P, P], F32, kind="ExternalInput").ap()
    ones_32x128 = nc.dram_tensor("ones_32x128", [NT, P], F32, kind="ExternalInput").ap()
    rep16 = nc.dram_tensor("rep16", [32, P], F32, kind="ExternalInput").ap()
    ewrap = nc.dram_tensor("ewrap", [32, 8 * P], F32, kind="ExternalInput").ap()

    out_row = nc.dram_tensor("out_row", [L, D], F32, kind="ExternalOutput").ap()

    GRPS = ((0, 20), (20, 30), (30, 31), (31, 32))
    scores_dg = [nc.dram_tensor(f"scores_dg{i}", [hi - lo, P], F32).ap()
                 for i, (lo, hi) in enumerate(GRPS)]

    with tile.TileContext(nc) as tc, ExitStack() as S0:
        const = S0.enter_context(tc.tile_pool(name="const", bufs=1))
        # pool stack (LIFO): const | ht | w1 | dig | ...phases
        ht_ctx = tc.tile_pool(name="ht", bufs=1)
        ht_pool = ht_ctx.__enter__()
        ht = ht_pool.tile([P, NM, SEL], BF16)
        w1_ctx = tc.tile_pool(name="w1bf", bufs=1)
        w1_pool = w1_ctx.__enter__()

        def cload(pool, ap, shape, dtype=F32, name=None):
            t = pool.tile(shape, dtype, name=name)
            nc.sync.dma_start(out=t[:], in_=ap)
            return t

        # ---- SP-queue order: wr, o1, oc, hbase FIRST (phase A needs them) ---
        wr_sb = cload(const, wr, [1, D], name="c_wr")
        o1x128_sb = cload(const, ones_1x128, [1, P], name="c_o1")
        o128x1_sb = cload(const, ones_128x1, [P, 1], name="c_oc")
        ones128_sb = cload(const, ones128, [P, P], name="c_o128")
        hb_sb = cload(const, hbase, [1, 1], name="c_hb")

        # w1 tiles exist from the start (loads are issued after the radix)
        w1bf = [w1_pool.tile([P, DFF], BF16, name=f"w1bf_{kd}")
                for kd in range(ND)]

        # ---- Pool-queue iotas (independent of SP queue) ---------------------
        # big digit-decomposition iota tables live only through phase E
        dig_ctx = tc.tile_pool(name="dig", bufs=1)
        dig = dig_ctx.__enter__()

        iota_i = const.tile([P, 1], I32)
        nc.gpsimd.iota(iota_i[:], pattern=[[1, 1]], base=0, channel_multiplier=1)
        tokid = const.tile([P, NT], I32)
        nc.gpsimd.iota(tokid[:], pattern=[[P, NT]], base=0, channel_multiplier=1)
        iC_i = const.tile([P, NT], I32)
        nc.gpsimd.iota(iC_i[:], pattern=[[1, NT]], base=0, channel_multiplier=0)
        iQ_i = const.tile([P, 128], I32)
        nc.gpsimd.iota(iQ_i[:], pattern=[[1, 128]], base=0, channel_multiplier=0)
        iK64_i = dig.tile([P, NT, 64], I16)
        nc.gpsimd.iota(iK64_i[:], pattern=[[0, NT], [1, 64]], base=0,
                       channel_multiplier=0)
        iJ16_i = dig.tile([P, NT, 16], I16)
        nc.gpsimd.iota(iJ16_i[:], pattern=[[0, NT], [1, 16]], base=0,
                       channel_multiplier=0)
        i7_i = const.tile([P, 7], I32)
        nc.gpsimd.iota(i7_i[:], pattern=[[1, 7]], base=1, channel_multiplier=0)

        iota_f = const.tile([P, 1], F32)
        nc.vector.tensor_copy(out=iota_f[:], in_=iota_i[:])
        tokidf = const.tile([P, NT], F32)
        nc.vector.tensor_copy(out=tokidf[:], in_=tokid[:])
        cvalf = const.tile([P, NT], F32)
        nc.vector.tensor_copy(out=cvalf[:], in_=iC_i[:])
        iK64b = dig.tile([P, NT, 64], BF16)
        nc.vector.tensor_copy(out=iK64b[:], in_=iK64_i[:])
        iJ16b = dig.tile([P, NT, 16], BF16)
        nc.vector.tensor_copy(out=iJ16b[:], in_=iJ16_i[:])
        iotab = const.tile([P, 1], BF16)
        nc.vector.tensor_copy(out=iotab[:], in_=iota_i[:])
        cvalb = const.tile([P, NT], BF16)
        nc.vector.tensor_copy(out=cvalb[:], in_=iC_i[:])
        i7f = const.tile([P, 7], F32)
        nc.vector.tensor_copy(out=i7f[:], in_=i7_i[:])
        thr128 = const.tile([P, 7], F32)
        nc.vector.tensor_scalar(out=thr128[:], in0=i7f[:], scalar1=128.0,
                                scalar2=None, op0=Alu.mult)
        thr16 = const.tile([P, 7], F32)
        nc.vector.tensor_scalar(out=thr16[:], in0=i7f[:], scalar1=16.0,
                                scalar2=None, op0=Alu.mult)
        # radix pass-1 threshold grid (build-time constants: lo=-16, w=0.25)
        iQf = const.tile([P, 128], F32)
        nc.vector.tensor_copy(out=iQf[:], in_=iQ_i[:])
        thr1row = const.tile([P, 128], F32)
        nc.vector.tensor_scalar(out=thr1row[:], in0=iQf[:], scalar1=32.0 / P,
                                scalar2=-16.0, op0=Alu.mult, op1=Alu.add)
        # negated per-pass threshold offsets for radix passes 2..4
        W1P = 32.0 / P
        nthrbs = []
        for p_ in range(1, RADIX_PASSES):
            w_p = W1P / (P ** p_)
            t_ = const.tile([P, 1], F32, name=f"nthrb{p_}")
            nc.vector.tensor_scalar(out=t_[:], in0=iota_f[:], scalar1=-w_p,
                                    scalar2=None, op0=Alu.mult)
            nthrbs.append((w_p, t_))
        hb_col = const.tile([P, 1], F32)
        nc.gpsimd.partition_broadcast(hb_col[:], hb_sb[:])

        scores_sb = const.tile([P, NT], F32)
        selidx_sb = const.tile([P, NSJ], I32)
        idx16_sb = const.tile([P, SEL // 16], I16)

        misc_psum_ctx = tc.tile_pool(name="misc_psum", bufs=2, space="PSUM")
        misc_psum = misc_psum_ctx.__enter__()

        # ---- phase A: router scores (fp32, exact; router bias dropped — it
        # shifts every score equally so the top-K set is unchanged).  The
        # first radix pass uses a build-time-constant threshold grid, so its
        # per-tile compare + count-matmul accumulation is folded in here. -----
        c1_psum_ctx = tc.tile_pool(name="c1_psum", bufs=1, space="PSUM")
        c1_psum = c1_psum_ctx.__enter__()
        cnt1_ps = c1_psum.tile([P, 128], F32, name="cnt1")
        nlo = const.tile([P, 1], F32, name="nlo")
        with ExitStack() as SA:
            apool = SA.enter_context(tc.tile_pool(name="apool", bufs=1))
            xs_pool = SA.enter_context(tc.tile_pool(name="xs", bufs=6))
            junk_pool = SA.enter_context(tc.tile_pool(name="junk", bufs=2))
            cmp_pool = SA.enter_context(tc.tile_pool(name="cmp", bufs=3))

            wrb = apool.tile([P, D], F32)
            for n in range(D // 512):
                pt = misc_psum.tile([P, 512], F32, name="mp")
                nc.tensor.matmul(out=pt[:], lhsT=o1x128_sb[:],
                                 rhs=wr_sb[:, n * 512:(n + 1) * 512],
                                 start=True, stop=True)
                nc.vector.tensor_copy(out=wrb[:, n * 512:(n + 1) * 512], in_=pt[:])

            x_last = None
            for t in range(NT):
                x_t = xs_pool.tile([P, D], F32)
                nc.sync.dma_start(out=x_t[:], in_=x_row[t * P:(t + 1) * P, :])
                x_last = x_t
                if t == 26:
                    nc.sync.dma_start(
                        out=scores_dg[0].rearrange("c p -> p c"),
                        in_=scores_sb[:, 0:20])
                prod = junk_pool.tile([P, D], F32, name="prod")
                nc.vector.tensor_tensor(out=prod[:], in0=x_t[:], in1=wrb[:],
                                        op=Alu.mult)
                sink = junk_pool.tile([P, D], BF16, name="sink")
                nc.scalar.activation(out=sink[:], in_=prod[:], func=Act.Identity,
                                     bias=0.0, scale=1.0,
                                     accum_out=scores_sb[:, t:t + 1])
                cmp_t = cmp_pool.tile([P, 128], F32, name="cmp")
                nc.vector.tensor_tensor(
                    out=cmp_t[:],
                    in0=scores_sb[:, t:t + 1].to_broadcast([P, 128]),
                    in1=thr1row[:], op=Alu.is_ge)
                nc.tensor.matmul(out=cnt1_ps[:], lhsT=ones128_sb[:], rhs=cmp_t[:],
                                 start=(t == 0), stop=(t == NT - 1),
                                 skip_group_check=True)


            # pass-1 finalize on every partition (count matmul used an
            # all-ones lhsT, so each partition holds the full count row):
            # nlo = -(lo1) = 16 - (sum(cnt>=K) - 1)*0.25
            selr = apool.tile([P, 128], F32, name="selr")
            nc.vector.tensor_scalar(out=selr[:], in0=cnt1_ps[:],
                                    scalar1=float(K), scalar2=None,
                                    op0=Alu.is_ge)
            s1 = apool.tile([P, 1], F32, name="s1")
            nc.vector.tensor_reduce(out=s1[:], in_=selr[:],
                                    axis=mybir.AxisListType.X, op=Alu.add)
            q1 = apool.tile([P, 1], F32, name="q1")
            nc.vector.tensor_scalar(out=q1[:], in0=s1[:], scalar1=-1.0,
                                    scalar2=-W1P, op0=Alu.add, op1=Alu.mult)
            nc.vector.tensor_scalar(out=nlo[:], in0=q1[:], scalar1=16.0,
                                    scalar2=None, op0=Alu.add)
        c1_psum_ctx.__exit__(None, None, None)

        offf_c = const.tile([P, NT], F32)
        maskf_c = const.tile([P, NT], F32)

        # ---- phases B+C+D: replicate scores, radix threshold, rank ----------
        with ExitStack() as SC:
            radix = SC.enter_context(tc.tile_pool(name="radix", bufs=2))
            rep_pool = SC.enter_context(tc.tile_pool(name="rep", bufs=1))

            # broadcast-read the spilled scores, one DMA per 1024-token group,
            # FIRST on the in-order SP queue right after the x loads (the
            # remaining const loads queue behind, they aren't needed till later)
            scores_rep = rep_pool.tile([P, L], F32)
            for gi, (glo, ghi) in enumerate(GRPS):
                n_ = (ghi - glo) * P
                if gi > 0:   # g0 was spilled inside the x stream
                    nc.sync.dma_start(
                        out=scores_dg[gi].rearrange("c p -> p c"),
                        in_=scores_sb[:, glo:ghi])
                nc.sync.dma_start(
                    out=scores_rep[:, glo * P:ghi * P],
                    in_=scores_dg[gi].rearrange("c p -> () (c p)")
                    .to_broadcast([P, n_]))

            # gate the w1 cast-loads behind the score broadcast so their DMAs
            # cannot delay it (WAW edge: the w1 DMA overwrites the gate byte)
            for kd in range(ND):
                nc.vector.tensor_copy(out=w1bf[kd][0:1, 0:1],
                                      in_=scores_rep[0:1, kd:kd + 1])

            # ---- remaining small consts on the SP queue ---------------------
            b1t_sb = cload(const, b1t, [P, NM], name="c_b1t")
            ltri_sb = cload(const, ltri, [P, P], name="c_lt")
            slt32_sb = cload(const, slt32, [NT, NT], name="c_sl")
            id32_sb = cload(const, id32, [NT, NT], name="c_id32")
            o1x128b_sb = cload(const, ones_1x128b, [1, P], BF16, name="c_o1b")
            o32x128_sb = cload(const, ones_32x128, [NT, P], name="c_o32")
            rep16_sb = cload(const, rep16, [32, P], name="c_rep16")
            ewrap_sb = cload(const, ewrap, [32, 8 * P], name="c_ew")
            b2bf_sb = const.tile([1, D], BF16)
            nc.gpsimd.dma_start(out=b2bf_sb[:], in_=b2)  # cast f32 -> bf16

            sjunk = rep_pool.tile([P, L], BF16, name="sjunk")
            for w_p, nthrb_p in nthrbs:
                nthr = radix.tile([P, 1], F32, name="nthr")
                nc.vector.tensor_tensor(out=nthr[:], in0=nlo[:], in1=nthrb_p[:],
                                        op=Alu.add)
                sgn = radix.tile([P, 1], F32, name="sgn")
                nc.scalar.activation(out=sjunk[:], in_=scores_rep[:],
                                     func=Act.Sign, bias=nthr[:, :1], scale=1.0,
                                     accum_out=sgn[:])
                sel = radix.tile([P, 1], F32, name="sel")
                nc.vector.tensor_scalar(out=sel[:], in0=sgn[:], scalar1=0.0,
                                        scalar2=None, op0=Alu.is_ge)
                s_all = radix.tile([P, 1], F32, name="s_all")
                nc.gpsimd.partition_all_reduce(s_all[:], sel[:], channels=P,
                                               reduce_op=Red.add)
                nd = radix.tile([P, 1], F32, name="nd")
                nc.vector.tensor_scalar(out=nd[:], in0=s_all[:], scalar1=-1.0,
                                        scalar2=-w_p, op0=Alu.add, op1=Alu.mult)
                nlo2 = radix.tile([P, 1], F32, name="nlo2")
                nc.vector.tensor_tensor(out=nlo2[:], in0=nlo[:], in1=nd[:],
                                        op=Alu.add)
                nlo = nlo2

            # ---- mask + global rank (exclusive prefix of mask) --------------
            m0 = radix.tile([P, NT], F32, name="m0")
            nc.vector.tensor_tensor(out=m0[:], in0=scores_sb[:],
                                    in1=nlo[:, :1].to_broadcast([P, NT]),
                                    op=Alu.add)
            maskf = radix.tile([P, NT], F32, name="maskf")
            nc.vector.tensor_scalar(out=maskf[:], in0=m0[:], scalar1=0.0,
                                    scalar2=None, op0=Alu.is_ge)
            colsum_p = misc_psum.tile([NT, 1], F32, name="mp")
            nc.tensor.matmul(out=colsum_p[:], lhsT=maskf[:], rhs=o128x1_sb[:],
                             start=True, stop=True)
            colsum = radix.tile([NT, 1], F32, name="colsum")
            nc.vector.tensor_copy(out=colsum[:], in_=colsum_p[:])
            excl_p = misc_psum.tile([NT, 1], F32, name="mp")
            nc.tensor.matmul(out=excl_p[:], lhsT=slt32_sb[:], rhs=colsum[:],
                             start=True, stop=True)
            excl = radix.tile([NT, 1], F32, name="excl")
            nc.vector.tensor_copy(out=excl[:], in_=excl_p[:])
            diag = radix.tile([NT, NT], F32, name="diag")
            nc.vector.tensor_tensor(out=diag[:], in0=id32_sb[:],
                                    in1=excl[:, :1].to_broadcast([NT, NT]),
                                    op=Alu.mult)
            rank_p = misc_psum.tile([P, NT], F32, name="mp")
            nc.tensor.matmul(out=rank_p[:], lhsT=ltri_sb[:], rhs=maskf[:],
                             start=True, stop=False, skip_group_check=True)
            nc.tensor.matmul(out=rank_p[:], lhsT=o32x128_sb[:], rhs=diag[:],
                             start=False, stop=True, skip_group_check=True)
            rank = radix.tile([P, NT], F32, name="rank")
            nc.vector.tensor_copy(out=rank[:], in_=rank_p[:])
            off = radix.tile([P, NT], F32, name="off")
            nc.vector.tensor_tensor(out=off[:], in0=rank[:],
                                    in1=hb_col[:, :1].to_broadcast([P, NT]),
                                    op=Alu.subtract)
            nc.vector.tensor_copy(out=offf_c[:], in_=off[:])
            nc.vector.tensor_copy(out=maskf_c[:], in_=maskf[:])

        misc_psum_ctx.__exit__(None, None, None)

        # ---- w1 cast-loads on the Pool queue.  Positioned after the radix
        # all_reduces so the in-order queue starts them only ~70us in, after
        # the x-tile DMAs have drained (they'd otherwise steal DMA bandwidth
        # from the critical-path score loads). ---------------------------------
        w1bf = []
        for kd in range(ND):
            t_ = w1_pool.tile([P, DFF], BF16, name=f"w1bf_{kd}")
            nc.gpsimd.dma_start(out=t_[:], in_=w1[kd * P:(kd + 1) * P, :])
            w1bf.append(t_)

        # ---- phase E: digit split + one-hot compaction matmuls --------------
        # off in [0, SEL) for in-window selected tokens; any other off value
        # (negative rank-window miss, >=SEL, or collision of an unselected
        # token) produces no match in the lo-digit equality below, and
        # unselected tokens are additionally zeroed via tokid*mask weights.
        with ExitStack() as SE:
            ep = SE.enter_context(tc.tile_pool(name="epool", bufs=1))
            e_psum = SE.enter_context(tc.tile_pool(name="e_psum", bufs=2,
                                                   space="PSUM"))
            off = offf_c
            eq7a = ep.tile([P, NT, 7], F32, name="eq7a")
            nc.vector.tensor_tensor(
                out=eq7a[:], in0=off[:, :, None].to_broadcast([P, NT, 7]),
                in1=thr128[:, None, :].to_broadcast([P, NT, 7]), op=Alu.is_ge)
            hi128 = ep.tile([P, NT], F32, name="hi128")
            nc.vector.tensor_reduce(out=hi128[:], in_=eq7a[:],
                                    axis=mybir.AxisListType.X, op=Alu.add)
            hm = ep.tile([P, NT], F32, name="hm")
            nc.vector.tensor_scalar(out=hm[:], in0=hi128[:], scalar1=-128.0,
                                    scalar2=None, op0=Alu.mult)
            lo128 = ep.tile([P, NT], F32, name="lo128")
            nc.vector.tensor_tensor(out=lo128[:], in0=off[:], in1=hm[:],
                                    op=Alu.add)
            eq7b = ep.tile([P, NT, 7], F32, name="eq7b")
            nc.vector.tensor_tensor(
                out=eq7b[:], in0=lo128[:, :, None].to_broadcast([P, NT, 7]),
                in1=thr16[:, None, :].to_broadcast([P, NT, 7]), op=Alu.is_ge)
            mid = ep.tile([P, NT], F32, name="mid")
            nc.vector.tensor_reduce(out=mid[:], in_=eq7b[:],
                                    axis=mybir.AxisListType.X, op=Alu.add)
            hm2 = ep.tile([P, NT], F32, name="hm2")
            nc.vector.tensor_scalar(out=hm2[:], in0=mid[:], scalar1=-16.0,
                                    scalar2=None, op0=Alu.mult)
            lo16 = ep.tile([P, NT], F32, name="lo16")
            nc.vector.tensor_tensor(out=lo16[:], in0=lo128[:], in1=hm2[:],
                                    op=Alu.add)
            h8 = ep.tile([P, NT], F32, name="h8")
            nc.vector.tensor_scalar(out=h8[:], in0=hi128[:], scalar1=8.0,
                                    scalar2=None, op0=Alu.mult)
            hi16 = ep.tile([P, NT], F32, name="hi16")
            nc.vector.tensor_tensor(out=hi16[:], in0=h8[:], in1=mid[:],
                                    op=Alu.add)
            # token id = c*128 + p; weight the SMALL equality factors by
            # c*mask (chain C, lhsT cols 0:16) and p*mask (chain D, cols
            # 16:32), then sel16 = 128*C + D.  All factors are small exact
            # integers, so the chain runs in bf16 (1 cycle/row matmuls).
            maskb = ep.tile([P, NT], BF16, name="maskb")
            nc.vector.tensor_copy(out=maskb[:], in_=maskf_c[:])
            cwm = ep.tile([P, NT], BF16, name="cwm")
            nc.vector.tensor_tensor(out=cwm[:], in0=cvalb[:], in1=maskb[:],
                                    op=Alu.mult)
            pwm = ep.tile([P, NT], BF16, name="pwm")
            nc.vector.tensor_tensor(out=pwm[:], in0=maskb[:],
                                    in1=iotab[:, :1].to_broadcast([P, NT]),
                                    op=Alu.mult)
            lo16b = ep.tile([P, NT], BF16, name="lo16b")
            nc.vector.tensor_copy(out=lo16b[:], in_=lo16[:])
            hi16b = ep.tile([P, NT], BF16, name="hi16b")
            nc.vector.tensor_copy(out=hi16b[:], in_=hi16[:])

            eq16 = ep.tile([P, NT, 16], BF16, name="eq16")
            nc.vector.tensor_tensor(
                out=eq16[:], in0=iJ16b[:],
                in1=lo16b[:, :, None].to_broadcast([P, NT, 16]), op=Alu.is_equal)
            eqcp = ep.tile([P, NT, 32], BF16, name="eqcp")
            nc.vector.tensor_tensor(
                out=eqcp[:, :, 0:16], in0=eq16[:],
                in1=cwm[:, :, None].to_broadcast([P, NT, 16]), op=Alu.mult)
            nc.vector.tensor_tensor(
                out=eqcp[:, :, 16:32], in0=eq16[:],
                in1=pwm[:, :, None].to_broadcast([P, NT, 16]), op=Alu.mult)
            eq64 = ep.tile([P, NT, 64], BF16, name="eq64")
            nc.vector.tensor_tensor(
                out=eq64[:], in0=iK64b[:],
                in1=hi16b[:, :, None].to_broadcast([P, NT, 64]), op=Alu.is_equal)

            pCD = e_psum.tile([32, 64], F32, name="pCD")
            for c in range(NT):
                nc.tensor.matmul(out=pCD[:], lhsT=eqcp[:, c, :],
                                 rhs=eq64[:, c, :], start=(c == 0),
                                 stop=(c == NT - 1), skip_group_check=True)

            sCD = ep.tile([32, 64], F32, name="sCD")
            nc.vector.tensor_copy(out=sCD[:], in_=pCD[:])

            # scatter index layout [128, 64] (16-wrap replicated to 128);
            # lhsT folds the 128*C + D combine (rows 0:16 scaled by 128)
            rep_ps = e_psum.tile([P, 64], F32, name="rep_ps")
            nc.tensor.matmul(out=rep_ps[:], lhsT=rep16_sb[:], rhs=sCD[:],
                             start=True, stop=True)
            nc.vector.tensor_copy(out=idx16_sb[:], in_=rep_ps[:])  # f32->i16

            # gather index layout [128, 8]: selidx[p, k] = sel16[p%16, 8k+p//16]
            selps = e_psum.tile([P, NSJ], F32, name="selps")
            for g in range(8):
                nc.tensor.matmul(out=selps[:], lhsT=ewrap_sb[:, g * P:(g + 1) * P],
                                 rhs=sCD[:, g::8], start=(g == 0),
                                 stop=(g == 7), skip_group_check=True)
            nc.vector.tensor_copy(out=selidx_sb[:], in_=selps[:])  # f32->i32

        dig_ctx.__exit__(None, None, None)

        # ---- gather + transpose + MLP ---------------------------------------
        if True:
            with ExitStack() as SB:
                xt_pool = SB.enter_context(tc.tile_pool(name="xt", bufs=1))
                xsel_pool = SB.enter_context(tc.tile_pool(name="xsel", bufs=5))
                mm1_psum = SB.enter_context(tc.tile_pool(name="mm1_psum", bufs=6,
                                                         space="PSUM"))

                # xt3[p, kd, t] = x_sel[t, kd*128+p], built by the DMA-engine
                # xbar transpose (one per gathered 128-token chunk)
                xt3 = xt_pool.tile([P, ND, SEL], BF16)
                for j in range(NSJ):
                    xs = xsel_pool.tile([P, D], BF16, name="xsel")
                    nc.gpsimd.indirect_dma_start(
                        out=xs[:], out_offset=None, in_=x_row,
                        in_offset=IndirectOffsetOnAxis(ap=selidx_sb[:, j:j + 1],
                                                       axis=0))
                    nc.scalar.dma_start_transpose(
                        out=xt3[:, :, j * P:(j + 1) * P], in_=xs[:])

                # ---- mm1: ht[m, sel] = gelu(w1^T x_sel^T + b1) ---------------
                for n in range(SEL // 512):
                    for m in range(NM):
                        ph = mm1_psum.tile([P, 512], F32, name="ph")
                        for kd in range(ND):
                            nc.tensor.matmul(
                                out=ph[:],
                                lhsT=w1bf[kd][:, m * P:(m + 1) * P],
                                rhs=xt3[:, kd, n * 512:(n + 1) * 512],
                                start=(kd == 0), stop=(kd == ND - 1),
                            )
                        nc.scalar.activation(
                            out=ht[:, m, n * 512:(n + 1) * 512], in_=ph[:],
                            func=Act.Gelu_apprx_tanh, bias=b1t_sb[:, m:m + 1],
                            scale=1.0,
                        )

            w1_ctx.__exit__(None, None, None)  # free w1 region for w2 stream

            # ---- mm2: y[sel, D] = ht^T @ w2 + b2, then scatter-add ----------
            with ExitStack() as SY:
                y_pool = SY.enter_context(tc.tile_pool(name="y", bufs=1))
                w2_pool = SY.enter_context(tc.tile_pool(name="w2s", bufs=16))
                mm2_psum = SY.enter_context(tc.tile_pool(name="mm2_psum", bufs=8,
                                                         space="PSUM"))
                # d-half 0: kg-major accumulation (w2 tiles stream in, all 8
                # token-block psums accumulate together)
                n = 0
                y_0 = y_pool.tile([P, NSJ, 512], F32, name="y0")
                pys = [mm2_psum.tile([P, 512], F32, name="py")
                       for _ in range(NSJ)]
                w2n1 = []   # d-half-1 tiles retained for the s-major pass
                for s in range(NSJ):
                    nc.tensor.matmul(
                        out=pys[s][:], lhsT=o1x128b_sb[:],
                        rhs=b2bf_sb[:, :512],
                        start=True, stop=False, skip_group_check=True,
                    )
                for kg in range(NM // NKGRP):
                    w2t = w2_pool.tile([P, NKGRP, 512], BF16, name="w2t")
                    if kg == 0:
                        # WAW gate: keep the w2 stream off the DMA engines
                        # until the gather/transpose pipeline has fed mm1
                        nc.vector.tensor_copy(out=w2t[0:1, 0, 0:1],
                                              in_=ht[0:1, 0, 0:1])
                    src = w2[:, :512].rearrange(
                        "(g p) f -> p g f", p=P)[:, kg * NKGRP:(kg + 1) * NKGRP, :]
                    nc.gpsimd.dma_start(out=w2t[:], in_=src)
                    for ki in range(NKGRP):
                        kk = kg * NKGRP + ki
                        for s in range(NSJ):
                            nc.tensor.matmul(
                                out=pys[s][:],
                                lhsT=ht[:, kk, s * P:(s + 1) * P],
                                rhs=w2t[:, ki, :],
                                start=False, stop=(kk == NM - 1),
                                skip_group_check=True,
                            )
                # prefetch d-half-1 w2 tiles while the n=0 tail accumulates
                for kg in range(NM // NKGRP):
                    w2t = w2_pool.tile([P, NKGRP, 512], BF16, name="w2t")
                    src = w2[:, 512:].rearrange(
                        "(g p) f -> p g f", p=P)[:, kg * NKGRP:(kg + 1) * NKGRP, :]
                    nc.gpsimd.dma_start(out=w2t[:], in_=src)
                    w2n1.append(w2t)
                for s in range(NSJ):
                    nc.scalar.activation(out=y_0[:, s, :], in_=pys[s][:],
                                         func=Act.Copy, bias=0.0, scale=1.0)
                    if s % 4 == 3:
                        h = s // 4
                        nc.gpsimd.dma_scatter_add(
                            out_row[:, :512],
                            y_0[:, h * 4:(h + 1) * 4, :],
                            idx16_sb[:, h * 32:(h + 1) * 32],
                            SEL // 2,
                            SEL // 2,
                            512,
                            elem_step=D,
                        )

                # d-half 1: s-major (each token block finishes early and its
                # rows scatter while the next block accumulates)
                y_1 = y_pool.tile([P, NSJ, 512], F32, name="y1")
                for s in range(NSJ):
                    py = mm2_psum.tile([P, 512], F32, name="py")
                    nc.tensor.matmul(
                        out=py[:], lhsT=o1x128b_sb[:], rhs=b2bf_sb[:, 512:],
                        start=True, stop=False, skip_group_check=True,
                    )
                    for kk in range(NM):
                        nc.tensor.matmul(
                            out=py[:],
                            lhsT=ht[:, kk, s * P:(s + 1) * P],
                            rhs=w2n1[kk // NKGRP][:, kk % NKGRP, :],
                            start=False, stop=(kk == NM - 1),
                            skip_group_check=True,
                        )
                    nc.scalar.activation(out=y_1[:, s, :], in_=py[:],
                                         func=Act.Copy, bias=0.0, scale=1.0)
                    nc.gpsimd.dma_scatter_add(
                        out_row[:, 512:],
                        y_1[:, s:s + 1, :],
                        idx16_sb[:, s * 8:(s + 1) * 8],
                        P,
                        P,
                        512,
                        elem_step=D,
                    )

        ht_ctx.__exit__(None, None, None)

    nc.compile()
    return nc


def make_consts():
    q = np.arange(P)
    import ml_dtypes
    consts = {
        "ltri128": (q[:, None] < q[None, :]).astype(np.float32),  # [q, p] = q < p
        "slt32": (np.arange(NT)[:, None] < np.arange(NT)[None, :]).astype(np.float32),
        "id32": np.eye(NT, dtype=np.float32),
        "ones_1x128": np.ones((1, P), np.float32),
        "ones_1x128b": np.ones((1, P), ml_dtypes.bfloat16),
        "ones_128x1": np.ones((P, 1), np.float32),
        "ones128": np.ones((P, P), np.float32),
        "ones_32x128": np.ones((NT, P), np.float32),
        "rep16": np.vstack([
            128.0 * (np.arange(16)[:, None] == (np.arange(P)[None, :] % 16)),
            1.0 * (np.arange(16)[:, None] == (np.arange(P)[None, :] % 16)),
        ]).astype(np.float32),
    }
    # ewrap[i, g*128 + p] = 1 iff p == g*16 + i  (16-wrap -> 128-wrap expand);
    # stacked [32, .]: rows 0:16 scaled by 128 (C chain), rows 16:32 raw (D)
    ew = np.zeros((16, 8 * P), np.float32)
    for i in range(16):
        for g in range(8):
            ew[i, g * P + g * 16 + i] = 1.0
    consts["ewrap"] = np.vstack([128.0 * ew, ew]).astype(np.float32)
    return consts


def make_in_maps(x, W1, b1, W2, b2, wr, br):
    consts = make_consts()
    x = np.ascontiguousarray(np.asarray(x, np.float32))
    in_maps = []
    for c in range(NCORES):
        b, h = divmod(c, 2)
        m = {
            "x_row": x[b],
            "w1": np.asarray(W1, np.float32),
            "w2": np.asarray(W2, np.float32),
            "wr": np.asarray(wr, np.float32).reshape(1, D),
            "b1t": np.ascontiguousarray(np.asarray(b1, np.float32).reshape(NM, P).T),
            "b2": np.asarray(b2, np.float32).reshape(1, D),
            "hbase": np.array([[h * SEL]], np.float32),
        }
        m.update(consts)
        in_maps.append(m)
    return in_maps


_NC_CACHE = None


def _get_program():
    global _NC_CACHE
    if _NC_CACHE is None:
        _NC_CACHE = build_program()
    return _NC_CACHE


def kernel(x, W1, b1, W2, b2, wr, br):
    from concourse.bass_utils import run_bass_kernel_spmd

    nc = _get_program()
    in_maps = make_in_maps(x, W1, b1, W2, b2, wr, br)
    res = run_bass_kernel_spmd(nc, in_maps, list(range(NCORES))).results
    out = np.stack(
        [res[2 * b]["out_row"] + res[2 * b + 1]["out_row"] for b in range(B)]
    )
    return out.astype(np.float32)


# revision 59
# speedup vs baseline: 2.5348x; 1.0483x over previous
"""MoD (mixture-of-depths) MLP wrapper kernel for Trainium2, 8 NeuronCores.

Sharding: core c handles batch row b = c//2 and the half of that row's
top-K tokens with global selection ranks in [h*1024, (h+1)*1024), h = c%2.
Each core computes the full row's router scores + top-K threshold locally
(no collectives), gathers exactly 1024 token rows by rank via indirect DMA,
runs the FFN in bf16 (fp32 accumulation), and scatters results back into the
pre-zeroed per-core output buffer with dma_scatter_add.  Host sums the two
buffers of each row.

Schedule: x-tile loads own the DMA engines first (weight loads are ordered
behind them); radix pass 1 folds into the score loop against a constant
threshold grid; passes 2-4 run as Sign-activation counts over a
DMA-broadcast score replica; rank compaction is a digit-decomposed one-hot
bf16 matmul whose stacked constants emit both the int32 gather and int16
scatter index layouts; gathered tokens are transposed by the DMA xbar
(dma_start_transpose); and the output scatter is dma_scatter_add (per-index
descriptors) overlapped with the tail of the second matmul.
"""

import sys

sys.path.insert(0, "/opt/trn_rl_repo")

from contextlib import ExitStack

import numpy as np

from concourse import bass, bass_isa, mybir
from concourse import bacc
import concourse.tile as tile
from concourse.bass import IndirectOffsetOnAxis

B, L, D = 4, 4096, 1024
DFF = 4 * D
K = L // 2              # 2048 selected tokens per row
NCORES = 8
P = 128
NT = L // P             # 32 token tiles per row
SEL = K // 2            # 1024 selected tokens per core
NSJ = SEL // P          # 8 selected-token blocks
ND = D // P             # 8 d chunks
NM = DFF // P           # 32 dff tiles
NKGRP = 4               # w2 k-chunks per streamed tile
RADIX_PASSES = 4

F32 = mybir.dt.float32
BF16 = mybir.dt.bfloat16
I32 = mybir.dt.int32
I16 = mybir.dt.int16
Alu = mybir.AluOpType
Act = mybir.ActivationFunctionType
Red = bass_isa.ReduceOp


def build_program():
    nc = bacc.Bacc(
        "TRN2",
        target_bir_lowering=False,
        debug=False,
        enable_asserts=False,
        num_devices=NCORES,
    )

    x_row = nc.dram_tensor("x_row", [L, D], F32, kind="ExternalInput").ap()
    w1 = nc.dram_tensor("w1", [D, DFF], F32, kind="ExternalInput").ap()
    w2 = nc.dram_tensor("w2", [DFF, D], F32, kind="ExternalInput").ap()
    wr = nc.dram_tensor("wr", [1, D], F32, kind="ExternalInput").ap()
    b1t = nc.dram_tensor("b1t", [P, NM], F32, kind="ExternalInput").ap()
    b2 = nc.dram_tensor("b2", [1, D], F32, kind="ExternalInput").ap()
    hbase = nc.dram_tensor("hbase", [1, 1], F32, kind="ExternalInput").ap()
    ltri = nc.dram_tensor("ltri128", [P, P], F32, kind="ExternalInput").ap()
    slt32 = nc.dram_tensor("slt32", [NT, NT], F32, kind="ExternalInput").ap()
    id32 = nc.dram_tensor("id32", [NT, NT], F32, kind="ExternalInput").ap()
    ones_1x128 = nc.dram_tensor("ones_1x128", [1, P], F32, kind="ExternalInput").ap()
    ones_1x128b = nc.dram_tensor("ones_1x128b", [1, P], BF16, kind="ExternalInput").ap()
    ones_128x1 = nc.dram_tensor("ones_128x1", [P, 1], F32, kind="ExternalInput").ap()
    ones128 = nc.dram_tensor("ones128", [P, P], F32, kind="ExternalInput").ap()
    ones_32x128 = nc.dram_tensor("ones_32x128", [NT, P], F32, kind="ExternalInput").ap()
    rep16 = nc.dram_tensor("rep16", [32, P], F32, kind="ExternalInput").ap()
    ewrap = nc.dram_tensor("ewrap", [32, 8 * P], F32, kind="ExternalInput").ap()

    out_row = nc.dram_tensor("out_row", [L, D], F32, kind="ExternalOutput").ap()

    GRPS = ((0, 20), (20, 30), (30, 31), (31, 32))
    scores_dg = [nc.dram_tensor(f"scores_dg{i}", [hi - lo, P], F32).ap()
                 for i, (lo, hi) in enumerate(GRPS)]

    with tile.TileContext(nc) as tc, ExitStack() as S0:
        const = S0.enter_context(tc.tile_pool(name="const", bufs=1))
        # pool stack (LIFO): const | ht | w1 | dig | ...phases
        ht_ctx = tc.tile_pool(name="ht", bufs=1)
        ht_pool = ht_ctx.__enter__()
        ht = ht_pool.tile([P, NM, SEL], BF16)
        w1_ctx = tc.tile_pool(name="w1bf", bufs=1)
        w1_pool = w1_ctx.__enter__()

        def cload(pool, ap, shape, dtype=F32, name=None):
            t = pool.tile(shape, dtype, name=name)
            nc.sync.dma_start(out=t[:], in_=ap)
            return t

        # ---- SP-queue order: wr, o1, oc, hbase FIRST (phase A needs them) ---
        wr_sb = cload(const, wr, [1, D], name="c_wr")
        o1x128_sb = cload(const, ones_1x128, [1, P], name="c_o1")
        o128x1_sb = cload(const, ones_128x1, [P, 1], name="c_oc")
        ones128_sb = cload(const, ones128, [P, P], name="c_o128")
        hb_sb = cload(const, hbase, [1, 1], name="c_hb")

        # w1 tiles exist from the start (loads are issued after the radix)
        w1bf = [w1_pool.tile([P, DFF], BF16, name=f"w1bf_{kd}")
                for kd in range(ND)]

        # ---- Pool-queue iotas (independent of SP queue) ---------------------
        # big digit-decomposition iota tables live only through phase E
        dig_ctx = tc.tile_pool(name="dig", bufs=1)
        dig = dig_ctx.__enter__()

        iota_i = const.tile([P, 1], I32)
        nc.gpsimd.iota(iota_i[:], pattern=[[1, 1]], base=0, channel_multiplier=1)
        tokid = const.tile([P, NT], I32)
        nc.gpsimd.iota(tokid[:], pattern=[[P, NT]], base=0, channel_multiplier=1)
        iC_i = const.tile([P, NT], I32)
        nc.gpsimd.iota(iC_i[:], pattern=[[1, NT]], base=0, channel_multiplier=0)
        iQ_i = const.tile([P, 128], I32)
        nc.gpsimd.iota(iQ_i[:], pattern=[[1, 128]], base=0, channel_multiplier=0)
        iK64_i = dig.tile([P, NT, 64], I16)
        nc.gpsimd.iota(iK64_i[:], pattern=[[0, NT], [1, 64]], base=0,
                       channel_multiplier=0)
        iJ16_i = dig.tile([P, NT, 16], I16)
        nc.gpsimd.iota(iJ16_i[:], pattern=[[0, NT], [1, 16]], base=0,
                       channel_multiplier=0)
        i7_i = const.tile([P, 7], I32)
        nc.gpsimd.iota(i7_i[:], pattern=[[1, 7]], base=1, channel_multiplier=0)

        iota_f = const.tile([P, 1], F32)
        nc.vector.tensor_copy(out=iota_f[:], in_=iota_i[:])
        tokidf = const.tile([P, NT], F32)
        nc.vector.tensor_copy(out=tokidf[:], in_=tokid[:])
        cvalf = const.tile([P, NT], F32)
        nc.vector.tensor_copy(out=cvalf[:], in_=iC_i[:])
        iK64b = dig.tile([P, NT, 64], BF16)
        nc.vector.tensor_copy(out=iK64b[:], in_=iK64_i[:])
        iJ16b = dig.tile([P, NT, 16], BF16)
        nc.vector.tensor_copy(out=iJ16b[:], in_=iJ16_i[:])
        iotab = const.tile([P, 1], BF16)
        nc.vector.tensor_copy(out=iotab[:], in_=iota_i[:])
        cvalb = const.tile([P, NT], BF16)
        nc.vector.tensor_copy(out=cvalb[:], in_=iC_i[:])
        i7f = const.tile([P, 7], F32)
        nc.vector.tensor_copy(out=i7f[:], in_=i7_i[:])
        thr128 = const.tile([P, 7], F32)
        nc.vector.tensor_scalar(out=thr128[:], in0=i7f[:], scalar1=128.0,
                                scalar2=None, op0=Alu.mult)
        thr16 = const.tile([P, 7], F32)
        nc.vector.tensor_scalar(out=thr16[:], in0=i7f[:], scalar1=16.0,
                                scalar2=None, op0=Alu.mult)
        # radix pass-1 threshold grid (build-time constants: lo=-16, w=0.25)
        iQf = const.tile([P, 128], F32)
        nc.vector.tensor_copy(out=iQf[:], in_=iQ_i[:])
        thr1row = const.tile([P, 128], F32)
        nc.vector.tensor_scalar(out=thr1row[:], in0=iQf[:], scalar1=32.0 / P,
                                scalar2=-16.0, op0=Alu.mult, op1=Alu.add)
        # negated per-pass threshold offsets for radix passes 2..4
        W1P = 32.0 / P
        nthrbs = []
        for p_ in range(1, RADIX_PASSES):
            w_p = W1P / (P ** p_)
            t_ = const.tile([P, 1], F32, name=f"nthrb{p_}")
            nc.vector.tensor_scalar(out=t_[:], in0=iota_f[:], scalar1=-w_p,
                                    scalar2=None, op0=Alu.mult)
            nthrbs.append((w_p, t_))
        hb_col = const.tile([P, 1], F32)
        nc.gpsimd.partition_broadcast(hb_col[:], hb_sb[:])

        scores_sb = const.tile([P, NT], F32)
        selidx_sb = const.tile([P, NSJ], I32)
        idx16_sb = const.tile([P, SEL // 16], I16)

        misc_psum_ctx = tc.tile_pool(name="misc_psum", bufs=2, space="PSUM")
        misc_psum = misc_psum_ctx.__enter__()

        # ---- phase A: router scores (fp32, exact; router bias dropped — it
        # shifts every score equally so the top-K set is unchanged).  The
        # first radix pass uses a build-time-constant threshold grid, so its
        # per-tile compare + count-matmul accumulation is folded in here. -----
        c1_psum_ctx = tc.tile_pool(name="c1_psum", bufs=1, space="PSUM")
        c1_psum = c1_psum_ctx.__enter__()
        cnt1_ps = c1_psum.tile([P, 128], F32, name="cnt1")
        nlo = const.tile([P, 1], F32, name="nlo")
        with ExitStack() as SA:
            apool = SA.enter_context(tc.tile_pool(name="apool", bufs=1))
            xs_pool = SA.enter_context(tc.tile_pool(name="xs", bufs=6))
            junk_pool = SA.enter_context(tc.tile_pool(name="junk", bufs=2))
            cmp_pool = SA.enter_context(tc.tile_pool(name="cmp", bufs=3))

            wrb = apool.tile([P, D], F32)
            for n in range(D // 512):
                pt = misc_psum.tile([P, 512], F32, name="mp")
                nc.tensor.matmul(out=pt[:], lhsT=o1x128_sb[:],
                                 rhs=wr_sb[:, n * 512:(n + 1) * 512],
                                 start=True, stop=True)
                nc.vector.tensor_copy(out=wrb[:, n * 512:(n + 1) * 512], in_=pt[:])

            x_last = None
            for t in range(NT):
                x_t = xs_pool.tile([P, D], F32)
                nc.sync.dma_start(out=x_t[:], in_=x_row[t * P:(t + 1) * P, :])
                x_last = x_t
                if t == 26:
                    nc.sync.dma_start(
                        out=scores_dg[0].rearrange("c p -> p c"),
                        in_=scores_sb[:, 0:20])
                prod = junk_pool.tile([P, D], F32, name="prod")
                nc.vector.tensor_tensor(out=prod[:], in0=x_t[:], in1=wrb[:],
                                        op=Alu.mult)
                sink = junk_pool.tile([P, D], BF16, name="sink")
                nc.scalar.activation(out=sink[:], in_=prod[:], func=Act.Identity,
                                     bias=0.0, scale=1.0,
                                     accum_out=scores_sb[:, t:t + 1])
                cmp_t = cmp_pool.tile([P, 128], F32, name="cmp")
                nc.vector.tensor_tensor(
                    out=cmp_t[:],
                    in0=scores_sb[:, t:t + 1].to_broadcast([P, 128]),
                    in1=thr1row[:], op=Alu.is_ge)
                nc.tensor.matmul(out=cnt1_ps[:], lhsT=ones128_sb[:], rhs=cmp_t[:],
                                 start=(t == 0), stop=(t == NT - 1),
                                 skip_group_check=True)


            # pass-1 finalize on every partition (count matmul used an
            # all-ones lhsT, so each partition holds the full count row):
            # nlo = -(lo1) = 16 - (sum(cnt>=K) - 1)*0.25
            selr = apool.tile([P, 128], F32, name="selr")
            nc.vector.tensor_scalar(out=selr[:], in0=cnt1_ps[:],
                                    scalar1=float(K), scalar2=None,
                                    op0=Alu.is_ge)
            s1 = apool.tile([P, 1], F32, name="s1")
            nc.vector.tensor_reduce(out=s1[:], in_=selr[:],
                                    axis=mybir.AxisListType.X, op=Alu.add)
            q1 = apool.tile([P, 1], F32, name="q1")
            nc.vector.tensor_scalar(out=q1[:], in0=s1[:], scalar1=-1.0,
                                    scalar2=-W1P, op0=Alu.add, op1=Alu.mult)
            nc.vector.tensor_scalar(out=nlo[:], in0=q1[:], scalar1=16.0,
                                    scalar2=None, op0=Alu.add)
        c1_psum_ctx.__exit__(None, None, None)

        offf_c = const.tile([P, NT], F32)
        maskf_c = const.tile([P, NT], F32)

        # ---- phases B+C+D: replicate scores, radix threshold, rank ----------
        with ExitStack() as SC:
            radix = SC.enter_context(tc.tile_pool(name="radix", bufs=2))
            rep_pool = SC.enter_context(tc.tile_pool(name="rep", bufs=1))

            # broadcast-read the spilled scores, one DMA per 1024-token group,
            # FIRST on the in-order SP queue right after the x loads (the
            # remaining const loads queue behind, they aren't needed till later)
            scores_rep = rep_pool.tile([P, L], F32)
            for gi, (glo, ghi) in enumerate(GRPS):
                n_ = (ghi - glo) * P
                if gi > 0:   # g0 was spilled inside the x stream
                    nc.sync.dma_start(
                        out=scores_dg[gi].rearrange("c p -> p c"),
                        in_=scores_sb[:, glo:ghi])
                nc.sync.dma_start(
                    out=scores_rep[:, glo * P:ghi * P],
                    in_=scores_dg[gi].rearrange("c p -> () (c p)")
                    .to_broadcast([P, n_]))

            # gate the w1 cast-loads behind the score broadcast so their DMAs
            # cannot delay it (WAW edge: the w1 DMA overwrites the gate byte)
            for kd in range(ND):
                nc.vector.tensor_copy(out=w1bf[kd][0:1, 0:1],
                                      in_=scores_rep[0:1, kd:kd + 1])

            # ---- remaining small consts on the SP queue ---------------------
            b1t_sb = cload(const, b1t, [P, NM], name="c_b1t")
            ltri_sb = cload(const, ltri, [P, P], name="c_lt")
            slt32_sb = cload(const, slt32, [NT, NT], name="c_sl")
            id32_sb = cload(const, id32, [NT, NT], name="c_id32")
            o1x128b_sb = cload(const, ones_1x128b, [1, P], BF16, name="c_o1b")
            o32x128_sb = cload(const, ones_32x128, [NT, P], name="c_o32")
            rep16_sb = cload(const, rep16, [32, P], name="c_rep16")
            ewrap_sb = cload(const, ewrap, [32, 8 * P], name="c_ew")
            b2bf_sb = const.tile([1, D], BF16)
            nc.gpsimd.dma_start(out=b2bf_sb[:], in_=b2)  # cast f32 -> bf16

            sjunk = rep_pool.tile([P, L], BF16, name="sjunk")
            for w_p, nthrb_p in nthrbs:
                nthr = radix.tile([P, 1], F32, name="nthr")
                nc.vector.tensor_tensor(out=nthr[:], in0=nlo[:], in1=nthrb_p[:],
                                        op=Alu.add)
                sgn = radix.tile([P, 1], F32, name="sgn")
                nc.scalar.activation(out=sjunk[:], in_=scores_rep[:],
                                     func=Act.Sign, bias=nthr[:, :1], scale=1.0,
                                     accum_out=sgn[:])
                sel = radix.tile([P, 1], F32, name="sel")
                nc.vector.tensor_scalar(out=sel[:], in0=sgn[:], scalar1=0.0,
                                        scalar2=None, op0=Alu.is_ge)
                s_all = radix.tile([P, 1], F32, name="s_all")
                nc.gpsimd.partition_all_reduce(s_all[:], sel[:], channels=P,
                                               reduce_op=Red.add)
                nd = radix.tile([P, 1], F32, name="nd")
                nc.vector.tensor_scalar(out=nd[:], in0=s_all[:], scalar1=-1.0,
                                        scalar2=-w_p, op0=Alu.add, op1=Alu.mult)
                nlo2 = radix.tile([P, 1], F32, name="nlo2")
                nc.vector.tensor_tensor(out=nlo2[:], in0=nlo[:], in1=nd[:],
                                        op=Alu.add)
                nlo = nlo2

            # ---- mask + global rank (exclusive prefix of mask) --------------
            m0 = radix.tile([P, NT], F32, name="m0")
            nc.vector.tensor_tensor(out=m0[:], in0=scores_sb[:],
                                    in1=nlo[:, :1].to_broadcast([P, NT]),
                                    op=Alu.add)
            maskf = radix.tile([P, NT], F32, name="maskf")
            nc.vector.tensor_scalar(out=maskf[:], in0=m0[:], scalar1=0.0,
                                    scalar2=None, op0=Alu.is_ge)
            colsum_p = misc_psum.tile([NT, 1], F32, name="mp")
            nc.tensor.matmul(out=colsum_p[:], lhsT=maskf[:], rhs=o128x1_sb[:],
                             start=True, stop=True)
            colsum = radix.tile([NT, 1], F32, name="colsum")
            nc.vector.tensor_copy(out=colsum[:], in_=colsum_p[:])
            excl_p = misc_psum.tile([NT, 1], F32, name="mp")
            nc.tensor.matmul(out=excl_p[:], lhsT=slt32_sb[:], rhs=colsum[:],
                             start=True, stop=True)
            excl = radix.tile([NT, 1], F32, name="excl")
            nc.vector.tensor_copy(out=excl[:], in_=excl_p[:])
            diag = radix.tile([NT, NT], F32, name="diag")
            nc.vector.tensor_tensor(out=diag[:], in0=id32_sb[:],
                                    in1=excl[:, :1].to_broadcast([NT, NT]),
                                    op=Alu.mult)
            rank_p = misc_psum.tile([P, NT], F32, name="mp")
            nc.tensor.matmul(out=rank_p[:], lhsT=ltri_sb[:], rhs=maskf[:],
                             start=True, stop=False, skip_group_check=True)
            nc.tensor.matmul(out=rank_p[:], lhsT=o32x128_sb[:], rhs=diag[:],
                             start=False, stop=True, skip_group_check=True)
            rank = radix.tile([P, NT], F32, name="rank")
            nc.vector.tensor_copy(out=rank[:], in_=rank_p[:])
            off = radix.tile([P, NT], F32, name="off")
            nc.vector.tensor_tensor(out=off[:], in0=rank[:],
                                    in1=hb_col[:, :1].to_broadcast([P, NT]),
                                    op=Alu.subtract)
            nc.vector.tensor_copy(out=offf_c[:], in_=off[:])
            nc.vector.tensor_copy(out=maskf_c[:], in_=maskf[:])

        misc_psum_ctx.__exit__(None, None, None)

        # ---- w1 cast-loads on the Pool queue.  Positioned after the radix
        # all_reduces so the in-order queue starts them only ~70us in, after
        # the x-tile DMAs have drained (they'd otherwise steal DMA bandwidth
        # from the critical-path score loads). ---------------------------------
        w1bf = []
        for kd in range(ND):
            t_ = w1_pool.tile([P, DFF], BF16, name=f"w1bf_{kd}")
            nc.gpsimd.dma_start(out=t_[:], in_=w1[kd * P:(kd + 1) * P, :])
            w1bf.append(t_)

        # ---- phase E: digit split + one-hot compaction matmuls --------------
        # off in [0, SEL) for in-window selected tokens; any other off value
        # (negative rank-window miss, >=SEL, or collision of an unselected
        # token) produces no match in the lo-digit equality below, and
        # unselected tokens are additionally zeroed via tokid*mask weights.
        with ExitStack() as SE:
            ep = SE.enter_context(tc.tile_pool(name="epool", bufs=1))
            e_psum = SE.enter_context(tc.tile_pool(name="e_psum", bufs=2,
                                                   space="PSUM"))
            off = offf_c
            eq7a = ep.tile([P, NT, 7], F32, name="eq7a")
            nc.vector.tensor_tensor(
                out=eq7a[:], in0=off[:, :, None].to_broadcast([P, NT, 7]),
                in1=thr128[:, None, :].to_broadcast([P, NT, 7]), op=Alu.is_ge)
            hi128 = ep.tile([P, NT], F32, name="hi128")
            nc.vector.tensor_reduce(out=hi128[:], in_=eq7a[:],
                                    axis=mybir.AxisListType.X, op=Alu.add)
            hm = ep.tile([P, NT], F32, name="hm")
            nc.vector.tensor_scalar(out=hm[:], in0=hi128[:], scalar1=-128.0,
                                    scalar2=None, op0=Alu.mult)
            lo128 = ep.tile([P, NT], F32, name="lo128")
            nc.vector.tensor_tensor(out=lo128[:], in0=off[:], in1=hm[:],
                                    op=Alu.add)
            eq7b = ep.tile([P, NT, 7], F32, name="eq7b")
            nc.vector.tensor_tensor(
                out=eq7b[:], in0=lo128[:, :, None].to_broadcast([P, NT, 7]),
                in1=thr16[:, None, :].to_broadcast([P, NT, 7]), op=Alu.is_ge)
            mid = ep.tile([P, NT], F32, name="mid")
            nc.vector.tensor_reduce(out=mid[:], in_=eq7b[:],
                                    axis=mybir.AxisListType.X, op=Alu.add)
            hm2 = ep.tile([P, NT], F32, name="hm2")
            nc.vector.tensor_scalar(out=hm2[:], in0=mid[:], scalar1=-16.0,
                                    scalar2=None, op0=Alu.mult)
            lo16 = ep.tile([P, NT], F32, name="lo16")
            nc.vector.tensor_tensor(out=lo16[:], in0=lo128[:], in1=hm2[:],
                                    op=Alu.add)
            h8 = ep.tile([P, NT], F32, name="h8")
            nc.vector.tensor_scalar(out=h8[:], in0=hi128[:], scalar1=8.0,
                                    scalar2=None, op0=Alu.mult)
            hi16 = ep.tile([P, NT], F32, name="hi16")
            nc.vector.tensor_tensor(out=hi16[:], in0=h8[:], in1=mid[:],
                                    op=Alu.add)
            # token id = c*128 + p; weight the SMALL equality factors by
            # c*mask (chain C, lhsT cols 0:16) and p*mask (chain D, cols
            # 16:32), then sel16 = 128*C + D.  All factors are small exact
            # integers, so the chain runs in bf16 (1 cycle/row matmuls).
            maskb = ep.tile([P, NT], BF16, name="maskb")
            nc.vector.tensor_copy(out=maskb[:], in_=maskf_c[:])
            cwm = ep.tile([P, NT], BF16, name="cwm")
            nc.vector.tensor_tensor(out=cwm[:], in0=cvalb[:], in1=maskb[:],
                                    op=Alu.mult)
            pwm = ep.tile([P, NT], BF16, name="pwm")
            nc.vector.tensor_tensor(out=pwm[:], in0=maskb[:],
                                    in1=iotab[:, :1].to_broadcast([P, NT]),
                                    op=Alu.mult)
            lo16b = ep.tile([P, NT], BF16, name="lo16b")
            nc.vector.tensor_copy(out=lo16b[:], in_=lo16[:])
            hi16b = ep.tile([P, NT], BF16, name="hi16b")
            nc.vector.tensor_copy(out=hi16b[:], in_=hi16[:])

            eq16 = ep.tile([P, NT, 16], BF16, name="eq16")
            nc.vector.tensor_tensor(
                out=eq16[:], in0=iJ16b[:],
                in1=lo16b[:, :, None].to_broadcast([P, NT, 16]), op=Alu.is_equal)
            eqcp = ep.tile([P, NT, 32], BF16, name="eqcp")
            nc.vector.tensor_tensor(
                out=eqcp[:, :, 0:16], in0=eq16[:],
                in1=cwm[:, :, None].to_broadcast([P, NT, 16]), op=Alu.mult)
            nc.vector.tensor_tensor(
                out=eqcp[:, :, 16:32], in0=eq16[:],
                in1=pwm[:, :, None].to_broadcast([P, NT, 16]), op=Alu.mult)
            eq64 = ep.tile([P, NT, 64], BF16, name="eq64")
            nc.vector.tensor_tensor(
                out=eq64[:], in0=iK64b[:],
                in1=hi16b[:, :, None].to_broadcast([P, NT, 64]), op=Alu.is_equal)

            pCD = e_psum.tile([32, 64], F32, name="pCD")
            for c in range(NT):
                nc.tensor.matmul(out=pCD[:], lhsT=eqcp[:, c, :],
                                 rhs=eq64[:, c, :], start=(c == 0),
                                 stop=(c == NT - 1), skip_group_check=True)

            sCD = ep.tile([32, 64], F32, name="sCD")
            nc.vector.tensor_copy(out=sCD[:], in_=pCD[:])

            # scatter index layout [128, 64] (16-wrap replicated to 128);
            # lhsT folds the 128*C + D combine (rows 0:16 scaled by 128)
            rep_ps = e_psum.tile([P, 64], F32, name="rep_ps")
            nc.tensor.matmul(out=rep_ps[:], lhsT=rep16_sb[:], rhs=sCD[:],
                             start=True, stop=True)
            nc.vector.tensor_copy(out=idx16_sb[:], in_=rep_ps[:])  # f32->i16

            # gather index layout [128, 8]: selidx[p, k] = sel16[p%16, 8k+p//16]
            selps = e_psum.tile([P, NSJ], F32, name="selps")
            for g in range(8):
                nc.tensor.matmul(out=selps[:], lhsT=ewrap_sb[:, g * P:(g + 1) * P],
                                 rhs=sCD[:, g::8], start=(g == 0),
                                 stop=(g == 7), skip_group_check=True)
            nc.vector.tensor_copy(out=selidx_sb[:], in_=selps[:])  # f32->i32

        dig_ctx.__exit__(None, None, None)

        # ---- gather + transpose + MLP ---------------------------------------
        if True:
            with ExitStack() as SB:
                xt_pool = SB.enter_context(tc.tile_pool(name="xt", bufs=1))
                xsel_pool = SB.enter_context(tc.tile_pool(name="xsel", bufs=5))
                mm1_psum = SB.enter_context(tc.tile_pool(name="mm1_psum", bufs=6,
                                                         space="PSUM"))

                # xt3[p, kd, t] = x_sel[t, kd*128+p], built by the DMA-engine
                # xbar transpose (one per gathered 128-token chunk)
                xt3 = xt_pool.tile([P, ND, SEL], BF16)
                for j in range(NSJ):
                    xs = xsel_pool.tile([P, D], BF16, name="xsel")
                    nc.gpsimd.indirect_dma_start(
                        out=xs[:], out_offset=None, in_=x_row,
                        in_offset=IndirectOffsetOnAxis(ap=selidx_sb[:, j:j + 1],
                                                       axis=0))
                    nc.scalar.dma_start_transpose(
                        out=xt3[:, :, j * P:(j + 1) * P], in_=xs[:])

                # ---- mm1: ht[m, sel] = gelu(w1^T x_sel^T + b1).  The first
                # four token blocks are 128 wide so the PE starts the moment
                # each transpose lands instead of waiting for four of them;
                # the second half runs as one 512-wide block.
                for t0, tw in [(0, P), (P, P), (2 * P, P), (3 * P, P),
                               (512, 512)]:
                    for m in range(NM):
                        ph = mm1_psum.tile([P, tw], F32, name="ph")
                        for kd in range(ND):
                            nc.tensor.matmul(
                                out=ph[:],
                                lhsT=w1bf[kd][:, m * P:(m + 1) * P],
                                rhs=xt3[:, kd, t0:t0 + tw],
                                start=(kd == 0), stop=(kd == ND - 1),
                            )
                        nc.scalar.activation(
                            out=ht[:, m, t0:t0 + tw], in_=ph[:],
                            func=Act.Gelu_apprx_tanh, bias=b1t_sb[:, m:m + 1],
                            scale=1.0,
                        )

            w1_ctx.__exit__(None, None, None)  # free w1 region for w2 stream

            # ---- mm2: y[sel, D] = ht^T @ w2 + b2, then scatter-add ----------
            with ExitStack() as SY:
                y_pool = SY.enter_context(tc.tile_pool(name="y", bufs=1))
                w2_pool = SY.enter_context(tc.tile_pool(name="w2s", bufs=16))
                mm2_psum = SY.enter_context(tc.tile_pool(name="mm2_psum", bufs=8,
                                                         space="PSUM"))
                # d-half 0: kg-major accumulation (w2 tiles stream in, all 8
                # token-block psums accumulate together)
                n = 0
                y_0 = y_pool.tile([P, NSJ, 512], F32, name="y0")
                pys = [mm2_psum.tile([P, 512], F32, name="py")
                       for _ in range(NSJ)]
                w2n1 = []   # d-half-1 tiles retained for the s-major pass
                for s in range(NSJ):
                    nc.tensor.matmul(
                        out=pys[s][:], lhsT=o1x128b_sb[:],
                        rhs=b2bf_sb[:, :512],
                        start=True, stop=False, skip_group_check=True,
                    )
                for kg in range(NM // NKGRP):
                    w2t = w2_pool.tile([P, NKGRP, 512], BF16, name="w2t")
                    if kg == 0:
                        # WAW gate: keep the w2 stream off the DMA engines
                        # until the gather/transpose pipeline has fed mm1
                        nc.vector.tensor_copy(out=w2t[0:1, 0, 0:1],
                                              in_=ht[0:1, 0, 0:1])
                    src = w2[:, :512].rearrange(
                        "(g p) f -> p g f", p=P)[:, kg * NKGRP:(kg + 1) * NKGRP, :]
                    nc.gpsimd.dma_start(out=w2t[:], in_=src)
                    for ki in range(NKGRP):
                        kk = kg * NKGRP + ki
                        for s in range(NSJ):
                            nc.tensor.matmul(
                                out=pys[s][:],
                                lhsT=ht[:, kk, s * P:(s + 1) * P],
                                rhs=w2t[:, ki, :],
                                start=False, stop=(kk == NM - 1),
                                skip_group_check=True,
                            )
                # prefetch d-half-1 w2 tiles while the n=0 tail accumulates
                for kg in range(NM // NKGRP):
                    w2t = w2_pool.tile([P, NKGRP, 512], BF16, name="w2t")
                    src = w2[:, 512:].rearrange(
                        "(g p) f -> p g f", p=P)[:, kg * NKGRP:(kg + 1) * NKGRP, :]
                    nc.gpsimd.dma_start(out=w2t[:], in_=src)
                    w2n1.append(w2t)
                for s in range(NSJ):
                    nc.scalar.activation(out=y_0[:, s, :], in_=pys[s][:],
                                         func=Act.Copy, bias=0.0, scale=1.0)
                    if s % 4 == 3:
                        h = s // 4
                        nc.gpsimd.dma_scatter_add(
                            out_row[:, :512],
                            y_0[:, h * 4:(h + 1) * 4, :],
                            idx16_sb[:, h * 32:(h + 1) * 32],
                            SEL // 2,
                            SEL // 2,
                            512,
                            elem_step=D,
                        )

                # d-half 1: s-major (each token block finishes early and its
                # rows scatter while the next block accumulates)
                y_1 = y_pool.tile([P, NSJ, 512], F32, name="y1")
                for s in range(NSJ):
                    py = mm2_psum.tile([P, 512], F32, name="py")
                    nc.tensor.matmul(
                        out=py[:], lhsT=o1x128b_sb[:], rhs=b2bf_sb[:, 512:],
                        start=True, stop=False, skip_group_check=True,
                    )
                    for kk in range(NM):
                        nc.tensor.matmul(
                            out=py[:],
                            lhsT=ht[:, kk, s * P:(s + 1) * P],
                            rhs=w2n1[kk // NKGRP][:, kk % NKGRP, :],
                            start=False, stop=(kk == NM - 1),
                            skip_group_check=True,
                        )
                    nc.scalar.activation(out=y_1[:, s, :], in_=py[:],
                                         func=Act.Copy, bias=0.0, scale=1.0)
                    nc.gpsimd.dma_scatter_add(
                        out_row[:, 512:],
                        y_1[:, s:s + 1, :],
                        idx16_sb[:, s * 8:(s + 1) * 8],
                        P,
                        P,
                        512,
                        elem_step=D,
                    )

        ht_ctx.__exit__(None, None, None)

    nc.compile()
    return nc


def make_consts():
    q = np.arange(P)
    import ml_dtypes
    consts = {
        "ltri128": (q[:, None] < q[None, :]).astype(np.float32),  # [q, p] = q < p
        "slt32": (np.arange(NT)[:, None] < np.arange(NT)[None, :]).astype(np.float32),
        "id32": np.eye(NT, dtype=np.float32),
        "ones_1x128": np.ones((1, P), np.float32),
        "ones_1x128b": np.ones((1, P), ml_dtypes.bfloat16),
        "ones_128x1": np.ones((P, 1), np.float32),
        "ones128": np.ones((P, P), np.float32),
        "ones_32x128": np.ones((NT, P), np.float32),
        "rep16": np.vstack([
            128.0 * (np.arange(16)[:, None] == (np.arange(P)[None, :] % 16)),
            1.0 * (np.arange(16)[:, None] == (np.arange(P)[None, :] % 16)),
        ]).astype(np.float32),
    }
    # ewrap[i, g*128 + p] = 1 iff p == g*16 + i  (16-wrap -> 128-wrap expand);
    # stacked [32, .]: rows 0:16 scaled by 128 (C chain), rows 16:32 raw (D)
    ew = np.zeros((16, 8 * P), np.float32)
    for i in range(16):
        for g in range(8):
            ew[i, g * P + g * 16 + i] = 1.0
    consts["ewrap"] = np.vstack([128.0 * ew, ew]).astype(np.float32)
    return consts


def make_in_maps(x, W1, b1, W2, b2, wr, br):
    consts = make_consts()
    x = np.ascontiguousarray(np.asarray(x, np.float32))
    in_maps = []
    for c in range(NCORES):
        b, h = divmod(c, 2)
        m = {
            "x_row": x[b],
            "w1": np.asarray(W1, np.float32),
            "w2": np.asarray(W2, np.float32),
            "wr": np.asarray(wr, np.float32).reshape(1, D),
            "b1t": np.ascontiguousarray(np.asarray(b1, np.float32).reshape(NM, P).T),
            "b2": np.asarray(b2, np.float32).reshape(1, D),
            "hbase": np.array([[h * SEL]], np.float32),
        }
        m.update(consts)
        in_maps.append(m)
    return in_maps


_NC_CACHE = None


def _get_program():
    global _NC_CACHE
    if _NC_CACHE is None:
        _NC_CACHE = build_program()
    return _NC_CACHE


def kernel(x, W1, b1, W2, b2, wr, br):
    from concourse.bass_utils import run_bass_kernel_spmd

    nc = _get_program()
    in_maps = make_in_maps(x, W1, b1, W2, b2, wr, br)
    res = run_bass_kernel_spmd(nc, in_maps, list(range(NCORES))).results
    out = np.stack(
        [res[2 * b]["out_row"] + res[2 * b + 1]["out_row"] for b in range(B)]
    )
    return out.astype(np.float32)


# revision 62
# speedup vs baseline: 2.5817x; 1.0185x over previous
"""MoD (mixture-of-depths) MLP wrapper kernel for Trainium2, 8 NeuronCores.

Sharding: core c handles batch row b = c//2 and the half of that row's
top-K tokens with global selection ranks in [h*1024, (h+1)*1024), h = c%2.
Each core computes the full row's router scores + top-K threshold locally
(no collectives), gathers exactly 1024 token rows by rank via indirect DMA,
runs the FFN in bf16 (fp32 accumulation), and scatters results back into the
pre-zeroed per-core output buffer with dma_scatter_add.  Host sums the two
buffers of each row.

Schedule: x-tile loads own the DMA engines first (weight loads are ordered
behind them); radix pass 1 folds into the score loop against a constant
threshold grid; passes 2-4 run as Sign-activation counts over a
DMA-broadcast score replica; rank compaction is a digit-decomposed one-hot
bf16 matmul whose stacked constants emit both the int32 gather and int16
scatter index layouts; gathered tokens are transposed by the DMA xbar
(dma_start_transpose); and the output scatter is dma_scatter_add (per-index
descriptors) overlapped with the tail of the second matmul.
"""

import sys

sys.path.insert(0, "/opt/trn_rl_repo")

from contextlib import ExitStack

import numpy as np

from concourse import bass, bass_isa, mybir
from concourse import bacc
import concourse.tile as tile
from concourse.bass import IndirectOffsetOnAxis

B, L, D = 4, 4096, 1024
DFF = 4 * D
K = L // 2              # 2048 selected tokens per row
NCORES = 8
P = 128
NT = L // P             # 32 token tiles per row
SEL = K // 2            # 1024 selected tokens per core
NSJ = SEL // P          # 8 selected-token blocks
ND = D // P             # 8 d chunks
NM = DFF // P           # 32 dff tiles
NKGRP = 4               # w2 k-chunks per streamed tile
RADIX_PASSES = 4

F32 = mybir.dt.float32
BF16 = mybir.dt.bfloat16
I32 = mybir.dt.int32
I16 = mybir.dt.int16
Alu = mybir.AluOpType
Act = mybir.ActivationFunctionType
Red = bass_isa.ReduceOp


def build_program():
    nc = bacc.Bacc(
        "TRN2",
        target_bir_lowering=False,
        debug=False,
        enable_asserts=False,
        num_devices=NCORES,
    )

    x_row = nc.dram_tensor("x_row", [L, D], F32, kind="ExternalInput").ap()
    w1 = nc.dram_tensor("w1", [D, DFF], F32, kind="ExternalInput").ap()
    w2 = nc.dram_tensor("w2", [DFF, D], F32, kind="ExternalInput").ap()
    wr = nc.dram_tensor("wr", [1, D], F32, kind="ExternalInput").ap()
    b1t = nc.dram_tensor("b1t", [P, NM], F32, kind="ExternalInput").ap()
    b2 = nc.dram_tensor("b2", [1, D], F32, kind="ExternalInput").ap()
    hbase = nc.dram_tensor("hbase", [1, 1], F32, kind="ExternalInput").ap()
    identb = nc.dram_tensor("identb", [P, P], BF16, kind="ExternalInput").ap()
    ltri = nc.dram_tensor("ltri128", [P, P], F32, kind="ExternalInput").ap()
    slt32 = nc.dram_tensor("slt32", [NT, NT], F32, kind="ExternalInput").ap()
    id32 = nc.dram_tensor("id32", [NT, NT], F32, kind="ExternalInput").ap()
    ones_1x128 = nc.dram_tensor("ones_1x128", [1, P], F32, kind="ExternalInput").ap()
    ones_1x128b = nc.dram_tensor("ones_1x128b", [1, P], BF16, kind="ExternalInput").ap()
    ones_128x1 = nc.dram_tensor("ones_128x1", [P, 1], F32, kind="ExternalInput").ap()
    ones128 = nc.dram_tensor("ones128", [P, P], F32, kind="ExternalInput").ap()
    ones_32x128 = nc.dram_tensor("ones_32x128", [NT, P], F32, kind="ExternalInput").ap()
    rep16 = nc.dram_tensor("rep16", [32, P], F32, kind="ExternalInput").ap()
    ewrap = nc.dram_tensor("ewrap", [32, 8 * P], F32, kind="ExternalInput").ap()

    out_row = nc.dram_tensor("out_row", [L, D], F32, kind="ExternalOutput").ap()

    GRPS = ((0, 20), (20, 30), (30, 31), (31, 32))
    scores_dg = [nc.dram_tensor(f"scores_dg{i}", [hi - lo, P], F32).ap()
                 for i, (lo, hi) in enumerate(GRPS)]

    with tile.TileContext(nc) as tc, ExitStack() as S0:
        const = S0.enter_context(tc.tile_pool(name="const", bufs=1))
        # pool stack (LIFO): const | ht | w1 | dig | ...phases
        ht_ctx = tc.tile_pool(name="ht", bufs=1)
        ht_pool = ht_ctx.__enter__()
        ht = ht_pool.tile([P, NM, SEL], BF16)
        w1_ctx = tc.tile_pool(name="w1bf", bufs=1)
        w1_pool = w1_ctx.__enter__()

        def cload(pool, ap, shape, dtype=F32, name=None):
            t = pool.tile(shape, dtype, name=name)
            nc.sync.dma_start(out=t[:], in_=ap)
            return t

        # ---- SP-queue order: wr, o1, oc, hbase FIRST (phase A needs them) ---
        wr_sb = cload(const, wr, [1, D], name="c_wr")
        o1x128_sb = cload(const, ones_1x128, [1, P], name="c_o1")
        o128x1_sb = cload(const, ones_128x1, [P, 1], name="c_oc")
        ones128_sb = cload(const, ones128, [P, P], name="c_o128")
        hb_sb = cload(const, hbase, [1, 1], name="c_hb")

        # w1 tiles exist from the start (loads are issued after the radix)
        w1bf = [w1_pool.tile([P, DFF], BF16, name=f"w1bf_{kd}")
                for kd in range(ND)]

        # ---- Pool-queue iotas (independent of SP queue) ---------------------
        # big digit-decomposition iota tables live only through phase E
        dig_ctx = tc.tile_pool(name="dig", bufs=1)
        dig = dig_ctx.__enter__()

        iota_i = const.tile([P, 1], I32)
        nc.gpsimd.iota(iota_i[:], pattern=[[1, 1]], base=0, channel_multiplier=1)
        tokid = const.tile([P, NT], I32)
        nc.gpsimd.iota(tokid[:], pattern=[[P, NT]], base=0, channel_multiplier=1)
        iC_i = const.tile([P, NT], I32)
        nc.gpsimd.iota(iC_i[:], pattern=[[1, NT]], base=0, channel_multiplier=0)
        iQ_i = const.tile([P, 128], I32)
        nc.gpsimd.iota(iQ_i[:], pattern=[[1, 128]], base=0, channel_multiplier=0)
        iK64_i = dig.tile([P, NT, 64], I16)
        nc.gpsimd.iota(iK64_i[:], pattern=[[0, NT], [1, 64]], base=0,
                       channel_multiplier=0)
        iJ16_i = dig.tile([P, NT, 16], I16)
        nc.gpsimd.iota(iJ16_i[:], pattern=[[0, NT], [1, 16]], base=0,
                       channel_multiplier=0)
        i7_i = const.tile([P, 7], I32)
        nc.gpsimd.iota(i7_i[:], pattern=[[1, 7]], base=1, channel_multiplier=0)

        iota_f = const.tile([P, 1], F32)
        nc.vector.tensor_copy(out=iota_f[:], in_=iota_i[:])
        tokidf = const.tile([P, NT], F32)
        nc.vector.tensor_copy(out=tokidf[:], in_=tokid[:])
        cvalf = const.tile([P, NT], F32)
        nc.vector.tensor_copy(out=cvalf[:], in_=iC_i[:])
        iK64b = dig.tile([P, NT, 64], BF16)
        nc.vector.tensor_copy(out=iK64b[:], in_=iK64_i[:])
        iJ16b = dig.tile([P, NT, 16], BF16)
        nc.vector.tensor_copy(out=iJ16b[:], in_=iJ16_i[:])
        iotab = const.tile([P, 1], BF16)
        nc.vector.tensor_copy(out=iotab[:], in_=iota_i[:])
        cvalb = const.tile([P, NT], BF16)
        nc.vector.tensor_copy(out=cvalb[:], in_=iC_i[:])
        i7f = const.tile([P, 7], F32)
        nc.vector.tensor_copy(out=i7f[:], in_=i7_i[:])
        thr128 = const.tile([P, 7], F32)
        nc.vector.tensor_scalar(out=thr128[:], in0=i7f[:], scalar1=128.0,
                                scalar2=None, op0=Alu.mult)
        thr16 = const.tile([P, 7], F32)
        nc.vector.tensor_scalar(out=thr16[:], in0=i7f[:], scalar1=16.0,
                                scalar2=None, op0=Alu.mult)
        # radix pass-1 threshold grid (build-time constants: lo=-16, w=0.25)
        iQf = const.tile([P, 128], F32)
        nc.vector.tensor_copy(out=iQf[:], in_=iQ_i[:])
        thr1row = const.tile([P, 128], F32)
        nc.vector.tensor_scalar(out=thr1row[:], in0=iQf[:], scalar1=32.0 / P,
                                scalar2=-16.0, op0=Alu.mult, op1=Alu.add)
        # negated per-pass threshold offsets for radix passes 2..4
        W1P = 32.0 / P
        nthrbs = []
        for p_ in range(1, RADIX_PASSES):
            w_p = W1P / (P ** p_)
            t_ = const.tile([P, 1], F32, name=f"nthrb{p_}")
            nc.vector.tensor_scalar(out=t_[:], in0=iota_f[:], scalar1=-w_p,
                                    scalar2=None, op0=Alu.mult)
            nthrbs.append((w_p, t_))
        hb_col = const.tile([P, 1], F32)
        nc.gpsimd.partition_broadcast(hb_col[:], hb_sb[:])

        scores_sb = const.tile([P, NT], F32)
        selidx_sb = const.tile([P, NSJ], I32)
        idx16_sb = const.tile([P, SEL // 16], I16)

        misc_psum_ctx = tc.tile_pool(name="misc_psum", bufs=2, space="PSUM")
        misc_psum = misc_psum_ctx.__enter__()

        # ---- phase A: router scores (fp32, exact; router bias dropped — it
        # shifts every score equally so the top-K set is unchanged).  The
        # first radix pass uses a build-time-constant threshold grid, so its
        # per-tile compare + count-matmul accumulation is folded in here. -----
        c1_psum_ctx = tc.tile_pool(name="c1_psum", bufs=1, space="PSUM")
        c1_psum = c1_psum_ctx.__enter__()
        cnt1_ps = c1_psum.tile([P, 128], F32, name="cnt1")
        nlo = const.tile([P, 1], F32, name="nlo")
        with ExitStack() as SA:
            apool = SA.enter_context(tc.tile_pool(name="apool", bufs=1))
            xs_pool = SA.enter_context(tc.tile_pool(name="xs", bufs=6))
            junk_pool = SA.enter_context(tc.tile_pool(name="junk", bufs=2))
            cmp_pool = SA.enter_context(tc.tile_pool(name="cmp", bufs=3))

            wrb = apool.tile([P, D], F32)
            for n in range(D // 512):
                pt = misc_psum.tile([P, 512], F32, name="mp")
                nc.tensor.matmul(out=pt[:], lhsT=o1x128_sb[:],
                                 rhs=wr_sb[:, n * 512:(n + 1) * 512],
                                 start=True, stop=True)
                nc.vector.tensor_copy(out=wrb[:, n * 512:(n + 1) * 512], in_=pt[:])

            x_last = None
            for t in range(NT):
                x_t = xs_pool.tile([P, D], F32)
                nc.sync.dma_start(out=x_t[:], in_=x_row[t * P:(t + 1) * P, :])
                x_last = x_t
                if t == 26:
                    nc.sync.dma_start(
                        out=scores_dg[0].rearrange("c p -> p c"),
                        in_=scores_sb[:, 0:20])
                prod = junk_pool.tile([P, D], F32, name="prod")
                nc.vector.tensor_tensor(out=prod[:], in0=x_t[:], in1=wrb[:],
                                        op=Alu.mult)
                sink = junk_pool.tile([P, D], BF16, name="sink")
                nc.scalar.activation(out=sink[:], in_=prod[:], func=Act.Identity,
                                     bias=0.0, scale=1.0,
                                     accum_out=scores_sb[:, t:t + 1])
                cmp_t = cmp_pool.tile([P, 128], F32, name="cmp")
                nc.vector.tensor_tensor(
                    out=cmp_t[:],
                    in0=scores_sb[:, t:t + 1].to_broadcast([P, 128]),
                    in1=thr1row[:], op=Alu.is_ge)
                nc.tensor.matmul(out=cnt1_ps[:], lhsT=ones128_sb[:], rhs=cmp_t[:],
                                 start=(t == 0), stop=(t == NT - 1),
                                 skip_group_check=True)


            # pass-1 finalize on every partition (count matmul used an
            # all-ones lhsT, so each partition holds the full count row):
            # nlo = -(lo1) = 16 - (sum(cnt>=K) - 1)*0.25
            selr = apool.tile([P, 128], F32, name="selr")
            nc.vector.tensor_scalar(out=selr[:], in0=cnt1_ps[:],
                                    scalar1=float(K), scalar2=None,
                                    op0=Alu.is_ge)
            s1 = apool.tile([P, 1], F32, name="s1")
            nc.vector.tensor_reduce(out=s1[:], in_=selr[:],
                                    axis=mybir.AxisListType.X, op=Alu.add)
            q1 = apool.tile([P, 1], F32, name="q1")
            nc.vector.tensor_scalar(out=q1[:], in0=s1[:], scalar1=-1.0,
                                    scalar2=-W1P, op0=Alu.add, op1=Alu.mult)
            nc.vector.tensor_scalar(out=nlo[:], in0=q1[:], scalar1=16.0,
                                    scalar2=None, op0=Alu.add)
        c1_psum_ctx.__exit__(None, None, None)

        offf_c = const.tile([P, NT], F32)
        maskf_c = const.tile([P, NT], F32)

        # ---- phases B+C+D: replicate scores, radix threshold, rank ----------
        with ExitStack() as SC:
            radix = SC.enter_context(tc.tile_pool(name="radix", bufs=2))
            rep_pool = SC.enter_context(tc.tile_pool(name="rep", bufs=1))

            # broadcast-read the spilled scores, one DMA per 1024-token group,
            # FIRST on the in-order SP queue right after the x loads (the
            # remaining const loads queue behind, they aren't needed till later)
            scores_rep = rep_pool.tile([P, L], F32)
            for gi, (glo, ghi) in enumerate(GRPS):
                n_ = (ghi - glo) * P
                if gi > 0:   # g0 was spilled inside the x stream
                    nc.sync.dma_start(
                        out=scores_dg[gi].rearrange("c p -> p c"),
                        in_=scores_sb[:, glo:ghi])
                nc.sync.dma_start(
                    out=scores_rep[:, glo * P:ghi * P],
                    in_=scores_dg[gi].rearrange("c p -> () (c p)")
                    .to_broadcast([P, n_]))

            # gate the w1 cast-loads behind the score broadcast so their DMAs
            # cannot delay it (WAW edge: the w1 DMA overwrites the gate byte)
            for kd in range(ND):
                nc.vector.tensor_copy(out=w1bf[kd][0:1, 0:1],
                                      in_=scores_rep[0:1, kd:kd + 1])

            # ---- remaining small consts on the SP queue ---------------------
            b1t_sb = cload(const, b1t, [P, NM], name="c_b1t")
            identb_sb = cload(const, identb, [P, P], BF16, name="c_idb")
            ltri_sb = cload(const, ltri, [P, P], name="c_lt")
            slt32_sb = cload(const, slt32, [NT, NT], name="c_sl")
            id32_sb = cload(const, id32, [NT, NT], name="c_id32")
            o1x128b_sb = cload(const, ones_1x128b, [1, P], BF16, name="c_o1b")
            o32x128_sb = cload(const, ones_32x128, [NT, P], name="c_o32")
            rep16_sb = cload(const, rep16, [32, P], name="c_rep16")
            ewrap_sb = cload(const, ewrap, [32, 8 * P], name="c_ew")
            b2bf_sb = const.tile([1, D], BF16)
            nc.gpsimd.dma_start(out=b2bf_sb[:], in_=b2)  # cast f32 -> bf16

            sjunk = rep_pool.tile([P, L], BF16, name="sjunk")
            for w_p, nthrb_p in nthrbs:
                nthr = radix.tile([P, 1], F32, name="nthr")
                nc.vector.tensor_tensor(out=nthr[:], in0=nlo[:], in1=nthrb_p[:],
                                        op=Alu.add)
                sgn = radix.tile([P, 1], F32, name="sgn")
                nc.scalar.activation(out=sjunk[:], in_=scores_rep[:],
                                     func=Act.Sign, bias=nthr[:, :1], scale=1.0,
                                     accum_out=sgn[:])
                sel = radix.tile([P, 1], F32, name="sel")
                nc.vector.tensor_scalar(out=sel[:], in0=sgn[:], scalar1=0.0,
                                        scalar2=None, op0=Alu.is_ge)
                s_all = radix.tile([P, 1], F32, name="s_all")
                nc.gpsimd.partition_all_reduce(s_all[:], sel[:], channels=P,
                                               reduce_op=Red.add)
                nd = radix.tile([P, 1], F32, name="nd")
                nc.vector.tensor_scalar(out=nd[:], in0=s_all[:], scalar1=-1.0,
                                        scalar2=-w_p, op0=Alu.add, op1=Alu.mult)
                nlo2 = radix.tile([P, 1], F32, name="nlo2")
                nc.vector.tensor_tensor(out=nlo2[:], in0=nlo[:], in1=nd[:],
                                        op=Alu.add)
                nlo = nlo2

            # ---- mask + global rank (exclusive prefix of mask) --------------
            m0 = radix.tile([P, NT], F32, name="m0")
            nc.vector.tensor_tensor(out=m0[:], in0=scores_sb[:],
                                    in1=nlo[:, :1].to_broadcast([P, NT]),
                                    op=Alu.add)
            maskf = radix.tile([P, NT], F32, name="maskf")
            nc.vector.tensor_scalar(out=maskf[:], in0=m0[:], scalar1=0.0,
                                    scalar2=None, op0=Alu.is_ge)
            colsum_p = misc_psum.tile([NT, 1], F32, name="mp")
            nc.tensor.matmul(out=colsum_p[:], lhsT=maskf[:], rhs=o128x1_sb[:],
                             start=True, stop=True)
            colsum = radix.tile([NT, 1], F32, name="colsum")
            nc.vector.tensor_copy(out=colsum[:], in_=colsum_p[:])
            excl_p = misc_psum.tile([NT, 1], F32, name="mp")
            nc.tensor.matmul(out=excl_p[:], lhsT=slt32_sb[:], rhs=colsum[:],
                             start=True, stop=True)
            excl = radix.tile([NT, 1], F32, name="excl")
            nc.vector.tensor_copy(out=excl[:], in_=excl_p[:])
            diag = radix.tile([NT, NT], F32, name="diag")
            nc.vector.tensor_tensor(out=diag[:], in0=id32_sb[:],
                                    in1=excl[:, :1].to_broadcast([NT, NT]),
                                    op=Alu.mult)
            rank_p = misc_psum.tile([P, NT], F32, name="mp")
            nc.tensor.matmul(out=rank_p[:], lhsT=ltri_sb[:], rhs=maskf[:],
                             start=True, stop=False, skip_group_check=True)
            nc.tensor.matmul(out=rank_p[:], lhsT=o32x128_sb[:], rhs=diag[:],
                             start=False, stop=True, skip_group_check=True)
            rank = radix.tile([P, NT], F32, name="rank")
            nc.vector.tensor_copy(out=rank[:], in_=rank_p[:])
            off = radix.tile([P, NT], F32, name="off")
            nc.vector.tensor_tensor(out=off[:], in0=rank[:],
                                    in1=hb_col[:, :1].to_broadcast([P, NT]),
                                    op=Alu.subtract)
            nc.vector.tensor_copy(out=offf_c[:], in_=off[:])
            nc.vector.tensor_copy(out=maskf_c[:], in_=maskf[:])

        misc_psum_ctx.__exit__(None, None, None)

        # ---- w1 cast-loads on the Pool queue.  Positioned after the radix
        # all_reduces so the in-order queue starts them only ~70us in, after
        # the x-tile DMAs have drained (they'd otherwise steal DMA bandwidth
        # from the critical-path score loads). ---------------------------------
        w1bf = []
        for kd in range(ND):
            t_ = w1_pool.tile([P, DFF], BF16, name=f"w1bf_{kd}")
            nc.gpsimd.dma_start(out=t_[:], in_=w1[kd * P:(kd + 1) * P, :])
            w1bf.append(t_)

        # ---- phase E: digit split + one-hot compaction matmuls --------------
        # off in [0, SEL) for in-window selected tokens; any other off value
        # (negative rank-window miss, >=SEL, or collision of an unselected
        # token) produces no match in the lo-digit equality below, and
        # unselected tokens are additionally zeroed via tokid*mask weights.
        with ExitStack() as SE:
            ep = SE.enter_context(tc.tile_pool(name="epool", bufs=1))
            e_psum = SE.enter_context(tc.tile_pool(name="e_psum", bufs=2,
                                                   space="PSUM"))
            off = offf_c
            eq7a = ep.tile([P, NT, 7], F32, name="eq7a")
            nc.vector.tensor_tensor(
                out=eq7a[:], in0=off[:, :, None].to_broadcast([P, NT, 7]),
                in1=thr128[:, None, :].to_broadcast([P, NT, 7]), op=Alu.is_ge)
            hi128 = ep.tile([P, NT], F32, name="hi128")
            nc.vector.tensor_reduce(out=hi128[:], in_=eq7a[:],
                                    axis=mybir.AxisListType.X, op=Alu.add)
            hm = ep.tile([P, NT], F32, name="hm")
            nc.vector.tensor_scalar(out=hm[:], in0=hi128[:], scalar1=-128.0,
                                    scalar2=None, op0=Alu.mult)
            lo128 = ep.tile([P, NT], F32, name="lo128")
            nc.vector.tensor_tensor(out=lo128[:], in0=off[:], in1=hm[:],
                                    op=Alu.add)
            eq7b = ep.tile([P, NT, 7], F32, name="eq7b")
            nc.vector.tensor_tensor(
                out=eq7b[:], in0=lo128[:, :, None].to_broadcast([P, NT, 7]),
                in1=thr16[:, None, :].to_broadcast([P, NT, 7]), op=Alu.is_ge)
            mid = ep.tile([P, NT], F32, name="mid")
            nc.vector.tensor_reduce(out=mid[:], in_=eq7b[:],
                                    axis=mybir.AxisListType.X, op=Alu.add)
            hm2 = ep.tile([P, NT], F32, name="hm2")
            nc.vector.tensor_scalar(out=hm2[:], in0=mid[:], scalar1=-16.0,
                                    scalar2=None, op0=Alu.mult)
            lo16 = ep.tile([P, NT], F32, name="lo16")
            nc.vector.tensor_tensor(out=lo16[:], in0=lo128[:], in1=hm2[:],
                                    op=Alu.add)
            h8 = ep.tile([P, NT], F32, name="h8")
            nc.vector.tensor_scalar(out=h8[:], in0=hi128[:], scalar1=8.0,
                                    scalar2=None, op0=Alu.mult)
            hi16 = ep.tile([P, NT], F32, name="hi16")
            nc.vector.tensor_tensor(out=hi16[:], in0=h8[:], in1=mid[:],
                                    op=Alu.add)
            # token id = c*128 + p; weight the SMALL equality factors by
            # c*mask (chain C, lhsT cols 0:16) and p*mask (chain D, cols
            # 16:32), then sel16 = 128*C + D.  All factors are small exact
            # integers, so the chain runs in bf16 (1 cycle/row matmuls).
            maskb = ep.tile([P, NT], BF16, name="maskb")
            nc.vector.tensor_copy(out=maskb[:], in_=maskf_c[:])
            cwm = ep.tile([P, NT], BF16, name="cwm")
            nc.vector.tensor_tensor(out=cwm[:], in0=cvalb[:], in1=maskb[:],
                                    op=Alu.mult)
            pwm = ep.tile([P, NT], BF16, name="pwm")
            nc.vector.tensor_tensor(out=pwm[:], in0=maskb[:],
                                    in1=iotab[:, :1].to_broadcast([P, NT]),
                                    op=Alu.mult)
            lo16b = ep.tile([P, NT], BF16, name="lo16b")
            nc.vector.tensor_copy(out=lo16b[:], in_=lo16[:])
            hi16b = ep.tile([P, NT], BF16, name="hi16b")
            nc.vector.tensor_copy(out=hi16b[:], in_=hi16[:])

            eq16 = ep.tile([P, NT, 16], BF16, name="eq16")
            nc.vector.tensor_tensor(
                out=eq16[:], in0=iJ16b[:],
                in1=lo16b[:, :, None].to_broadcast([P, NT, 16]), op=Alu.is_equal)
            eqcp = ep.tile([P, NT, 32], BF16, name="eqcp")
            nc.vector.tensor_tensor(
                out=eqcp[:, :, 0:16], in0=eq16[:],
                in1=cwm[:, :, None].to_broadcast([P, NT, 16]), op=Alu.mult)
            nc.vector.tensor_tensor(
                out=eqcp[:, :, 16:32], in0=eq16[:],
                in1=pwm[:, :, None].to_broadcast([P, NT, 16]), op=Alu.mult)
            eq64 = ep.tile([P, NT, 64], BF16, name="eq64")
            nc.vector.tensor_tensor(
                out=eq64[:], in0=iK64b[:],
                in1=hi16b[:, :, None].to_broadcast([P, NT, 64]), op=Alu.is_equal)

            pCD = e_psum.tile([32, 64], F32, name="pCD")
            for c in range(NT):
                nc.tensor.matmul(out=pCD[:], lhsT=eqcp[:, c, :],
                                 rhs=eq64[:, c, :], start=(c == 0),
                                 stop=(c == NT - 1), skip_group_check=True)

            sCD = ep.tile([32, 64], F32, name="sCD")
            nc.vector.tensor_copy(out=sCD[:], in_=pCD[:])

            # scatter index layout [128, 64] (16-wrap replicated to 128);
            # lhsT folds the 128*C + D combine (rows 0:16 scaled by 128)
            rep_ps = e_psum.tile([P, 64], F32, name="rep_ps")
            nc.tensor.matmul(out=rep_ps[:], lhsT=rep16_sb[:], rhs=sCD[:],
                             start=True, stop=True)
            nc.vector.tensor_copy(out=idx16_sb[:], in_=rep_ps[:])  # f32->i16

            # gather index layout [128, 8]: selidx[p, k] = sel16[p%16, 8k+p//16]
            selps = e_psum.tile([P, NSJ], F32, name="selps")
            for g in range(8):
                nc.tensor.matmul(out=selps[:], lhsT=ewrap_sb[:, g * P:(g + 1) * P],
                                 rhs=sCD[:, g::8], start=(g == 0),
                                 stop=(g == 7), skip_group_check=True)
            nc.vector.tensor_copy(out=selidx_sb[:], in_=selps[:])  # f32->i32

        dig_ctx.__exit__(None, None, None)

        # ---- gather + transpose + MLP ---------------------------------------
        if True:
            with ExitStack() as SB:
                xt_pool = SB.enter_context(tc.tile_pool(name="xt", bufs=1))
                xsel_pool = SB.enter_context(tc.tile_pool(name="xsel", bufs=5))
                mm1_psum = SB.enter_context(tc.tile_pool(name="mm1_psum", bufs=6,
                                                         space="PSUM"))

                # xt3[p, kd, t] = x_sel[t, kd*128+p], built by the DMA-engine
                # xbar transpose (one per gathered 128-token chunk)
                xt3 = xt_pool.tile([P, ND, SEL], BF16)
                for j in range(NSJ):
                    xs = xsel_pool.tile([P, D], BF16, name="xsel")
                    nc.gpsimd.indirect_dma_start(
                        out=xs[:], out_offset=None, in_=x_row,
                        in_offset=IndirectOffsetOnAxis(ap=selidx_sb[:, j:j + 1],
                                                       axis=0))
                    nc.scalar.dma_start_transpose(
                        out=xt3[:, :, j * P:(j + 1) * P], in_=xs[:])

                # ---- mm1: ht[m, sel] = gelu(w1^T x_sel^T + b1).  The first
                # four token blocks are 128 wide so the PE starts the moment
                # each transpose lands instead of waiting for four of them;
                # the second half runs as one 512-wide block.
                for t0, tw in [(0, P), (P, P), (2 * P, P), (3 * P, P),
                               (512, 512)]:
                    for m in range(NM):
                        ph = mm1_psum.tile([P, tw], F32, name="ph")
                        for kd in range(ND):
                            nc.tensor.matmul(
                                out=ph[:],
                                lhsT=w1bf[kd][:, m * P:(m + 1) * P],
                                rhs=xt3[:, kd, t0:t0 + tw],
                                start=(kd == 0), stop=(kd == ND - 1),
                            )
                        nc.scalar.activation(
                            out=ht[:, m, t0:t0 + tw], in_=ph[:],
                            func=Act.Gelu_apprx_tanh, bias=b1t_sb[:, m:m + 1],
                            scale=1.0,
                        )

            w1_ctx.__exit__(None, None, None)  # free w1 region for w2 stream

            # ---- mm2: y[sel, D] = ht^T @ w2 + b2, then scatter-add ----------
            with ExitStack() as SY:
                y_pool = SY.enter_context(tc.tile_pool(name="y", bufs=1))
                w2_pool = SY.enter_context(tc.tile_pool(name="w2s", bufs=16))
                mm2_psum = SY.enter_context(tc.tile_pool(name="mm2_psum", bufs=8,
                                                         space="PSUM"))
                # d-half 0: kg-major accumulation (w2 tiles stream in, all 8
                # token-block psums accumulate together)
                n = 0
                y_0 = y_pool.tile([P, NSJ, 512], F32, name="y0")
                pys = [mm2_psum.tile([P, 512], F32, name="py")
                       for _ in range(NSJ)]
                w2n1 = []   # d-half-1 tiles retained for the s-major pass
                for s in range(NSJ):
                    nc.tensor.matmul(
                        out=pys[s][:], lhsT=o1x128b_sb[:],
                        rhs=b2bf_sb[:, :512],
                        start=True, stop=False, skip_group_check=True,
                    )
                for kg in range(NM // NKGRP):
                    w2t = w2_pool.tile([P, NKGRP, 512], BF16, name="w2t")
                    if kg == 0:
                        # WAW gate: keep the w2 stream off the DMA engines
                        # until the gather/transpose pipeline has fed mm1
                        nc.vector.tensor_copy(out=w2t[0:1, 0, 0:1],
                                              in_=ht[0:1, 0, 0:1])
                    src = w2[:, :512].rearrange(
                        "(g p) f -> p g f", p=P)[:, kg * NKGRP:(kg + 1) * NKGRP, :]
                    nc.gpsimd.dma_start(out=w2t[:], in_=src)
                    for ki in range(NKGRP):
                        kk = kg * NKGRP + ki
                        for s in range(NSJ):
                            nc.tensor.matmul(
                                out=pys[s][:],
                                lhsT=ht[:, kk, s * P:(s + 1) * P],
                                rhs=w2t[:, ki, :],
                                start=False, stop=(kk == NM - 1),
                                skip_group_check=True,
                            )
                # prefetch d-half-1 w2 tiles while the n=0 tail accumulates
                for kg in range(NM // NKGRP):
                    w2t = w2_pool.tile([P, NKGRP, 512], BF16, name="w2t")
                    src = w2[:, 512:].rearrange(
                        "(g p) f -> p g f", p=P)[:, kg * NKGRP:(kg + 1) * NKGRP, :]
                    nc.gpsimd.dma_start(out=w2t[:], in_=src)
                    w2n1.append(w2t)
                for s in range(NSJ):
                    nc.scalar.activation(out=y_0[:, s, :], in_=pys[s][:],
                                         func=Act.Copy, bias=0.0, scale=1.0)
                    if s % 4 == 3:
                        h = s // 4
                        nc.gpsimd.dma_scatter_add(
                            out_row[:, :512],
                            y_0[:, h * 4:(h + 1) * 4, :],
                            idx16_sb[:, h * 32:(h + 1) * 32],
                            SEL // 2,
                            SEL // 2,
                            512,
                            elem_step=D,
                        )

                # d-half 1: s-major (each token block finishes early and its
                # rows scatter while the next block accumulates)
                y_1 = y_pool.tile([P, NSJ, 512], F32, name="y1")
                for s in range(NSJ):
                    py = mm2_psum.tile([P, 512], F32, name="py")
                    nc.tensor.matmul(
                        out=py[:], lhsT=o1x128b_sb[:], rhs=b2bf_sb[:, 512:],
                        start=True, stop=False, skip_group_check=True,
                    )
                    for kk in range(NM):
                        nc.tensor.matmul(
                            out=py[:],
                            lhsT=ht[:, kk, s * P:(s + 1) * P],
                            rhs=w2n1[kk // NKGRP][:, kk % NKGRP, :],
                            start=False, stop=(kk == NM - 1),
                            skip_group_check=True,
                        )
                    nc.scalar.activation(out=y_1[:, s, :], in_=py[:],
                                         func=Act.Copy, bias=0.0, scale=1.0)
                    nc.gpsimd.dma_scatter_add(
                        out_row[:, 512:],
                        y_1[:, s:s + 1, :],
                        idx16_sb[:, s * 8:(s + 1) * 8],
                        P,
                        P,
                        512,
                        elem_step=D,
                    )

        ht_ctx.__exit__(None, None, None)

    nc.compile()
    return nc


def make_consts():
    q = np.arange(P)
    import ml_dtypes
    consts = {
        "identb": np.eye(P, dtype=ml_dtypes.bfloat16),
        "ltri128": (q[:, None] < q[None, :]).astype(np.float32),  # [q, p] = q < p
        "slt32": (np.arange(NT)[:, None] < np.arange(NT)[None, :]).astype(np.float32),
        "id32": np.eye(NT, dtype=np.float32),
        "ones_1x128": np.ones((1, P), np.float32),
        "ones_1x128b": np.ones((1, P), ml_dtypes.bfloat16),
        "ones_128x1": np.ones((P, 1), np.float32),
        "ones128": np.ones((P, P), np.float32),
        "ones_32x128": np.ones((NT, P), np.float32),
        "rep16": np.vstack([
            128.0 * (np.arange(16)[:, None] == (np.arange(P)[None, :] % 16)),
            1.0 * (np.arange(16)[:, None] == (np.arange(P)[None, :] % 16)),
        ]).astype(np.float32),
    }
    # ewrap[i, g*128 + p] = 1 iff p == g*16 + i  (16-wrap -> 128-wrap expand);
    # stacked [32, .]: rows 0:16 scaled by 128 (C chain), rows 16:32 raw (D)
    ew = np.zeros((16, 8 * P), np.float32)
    for i in range(16):
        for g in range(8):
            ew[i, g * P + g * 16 + i] = 1.0
    consts["ewrap"] = np.vstack([128.0 * ew, ew]).astype(np.float32)
    return consts


def make_in_maps(x, W1, b1, W2, b2, wr, br):
    consts = make_consts()
    x = np.ascontiguousarray(np.asarray(x, np.float32))
    in_maps = []
    for c in range(NCORES):
        b, h = divmod(c, 2)
        m = {
            "x_row": x[b],
            "w1": np.asarray(W1, np.float32),
            "w2": np.asarray(W2, np.float32),
            "wr": np.asarray(wr, np.float32).reshape(1, D),
            "b1t": np.ascontiguousarray(np.asarray(b1, np.float32).reshape(NM, P).T),
            "b2": np.asarray(b2, np.float32).reshape(1, D),
            "hbase": np.array([[h * SEL]], np.float32),
        }
        m.update(consts)
        in_maps.append(m)
    return in_maps


_NC_CACHE = None


def _get_program():
    global _NC_CACHE
    if _NC_CACHE is None:
        _NC_CACHE = build_program()
    return _NC_CACHE


def kernel(x, W1, b1, W2, b2, wr, br):
    from concourse.bass_utils import run_bass_kernel_spmd

    nc = _get_program()
    in_maps = make_in_maps(x, W1, b1, W2, b2, wr, br)
    res = run_bass_kernel_spmd(nc, in_maps, list(range(NCORES))).results
    out = np.stack(
        [res[2 * b]["out_row"] + res[2 * b + 1]["out_row"] for b in range(B)]
    )
    return out.astype(np.float32)


# revision 65
# speedup vs baseline: 2.5874x; 1.0022x over previous
"""MoD (mixture-of-depths) MLP wrapper kernel for Trainium2, 8 NeuronCores.

Sharding: core c handles batch row b = c//2 and the half of that row's
top-K tokens with global selection ranks in [h*1024, (h+1)*1024), h = c%2.
Each core computes the full row's router scores + top-K threshold locally
(no collectives), gathers exactly 1024 token rows by rank via indirect DMA,
runs the FFN in bf16 (fp32 accumulation), and scatters results back into the
pre-zeroed per-core output buffer with dma_scatter_add.  Host sums the two
buffers of each row.

Schedule: x-tile loads own the DMA engines first (weight loads are ordered
behind them); radix pass 1 folds into the score loop against a constant
threshold grid; passes 2-4 run as Sign-activation counts over a
DMA-broadcast score replica; rank compaction is a digit-decomposed one-hot
bf16 matmul whose stacked constants emit both the int32 gather and int16
scatter index layouts; gathered tokens are transposed by the DMA xbar
(dma_start_transpose); and the output scatter is dma_scatter_add (per-index
descriptors) overlapped with the tail of the second matmul.
"""

import sys

sys.path.insert(0, "/opt/trn_rl_repo")

from contextlib import ExitStack

import numpy as np

from concourse import bass, bass_isa, mybir
from concourse import bacc
import concourse.tile as tile
from concourse.bass import IndirectOffsetOnAxis

B, L, D = 4, 4096, 1024
DFF = 4 * D
K = L // 2              # 2048 selected tokens per row
NCORES = 8
P = 128
NT = L // P             # 32 token tiles per row
SEL = K // 2            # 1024 selected tokens per core
NSJ = SEL // P          # 8 selected-token blocks
ND = D // P             # 8 d chunks
NM = DFF // P           # 32 dff tiles
NKGRP = 4               # w2 k-chunks per streamed tile
RADIX_PASSES = 4

F32 = mybir.dt.float32
BF16 = mybir.dt.bfloat16
I32 = mybir.dt.int32
I16 = mybir.dt.int16
Alu = mybir.AluOpType
Act = mybir.ActivationFunctionType
Red = bass_isa.ReduceOp


def build_program():
    nc = bacc.Bacc(
        "TRN2",
        target_bir_lowering=False,
        debug=False,
        enable_asserts=False,
        num_devices=NCORES,
    )

    x_row = nc.dram_tensor("x_row", [L, D], F32, kind="ExternalInput").ap()
    w1 = nc.dram_tensor("w1", [D, DFF], F32, kind="ExternalInput").ap()
    w2 = nc.dram_tensor("w2", [DFF, D], F32, kind="ExternalInput").ap()
    wr = nc.dram_tensor("wr", [1, D], F32, kind="ExternalInput").ap()
    b1t = nc.dram_tensor("b1t", [P, NM], F32, kind="ExternalInput").ap()
    b2 = nc.dram_tensor("b2", [1, D], F32, kind="ExternalInput").ap()
    hbase = nc.dram_tensor("hbase", [1, 1], F32, kind="ExternalInput").ap()
    identb = nc.dram_tensor("identb", [P, P], BF16, kind="ExternalInput").ap()
    ltri = nc.dram_tensor("ltri128", [P, P], F32, kind="ExternalInput").ap()
    slt32 = nc.dram_tensor("slt32", [NT, NT], F32, kind="ExternalInput").ap()
    id32 = nc.dram_tensor("id32", [NT, NT], F32, kind="ExternalInput").ap()
    ones_1x128 = nc.dram_tensor("ones_1x128", [1, P], F32, kind="ExternalInput").ap()
    ones_1x128b = nc.dram_tensor("ones_1x128b", [1, P], BF16, kind="ExternalInput").ap()
    ones_128x1 = nc.dram_tensor("ones_128x1", [P, 1], F32, kind="ExternalInput").ap()
    ones128 = nc.dram_tensor("ones128", [P, P], F32, kind="ExternalInput").ap()
    ones_32x128 = nc.dram_tensor("ones_32x128", [NT, P], F32, kind="ExternalInput").ap()
    rep16 = nc.dram_tensor("rep16", [32, P], F32, kind="ExternalInput").ap()
    ewrap = nc.dram_tensor("ewrap", [32, 8 * P], F32, kind="ExternalInput").ap()

    out_row = nc.dram_tensor("out_row", [L, D], F32, kind="ExternalOutput").ap()

    GRPS = ((0, 20), (20, 30), (30, 31), (31, 32))
    scores_dg = [nc.dram_tensor(f"scores_dg{i}", [hi - lo, P], F32).ap()
                 for i, (lo, hi) in enumerate(GRPS)]

    with tile.TileContext(nc) as tc, ExitStack() as S0:
        const = S0.enter_context(tc.tile_pool(name="const", bufs=1))
        # pool stack (LIFO): const | ht | w1 | dig | ...phases
        ht_ctx = tc.tile_pool(name="ht", bufs=1)
        ht_pool = ht_ctx.__enter__()
        ht = ht_pool.tile([P, NM, SEL], BF16)
        w1_ctx = tc.tile_pool(name="w1bf", bufs=1)
        w1_pool = w1_ctx.__enter__()

        def cload(pool, ap, shape, dtype=F32, name=None):
            t = pool.tile(shape, dtype, name=name)
            nc.sync.dma_start(out=t[:], in_=ap)
            return t

        # ---- SP-queue order: wr, o1, oc, hbase FIRST (phase A needs them) ---
        wr_sb = cload(const, wr, [1, D], name="c_wr")
        o1x128_sb = cload(const, ones_1x128, [1, P], name="c_o1")
        o128x1_sb = cload(const, ones_128x1, [P, 1], name="c_oc")
        ones128_sb = cload(const, ones128, [P, P], name="c_o128")
        hb_sb = cload(const, hbase, [1, 1], name="c_hb")

        # w1 tiles exist from the start (loads are issued after the radix)
        w1bf = [w1_pool.tile([P, DFF], BF16, name=f"w1bf_{kd}")
                for kd in range(ND)]

        # ---- Pool-queue iotas (independent of SP queue) ---------------------
        # big digit-decomposition iota tables live only through phase E
        dig_ctx = tc.tile_pool(name="dig", bufs=1)
        dig = dig_ctx.__enter__()

        iota_i = const.tile([P, 1], I32)
        nc.gpsimd.iota(iota_i[:], pattern=[[1, 1]], base=0, channel_multiplier=1)
        tokid = const.tile([P, NT], I32)
        nc.gpsimd.iota(tokid[:], pattern=[[P, NT]], base=0, channel_multiplier=1)
        iC_i = const.tile([P, NT], I32)
        nc.gpsimd.iota(iC_i[:], pattern=[[1, NT]], base=0, channel_multiplier=0)
        iQ_i = const.tile([P, 128], I32)
        nc.gpsimd.iota(iQ_i[:], pattern=[[1, 128]], base=0, channel_multiplier=0)
        iK64_i = dig.tile([P, NT, 64], I16)
        nc.gpsimd.iota(iK64_i[:], pattern=[[0, NT], [1, 64]], base=0,
                       channel_multiplier=0)
        iJ16_i = dig.tile([P, NT, 16], I16)
        nc.gpsimd.iota(iJ16_i[:], pattern=[[0, NT], [1, 16]], base=0,
                       channel_multiplier=0)
        i7_i = const.tile([P, 7], I32)
        nc.gpsimd.iota(i7_i[:], pattern=[[1, 7]], base=1, channel_multiplier=0)

        iota_f = const.tile([P, 1], F32)
        nc.vector.tensor_copy(out=iota_f[:], in_=iota_i[:])
        tokidf = const.tile([P, NT], F32)
        nc.vector.tensor_copy(out=tokidf[:], in_=tokid[:])
        cvalf = const.tile([P, NT], F32)
        nc.vector.tensor_copy(out=cvalf[:], in_=iC_i[:])
        iK64b = dig.tile([P, NT, 64], BF16)
        nc.vector.tensor_copy(out=iK64b[:], in_=iK64_i[:])
        iJ16b = dig.tile([P, NT, 16], BF16)
        nc.vector.tensor_copy(out=iJ16b[:], in_=iJ16_i[:])
        iotab = const.tile([P, 1], BF16)
        nc.vector.tensor_copy(out=iotab[:], in_=iota_i[:])
        cvalb = const.tile([P, NT], BF16)
        nc.vector.tensor_copy(out=cvalb[:], in_=iC_i[:])
        i7f = const.tile([P, 7], F32)
        nc.vector.tensor_copy(out=i7f[:], in_=i7_i[:])
        thr128 = const.tile([P, 7], F32)
        nc.vector.tensor_scalar(out=thr128[:], in0=i7f[:], scalar1=128.0,
                                scalar2=None, op0=Alu.mult)
        thr16 = const.tile([P, 7], F32)
        nc.vector.tensor_scalar(out=thr16[:], in0=i7f[:], scalar1=16.0,
                                scalar2=None, op0=Alu.mult)
        # radix pass-1 threshold grid (build-time constants: lo=-16, w=0.25)
        iQf = const.tile([P, 128], F32)
        nc.vector.tensor_copy(out=iQf[:], in_=iQ_i[:])
        thr1row = const.tile([P, 128], F32)
        nc.vector.tensor_scalar(out=thr1row[:], in0=iQf[:], scalar1=32.0 / P,
                                scalar2=-16.0, op0=Alu.mult, op1=Alu.add)
        # negated per-pass threshold offsets for radix passes 2..4
        W1P = 32.0 / P
        nthrbs = []
        for p_ in range(1, RADIX_PASSES):
            w_p = W1P / (P ** p_)
            t_ = const.tile([P, 1], F32, name=f"nthrb{p_}")
            nc.vector.tensor_scalar(out=t_[:], in0=iota_f[:], scalar1=-w_p,
                                    scalar2=None, op0=Alu.mult)
            nthrbs.append((w_p, t_))
        hb_col = const.tile([P, 1], F32)
        nc.gpsimd.partition_broadcast(hb_col[:], hb_sb[:])

        scores_sb = const.tile([P, NT], F32)
        selidx_sb = const.tile([P, NSJ], I32)
        idx16_sb = const.tile([P, SEL // 16], I16)

        misc_psum_ctx = tc.tile_pool(name="misc_psum", bufs=2, space="PSUM")
        misc_psum = misc_psum_ctx.__enter__()

        # ---- phase A: router scores (fp32, exact; router bias dropped — it
        # shifts every score equally so the top-K set is unchanged).  The
        # first radix pass uses a build-time-constant threshold grid, so its
        # per-tile compare + count-matmul accumulation is folded in here. -----
        c1_psum_ctx = tc.tile_pool(name="c1_psum", bufs=1, space="PSUM")
        c1_psum = c1_psum_ctx.__enter__()
        cnt1_ps = c1_psum.tile([P, 128], F32, name="cnt1")
        nlo = const.tile([P, 1], F32, name="nlo")
        with ExitStack() as SA:
            apool = SA.enter_context(tc.tile_pool(name="apool", bufs=1))
            xs_pool = SA.enter_context(tc.tile_pool(name="xs", bufs=6))
            junk_pool = SA.enter_context(tc.tile_pool(name="junk", bufs=2))
            cmp_pool = SA.enter_context(tc.tile_pool(name="cmp", bufs=3))

            wrb = apool.tile([P, D], F32)
            for n in range(D // 512):
                pt = misc_psum.tile([P, 512], F32, name="mp")
                nc.tensor.matmul(out=pt[:], lhsT=o1x128_sb[:],
                                 rhs=wr_sb[:, n * 512:(n + 1) * 512],
                                 start=True, stop=True)
                nc.vector.tensor_copy(out=wrb[:, n * 512:(n + 1) * 512], in_=pt[:])

            x_last = None
            for t in range(NT):
                x_t = xs_pool.tile([P, D], F32)
                nc.sync.dma_start(out=x_t[:], in_=x_row[t * P:(t + 1) * P, :])
                x_last = x_t
                if t == 26:
                    nc.sync.dma_start(
                        out=scores_dg[0].rearrange("c p -> p c"),
                        in_=scores_sb[:, 0:20])
                prod = junk_pool.tile([P, D], F32, name="prod")
                nc.vector.tensor_tensor(out=prod[:], in0=x_t[:], in1=wrb[:],
                                        op=Alu.mult)
                sink = junk_pool.tile([P, D], BF16, name="sink")
                nc.scalar.activation(out=sink[:], in_=prod[:], func=Act.Identity,
                                     bias=0.0, scale=1.0,
                                     accum_out=scores_sb[:, t:t + 1])
                cmp_t = cmp_pool.tile([P, 128], F32, name="cmp")
                nc.vector.tensor_tensor(
                    out=cmp_t[:],
                    in0=scores_sb[:, t:t + 1].to_broadcast([P, 128]),
                    in1=thr1row[:], op=Alu.is_ge)
                nc.tensor.matmul(out=cnt1_ps[:], lhsT=ones128_sb[:], rhs=cmp_t[:],
                                 start=(t == 0), stop=(t == NT - 1),
                                 skip_group_check=True)


            # pass-1 finalize on every partition (count matmul used an
            # all-ones lhsT, so each partition holds the full count row):
            # nlo = -(lo1) = 16 - (sum(cnt>=K) - 1)*0.25
            selr = apool.tile([P, 128], F32, name="selr")
            nc.vector.tensor_scalar(out=selr[:], in0=cnt1_ps[:],
                                    scalar1=float(K), scalar2=None,
                                    op0=Alu.is_ge)
            s1 = apool.tile([P, 1], F32, name="s1")
            nc.vector.tensor_reduce(out=s1[:], in_=selr[:],
                                    axis=mybir.AxisListType.X, op=Alu.add)
            q1 = apool.tile([P, 1], F32, name="q1")
            nc.vector.tensor_scalar(out=q1[:], in0=s1[:], scalar1=-1.0,
                                    scalar2=-W1P, op0=Alu.add, op1=Alu.mult)
            nc.vector.tensor_scalar(out=nlo[:], in0=q1[:], scalar1=16.0,
                                    scalar2=None, op0=Alu.add)
        c1_psum_ctx.__exit__(None, None, None)

        offf_c = const.tile([P, NT], F32)
        maskf_c = const.tile([P, NT], F32)

        # ---- phases B+C+D: replicate scores, radix threshold, rank ----------
        with ExitStack() as SC:
            radix = SC.enter_context(tc.tile_pool(name="radix", bufs=2))
            rep_pool = SC.enter_context(tc.tile_pool(name="rep", bufs=1))

            # broadcast-read the spilled scores, one DMA per 1024-token group,
            # FIRST on the in-order SP queue right after the x loads (the
            # remaining const loads queue behind, they aren't needed till later)
            scores_rep = rep_pool.tile([P, L], F32)
            for gi, (glo, ghi) in enumerate(GRPS):
                n_ = (ghi - glo) * P
                if gi > 0:   # g0 was spilled inside the x stream
                    nc.sync.dma_start(
                        out=scores_dg[gi].rearrange("c p -> p c"),
                        in_=scores_sb[:, glo:ghi])
                nc.sync.dma_start(
                    out=scores_rep[:, glo * P:ghi * P],
                    in_=scores_dg[gi].rearrange("c p -> () (c p)")
                    .to_broadcast([P, n_]))

            # gate the w1 cast-loads behind the score broadcast so their DMAs
            # cannot delay it (WAW edge: the w1 DMA overwrites the gate byte)
            for kd in range(ND):
                nc.vector.tensor_copy(out=w1bf[kd][0:1, 0:1],
                                      in_=scores_rep[0:1, kd:kd + 1])

            # ---- remaining small consts on the SP queue ---------------------
            b1t_sb = cload(const, b1t, [P, NM], name="c_b1t")
            identb_sb = cload(const, identb, [P, P], BF16, name="c_idb")
            ltri_sb = cload(const, ltri, [P, P], name="c_lt")
            slt32_sb = cload(const, slt32, [NT, NT], name="c_sl")
            id32_sb = cload(const, id32, [NT, NT], name="c_id32")
            o1x128b_sb = cload(const, ones_1x128b, [1, P], BF16, name="c_o1b")
            o32x128_sb = cload(const, ones_32x128, [NT, P], name="c_o32")
            rep16_sb = cload(const, rep16, [32, P], name="c_rep16")
            ewrap_sb = cload(const, ewrap, [32, 8 * P], name="c_ew")
            b2bf_sb = const.tile([1, D], BF16)
            nc.gpsimd.dma_start(out=b2bf_sb[:], in_=b2)  # cast f32 -> bf16

            sjunk = rep_pool.tile([P, L], BF16, name="sjunk")
            for w_p, nthrb_p in nthrbs:
                nthr = radix.tile([P, 1], F32, name="nthr")
                nc.vector.tensor_tensor(out=nthr[:], in0=nlo[:], in1=nthrb_p[:],
                                        op=Alu.add)
                sgn = radix.tile([P, 1], F32, name="sgn")
                nc.scalar.activation(out=sjunk[:], in_=scores_rep[:],
                                     func=Act.Sign, bias=nthr[:, :1], scale=1.0,
                                     accum_out=sgn[:])
                sel = radix.tile([P, 1], F32, name="sel")
                nc.vector.tensor_scalar(out=sel[:], in0=sgn[:], scalar1=0.0,
                                        scalar2=None, op0=Alu.is_ge)
                s_all = radix.tile([P, 1], F32, name="s_all")
                nc.gpsimd.partition_all_reduce(s_all[:], sel[:], channels=P,
                                               reduce_op=Red.add)
                nd = radix.tile([P, 1], F32, name="nd")
                nc.vector.tensor_scalar(out=nd[:], in0=s_all[:], scalar1=-1.0,
                                        scalar2=-w_p, op0=Alu.add, op1=Alu.mult)
                nlo2 = radix.tile([P, 1], F32, name="nlo2")
                nc.vector.tensor_tensor(out=nlo2[:], in0=nlo[:], in1=nd[:],
                                        op=Alu.add)
                nlo = nlo2

            # ---- mask + global rank (exclusive prefix of mask) --------------
            maskf = radix.tile([P, NT], F32, name="maskf")
            nc.vector.tensor_scalar(out=maskf[:], in0=scores_sb[:],
                                    scalar1=nlo[:, :1], scalar2=0.0,
                                    op0=Alu.add, op1=Alu.is_ge)
            colsum_p = misc_psum.tile([NT, 1], F32, name="mp")
            nc.tensor.matmul(out=colsum_p[:], lhsT=maskf[:], rhs=o128x1_sb[:],
                             start=True, stop=True)
            colsum = radix.tile([NT, 1], F32, name="colsum")
            nc.vector.tensor_copy(out=colsum[:], in_=colsum_p[:])
            excl_p = misc_psum.tile([NT, 1], F32, name="mp")
            nc.tensor.matmul(out=excl_p[:], lhsT=slt32_sb[:], rhs=colsum[:],
                             start=True, stop=True)
            excl = radix.tile([NT, 1], F32, name="excl")
            nc.vector.tensor_copy(out=excl[:], in_=excl_p[:])
            diag = radix.tile([NT, NT], F32, name="diag")
            nc.vector.tensor_tensor(out=diag[:], in0=id32_sb[:],
                                    in1=excl[:, :1].to_broadcast([NT, NT]),
                                    op=Alu.mult)
            rank_p = misc_psum.tile([P, NT], F32, name="mp")
            nc.tensor.matmul(out=rank_p[:], lhsT=ltri_sb[:], rhs=maskf[:],
                             start=True, stop=False, skip_group_check=True)
            nc.tensor.matmul(out=rank_p[:], lhsT=o32x128_sb[:], rhs=diag[:],
                             start=False, stop=True, skip_group_check=True)
            nc.vector.tensor_scalar(out=offf_c[:], in0=rank_p[:],
                                    scalar1=hb_col[:, :1], scalar2=None,
                                    op0=Alu.subtract)
            nc.vector.tensor_copy(out=maskf_c[:], in_=maskf[:])

        misc_psum_ctx.__exit__(None, None, None)

        # ---- w1 cast-loads on the Pool queue.  Positioned after the radix
        # all_reduces so the in-order queue starts them only ~70us in, after
        # the x-tile DMAs have drained (they'd otherwise steal DMA bandwidth
        # from the critical-path score loads). ---------------------------------
        w1bf = []
        for kd in range(ND):
            t_ = w1_pool.tile([P, DFF], BF16, name=f"w1bf_{kd}")
            nc.gpsimd.dma_start(out=t_[:], in_=w1[kd * P:(kd + 1) * P, :])
            w1bf.append(t_)

        # ---- phase E: digit split + one-hot compaction matmuls --------------
        # off in [0, SEL) for in-window selected tokens; any other off value
        # (negative rank-window miss, >=SEL, or collision of an unselected
        # token) produces no match in the lo-digit equality below, and
        # unselected tokens are additionally zeroed via tokid*mask weights.
        with ExitStack() as SE:
            ep = SE.enter_context(tc.tile_pool(name="epool", bufs=1))
            e_psum = SE.enter_context(tc.tile_pool(name="e_psum", bufs=2,
                                                   space="PSUM"))
            off = offf_c
            eq7a = ep.tile([P, NT, 7], F32, name="eq7a")
            nc.vector.tensor_tensor(
                out=eq7a[:], in0=off[:, :, None].to_broadcast([P, NT, 7]),
                in1=thr128[:, None, :].to_broadcast([P, NT, 7]), op=Alu.is_ge)
            hi128 = ep.tile([P, NT], F32, name="hi128")
            nc.vector.tensor_reduce(out=hi128[:], in_=eq7a[:],
                                    axis=mybir.AxisListType.X, op=Alu.add)
            hm = ep.tile([P, NT], F32, name="hm")
            nc.vector.tensor_scalar(out=hm[:], in0=hi128[:], scalar1=-128.0,
                                    scalar2=None, op0=Alu.mult)
            lo128 = ep.tile([P, NT], F32, name="lo128")
            nc.vector.tensor_tensor(out=lo128[:], in0=off[:], in1=hm[:],
                                    op=Alu.add)
            eq7b = ep.tile([P, NT, 7], F32, name="eq7b")
            nc.vector.tensor_tensor(
                out=eq7b[:], in0=lo128[:, :, None].to_broadcast([P, NT, 7]),
                in1=thr16[:, None, :].to_broadcast([P, NT, 7]), op=Alu.is_ge)
            mid = ep.tile([P, NT], F32, name="mid")
            nc.vector.tensor_reduce(out=mid[:], in_=eq7b[:],
                                    axis=mybir.AxisListType.X, op=Alu.add)
            hm2 = ep.tile([P, NT], F32, name="hm2")
            nc.vector.tensor_scalar(out=hm2[:], in0=mid[:], scalar1=-16.0,
                                    scalar2=None, op0=Alu.mult)
            lo16b = ep.tile([P, NT], BF16, name="lo16b")
            nc.vector.tensor_tensor(out=lo16b[:], in0=lo128[:], in1=hm2[:],
                                    op=Alu.add)
            h8 = ep.tile([P, NT], F32, name="h8")
            nc.vector.tensor_scalar(out=h8[:], in0=hi128[:], scalar1=8.0,
                                    scalar2=None, op0=Alu.mult)
            hi16b = ep.tile([P, NT], BF16, name="hi16b")
            nc.vector.tensor_tensor(out=hi16b[:], in0=h8[:], in1=mid[:],
                                    op=Alu.add)
            # token id = c*128 + p; weight the SMALL equality factors by
            # c*mask (chain C, lhsT cols 0:16) and p*mask (chain D, cols
            # 16:32), then sel16 = 128*C + D.  All factors are small exact
            # integers, so the chain runs in bf16 (1 cycle/row matmuls).
            maskb = ep.tile([P, NT], BF16, name="maskb")
            nc.vector.tensor_copy(out=maskb[:], in_=maskf_c[:])
            cwm = ep.tile([P, NT], BF16, name="cwm")
            nc.vector.tensor_tensor(out=cwm[:], in0=cvalb[:], in1=maskb[:],
                                    op=Alu.mult)
            pwm = ep.tile([P, NT], BF16, name="pwm")
            nc.vector.tensor_tensor(out=pwm[:], in0=maskb[:],
                                    in1=iotab[:, :1].to_broadcast([P, NT]),
                                    op=Alu.mult)

            eq16 = ep.tile([P, NT, 16], BF16, name="eq16")
            nc.vector.tensor_tensor(
                out=eq16[:], in0=iJ16b[:],
                in1=lo16b[:, :, None].to_broadcast([P, NT, 16]), op=Alu.is_equal)
            eqcp = ep.tile([P, NT, 32], BF16, name="eqcp")
            nc.vector.tensor_tensor(
                out=eqcp[:, :, 0:16], in0=eq16[:],
                in1=cwm[:, :, None].to_broadcast([P, NT, 16]), op=Alu.mult)
            nc.vector.tensor_tensor(
                out=eqcp[:, :, 16:32], in0=eq16[:],
                in1=pwm[:, :, None].to_broadcast([P, NT, 16]), op=Alu.mult)
            eq64 = ep.tile([P, NT, 64], BF16, name="eq64")
            nc.vector.tensor_tensor(
                out=eq64[:], in0=iK64b[:],
                in1=hi16b[:, :, None].to_broadcast([P, NT, 64]), op=Alu.is_equal)

            pCD = e_psum.tile([32, 64], F32, name="pCD")
            for c in range(NT):
                nc.tensor.matmul(out=pCD[:], lhsT=eqcp[:, c, :],
                                 rhs=eq64[:, c, :], start=(c == 0),
                                 stop=(c == NT - 1), skip_group_check=True)

            sCD = ep.tile([32, 64], F32, name="sCD")
            nc.vector.tensor_copy(out=sCD[:], in_=pCD[:])

            # scatter index layout [128, 64] (16-wrap replicated to 128);
            # lhsT folds the 128*C + D combine (rows 0:16 scaled by 128)
            rep_ps = e_psum.tile([P, 64], F32, name="rep_ps")
            nc.tensor.matmul(out=rep_ps[:], lhsT=rep16_sb[:], rhs=sCD[:],
                             start=True, stop=True)
            nc.vector.tensor_copy(out=idx16_sb[:], in_=rep_ps[:])  # f32->i16

            # gather index layout [128, 8]: selidx[p, k] = sel16[p%16, 8k+p//16]
            selps = e_psum.tile([P, NSJ], F32, name="selps")
            for g in range(8):
                nc.tensor.matmul(out=selps[:], lhsT=ewrap_sb[:, g * P:(g + 1) * P],
                                 rhs=sCD[:, g::8], start=(g == 0),
                                 stop=(g == 7), skip_group_check=True)
            nc.vector.tensor_copy(out=selidx_sb[:], in_=selps[:])  # f32->i32

        dig_ctx.__exit__(None, None, None)

        # ---- gather + transpose + MLP ---------------------------------------
        if True:
            with ExitStack() as SB:
                xt_pool = SB.enter_context(tc.tile_pool(name="xt", bufs=1))
                xsel_pool = SB.enter_context(tc.tile_pool(name="xsel", bufs=5))
                mm1_psum = SB.enter_context(tc.tile_pool(name="mm1_psum", bufs=6,
                                                         space="PSUM"))

                # xt3[p, kd, t] = x_sel[t, kd*128+p], built by the DMA-engine
                # xbar transpose (one per gathered 128-token chunk)
                xt3 = xt_pool.tile([P, ND, SEL], BF16)
                for j in range(NSJ):
                    xs = xsel_pool.tile([P, D], BF16, name="xsel")
                    nc.gpsimd.indirect_dma_start(
                        out=xs[:], out_offset=None, in_=x_row,
                        in_offset=IndirectOffsetOnAxis(ap=selidx_sb[:, j:j + 1],
                                                       axis=0))
                    nc.scalar.dma_start_transpose(
                        out=xt3[:, :, j * P:(j + 1) * P], in_=xs[:])

                # ---- mm1: ht[m, sel] = gelu(w1^T x_sel^T + b1).  The first
                # four token blocks are 128 wide so the PE starts the moment
                # each transpose lands instead of waiting for four of them;
                # the second half runs as one 512-wide block.
                for t0, tw in [(0, P), (P, P), (2 * P, P), (3 * P, P),
                               (512, 512)]:
                    for m in range(NM):
                        ph = mm1_psum.tile([P, tw], F32, name="ph")
                        for kd in range(ND):
                            nc.tensor.matmul(
                                out=ph[:],
                                lhsT=w1bf[kd][:, m * P:(m + 1) * P],
                                rhs=xt3[:, kd, t0:t0 + tw],
                                start=(kd == 0), stop=(kd == ND - 1),
                            )
                        nc.scalar.activation(
                            out=ht[:, m, t0:t0 + tw], in_=ph[:],
                            func=Act.Gelu_apprx_tanh, bias=b1t_sb[:, m:m + 1],
                            scale=1.0,
                        )

            w1_ctx.__exit__(None, None, None)  # free w1 region for w2 stream

            # ---- mm2: y[sel, D] = ht^T @ w2 + b2, then scatter-add ----------
            with ExitStack() as SY:
                y_pool = SY.enter_context(tc.tile_pool(name="y", bufs=1))
                w2_pool = SY.enter_context(tc.tile_pool(name="w2s", bufs=16))
                mm2_psum = SY.enter_context(tc.tile_pool(name="mm2_psum", bufs=8,
                                                         space="PSUM"))
                # d-half 0: kg-major accumulation (w2 tiles stream in, all 8
                # token-block psums accumulate together)
                n = 0
                y_0 = y_pool.tile([P, NSJ, 512], F32, name="y0")
                pys = [mm2_psum.tile([P, 512], F32, name="py")
                       for _ in range(NSJ)]
                w2n1 = []   # d-half-1 tiles retained for the s-major pass
                for s in range(NSJ):
                    nc.tensor.matmul(
                        out=pys[s][:], lhsT=o1x128b_sb[:],
                        rhs=b2bf_sb[:, :512],
                        start=True, stop=False, skip_group_check=True,
                    )
                for kg in range(NM // NKGRP):
                    w2t = w2_pool.tile([P, NKGRP, 512], BF16, name="w2t")
                    if kg == 0:
                        # WAW gate: keep the w2 stream off the DMA engines
                        # until the gather/transpose pipeline has fed mm1
                        nc.vector.tensor_copy(out=w2t[0:1, 0, 0:1],
                                              in_=ht[0:1, 0, 0:1])
                    src = w2[:, :512].rearrange(
                        "(g p) f -> p g f", p=P)[:, kg * NKGRP:(kg + 1) * NKGRP, :]
                    nc.gpsimd.dma_start(out=w2t[:], in_=src)
                    for ki in range(NKGRP):
                        kk = kg * NKGRP + ki
                        for s in range(NSJ):
                            nc.tensor.matmul(
                                out=pys[s][:],
                                lhsT=ht[:, kk, s * P:(s + 1) * P],
                                rhs=w2t[:, ki, :],
                                start=False, stop=(kk == NM - 1),
                                skip_group_check=True,
                            )
                # prefetch d-half-1 w2 tiles while the n=0 tail accumulates
                for kg in range(NM // NKGRP):
                    w2t = w2_pool.tile([P, NKGRP, 512], BF16, name="w2t")
                    src = w2[:, 512:].rearrange(
                        "(g p) f -> p g f", p=P)[:, kg * NKGRP:(kg + 1) * NKGRP, :]
                    nc.gpsimd.dma_start(out=w2t[:], in_=src)
                    w2n1.append(w2t)
                for s in range(NSJ):
                    nc.scalar.activation(out=y_0[:, s, :], in_=pys[s][:],
                                         func=Act.Copy, bias=0.0, scale=1.0)
                    if s % 4 == 3:
                        h = s // 4
                        nc.gpsimd.dma_scatter_add(
                            out_row[:, :512],
                            y_0[:, h * 4:(h + 1) * 4, :],
                            idx16_sb[:, h * 32:(h + 1) * 32],
                            SEL // 2,
                            SEL // 2,
                            512,
                            elem_step=D,
                        )

                # d-half 1: s-major (each token block finishes early and its
                # rows scatter while the next block accumulates)
                y_1 = y_pool.tile([P, NSJ, 512], F32, name="y1")
                for s in range(NSJ):
                    py = mm2_psum.tile([P, 512], F32, name="py")
                    nc.tensor.matmul(
                        out=py[:], lhsT=o1x128b_sb[:], rhs=b2bf_sb[:, 512:],
                        start=True, stop=False, skip_group_check=True,
                    )
                    for kk in range(NM):
                        nc.tensor.matmul(
                            out=py[:],
                            lhsT=ht[:, kk, s * P:(s + 1) * P],
                            rhs=w2n1[kk // NKGRP][:, kk % NKGRP, :],
                            start=False, stop=(kk == NM - 1),
                            skip_group_check=True,
                        )
                    nc.scalar.activation(out=y_1[:, s, :], in_=py[:],
                                         func=Act.Copy, bias=0.0, scale=1.0)
                    nc.gpsimd.dma_scatter_add(
                        out_row[:, 512:],
                        y_1[:, s:s + 1, :],
                        idx16_sb[:, s * 8:(s + 1) * 8],
                        P,
                        P,
                        512,
                        elem_step=D,
                    )

        ht_ctx.__exit__(None, None, None)

    nc.compile()
    return nc


def make_consts():
    q = np.arange(P)
    import ml_dtypes
    consts = {
        "identb": np.eye(P, dtype=ml_dtypes.bfloat16),
        "ltri128": (q[:, None] < q[None, :]).astype(np.float32),  # [q, p] = q < p
        "slt32": (np.arange(NT)[:, None] < np.arange(NT)[None, :]).astype(np.float32),
        "id32": np.eye(NT, dtype=np.float32),
        "ones_1x128": np.ones((1, P), np.float32),
        "ones_1x128b": np.ones((1, P), ml_dtypes.bfloat16),
        "ones_128x1": np.ones((P, 1), np.float32),
        "ones128": np.ones((P, P), np.float32),
        "ones_32x128": np.ones((NT, P), np.float32),
        "rep16": np.vstack([
            128.0 * (np.arange(16)[:, None] == (np.arange(P)[None, :] % 16)),
            1.0 * (np.arange(16)[:, None] == (np.arange(P)[None, :] % 16)),
        ]).astype(np.float32),
    }
    # ewrap[i, g*128 + p] = 1 iff p == g*16 + i  (16-wrap -> 128-wrap expand);
    # stacked [32, .]: rows 0:16 scaled by 128 (C chain), rows 16:32 raw (D)
    ew = np.zeros((16, 8 * P), np.float32)
    for i in range(16):
        for g in range(8):
            ew[i, g * P + g * 16 + i] = 1.0
    consts["ewrap"] = np.vstack([128.0 * ew, ew]).astype(np.float32)
    return consts


def make_in_maps(x, W1, b1, W2, b2, wr, br):
    consts = make_consts()
    x = np.ascontiguousarray(np.asarray(x, np.float32))
    in_maps = []
    for c in range(NCORES):
        b, h = divmod(c, 2)
        m = {
            "x_row": x[b],
            "w1": np.asarray(W1, np.float32),
            "w2": np.asarray(W2, np.float32),
            "wr": np.asarray(wr, np.float32).reshape(1, D),
            "b1t": np.ascontiguousarray(np.asarray(b1, np.float32).reshape(NM, P).T),
            "b2": np.asarray(b2, np.float32).reshape(1, D),
            "hbase": np.array([[h * SEL]], np.float32),
        }
        m.update(consts)
        in_maps.append(m)
    return in_maps


_NC_CACHE = None


def _get_program():
    global _NC_CACHE
    if _NC_CACHE is None:
        _NC_CACHE = build_program()
    return _NC_CACHE


def kernel(x, W1, b1, W2, b2, wr, br):
    from concourse.bass_utils import run_bass_kernel_spmd

    nc = _get_program()
    in_maps = make_in_maps(x, W1, b1, W2, b2, wr, br)
    res = run_bass_kernel_spmd(nc, in_maps, list(range(NCORES))).results
    out = np.stack(
        [res[2 * b]["out_row"] + res[2 * b + 1]["out_row"] for b in range(B)]
    )
    return out.astype(np.float32)


# revision 66
# speedup vs baseline: 2.6014x; 1.0054x over previous
"""MoD (mixture-of-depths) MLP wrapper kernel for Trainium2, 8 NeuronCores.

Sharding: core c handles batch row b = c//2 and the half of that row's
top-K tokens with global selection ranks in [h*1024, (h+1)*1024), h = c%2.
Each core computes the full row's router scores + top-K threshold locally
(no collectives), gathers exactly 1024 token rows by rank via indirect DMA,
runs the FFN in bf16 (fp32 accumulation), and scatters results back into the
pre-zeroed per-core output buffer with dma_scatter_add.  Host sums the two
buffers of each row.

Schedule: x-tile loads own the DMA engines first (weight loads are ordered
behind them); radix pass 1 folds into the score loop against a constant
threshold grid; passes 2-4 run as Sign-activation counts over a
DMA-broadcast score replica; rank compaction is a digit-decomposed one-hot
bf16 matmul whose stacked constants emit both the int32 gather and int16
scatter index layouts; gathered tokens are transposed by the DMA xbar
(dma_start_transpose); and the output scatter is dma_scatter_add (per-index
descriptors) overlapped with the tail of the second matmul.
"""

import sys

sys.path.insert(0, "/opt/trn_rl_repo")

from contextlib import ExitStack

import numpy as np

from concourse import bass, bass_isa, mybir
from concourse import bacc
import concourse.tile as tile
from concourse.bass import IndirectOffsetOnAxis

B, L, D = 4, 4096, 1024
DFF = 4 * D
K = L // 2              # 2048 selected tokens per row
NCORES = 8
P = 128
NT = L // P             # 32 token tiles per row
SEL = K // 2            # 1024 selected tokens per core
NSJ = SEL // P          # 8 selected-token blocks
ND = D // P             # 8 d chunks
NM = DFF // P           # 32 dff tiles
NKGRP = 4               # w2 k-chunks per streamed tile
RADIX_PASSES = 4

F32 = mybir.dt.float32
BF16 = mybir.dt.bfloat16
I32 = mybir.dt.int32
I16 = mybir.dt.int16
Alu = mybir.AluOpType
Act = mybir.ActivationFunctionType
Red = bass_isa.ReduceOp


def build_program():
    nc = bacc.Bacc(
        "TRN2",
        target_bir_lowering=False,
        debug=False,
        enable_asserts=False,
        num_devices=NCORES,
    )

    x_row = nc.dram_tensor("x_row", [L, D], F32, kind="ExternalInput").ap()
    w1 = nc.dram_tensor("w1", [D, DFF], F32, kind="ExternalInput").ap()
    w2 = nc.dram_tensor("w2", [DFF, D], F32, kind="ExternalInput").ap()
    wr = nc.dram_tensor("wr", [1, D], F32, kind="ExternalInput").ap()
    b1t = nc.dram_tensor("b1t", [P, NM], F32, kind="ExternalInput").ap()
    b2 = nc.dram_tensor("b2", [1, D], F32, kind="ExternalInput").ap()
    hbase = nc.dram_tensor("hbase", [1, 1], F32, kind="ExternalInput").ap()
    identb = nc.dram_tensor("identb", [P, P], BF16, kind="ExternalInput").ap()
    ltri = nc.dram_tensor("ltri128", [P, P], F32, kind="ExternalInput").ap()
    slt32 = nc.dram_tensor("slt32", [NT, NT], F32, kind="ExternalInput").ap()
    id32 = nc.dram_tensor("id32", [NT, NT], F32, kind="ExternalInput").ap()
    ones_1x128 = nc.dram_tensor("ones_1x128", [1, P], F32, kind="ExternalInput").ap()
    ones_1x128b = nc.dram_tensor("ones_1x128b", [1, P], BF16, kind="ExternalInput").ap()
    ones_128x1 = nc.dram_tensor("ones_128x1", [P, 1], F32, kind="ExternalInput").ap()
    ones128 = nc.dram_tensor("ones128", [P, P], F32, kind="ExternalInput").ap()
    ones_32x128 = nc.dram_tensor("ones_32x128", [NT, P], F32, kind="ExternalInput").ap()
    rep16 = nc.dram_tensor("rep16", [32, P], F32, kind="ExternalInput").ap()
    ewrap = nc.dram_tensor("ewrap", [32, 8 * P], F32, kind="ExternalInput").ap()

    out_row = nc.dram_tensor("out_row", [L, D], F32, kind="ExternalOutput").ap()

    GRPS = ((0, 20), (20, 30), (30, 31), (31, 32))
    scores_dg = [nc.dram_tensor(f"scores_dg{i}", [hi - lo, P], F32).ap()
                 for i, (lo, hi) in enumerate(GRPS)]

    with tile.TileContext(nc) as tc, ExitStack() as S0:
        const = S0.enter_context(tc.tile_pool(name="const", bufs=1))
        # pool stack (LIFO): const | ht | w1 | dig | ...phases
        ht_ctx = tc.tile_pool(name="ht", bufs=1)
        ht_pool = ht_ctx.__enter__()
        ht = ht_pool.tile([P, NM, SEL], BF16)
        w1_ctx = tc.tile_pool(name="w1bf", bufs=1)
        w1_pool = w1_ctx.__enter__()

        def cload(pool, ap, shape, dtype=F32, name=None):
            t = pool.tile(shape, dtype, name=name)
            nc.sync.dma_start(out=t[:], in_=ap)
            return t

        # ---- SP-queue order: wr, o1, oc, hbase FIRST (phase A needs them) ---
        wr_sb = cload(const, wr, [1, D], name="c_wr")
        o1x128_sb = cload(const, ones_1x128, [1, P], name="c_o1")
        o128x1_sb = cload(const, ones_128x1, [P, 1], name="c_oc")
        ones128_sb = cload(const, ones128, [P, P], name="c_o128")
        hb_sb = cload(const, hbase, [1, 1], name="c_hb")

        # w1 tiles exist from the start (loads are issued after the radix)
        w1bf = [w1_pool.tile([P, DFF], BF16, name=f"w1bf_{kd}")
                for kd in range(ND)]

        # ---- Pool-queue iotas (independent of SP queue) ---------------------
        # big digit-decomposition iota tables live only through phase E
        dig_ctx = tc.tile_pool(name="dig", bufs=1)
        dig = dig_ctx.__enter__()

        iota_i = const.tile([P, 1], I32)
        nc.gpsimd.iota(iota_i[:], pattern=[[1, 1]], base=0, channel_multiplier=1)
        tokid = const.tile([P, NT], I32)
        nc.gpsimd.iota(tokid[:], pattern=[[P, NT]], base=0, channel_multiplier=1)
        iC_i = const.tile([P, NT], I32)
        nc.gpsimd.iota(iC_i[:], pattern=[[1, NT]], base=0, channel_multiplier=0)
        iQ_i = const.tile([P, 128], I32)
        nc.gpsimd.iota(iQ_i[:], pattern=[[1, 128]], base=0, channel_multiplier=0)
        iK64_i = dig.tile([P, NT, 64], I16)
        nc.gpsimd.iota(iK64_i[:], pattern=[[0, NT], [1, 64]], base=0,
                       channel_multiplier=0)
        iJ16_i = dig.tile([P, NT, 16], I16)
        nc.gpsimd.iota(iJ16_i[:], pattern=[[0, NT], [1, 16]], base=0,
                       channel_multiplier=0)
        i7_i = const.tile([P, 7], I32)
        nc.gpsimd.iota(i7_i[:], pattern=[[1, 7]], base=1, channel_multiplier=0)

        iota_f = const.tile([P, 1], F32)
        nc.vector.tensor_copy(out=iota_f[:], in_=iota_i[:])
        tokidf = const.tile([P, NT], F32)
        nc.vector.tensor_copy(out=tokidf[:], in_=tokid[:])
        cvalf = const.tile([P, NT], F32)
        nc.vector.tensor_copy(out=cvalf[:], in_=iC_i[:])
        iK64b = dig.tile([P, NT, 64], BF16)
        nc.vector.tensor_copy(out=iK64b[:], in_=iK64_i[:])
        iJ16b = dig.tile([P, NT, 16], BF16)
        nc.vector.tensor_copy(out=iJ16b[:], in_=iJ16_i[:])
        iotab = const.tile([P, 1], BF16)
        nc.vector.tensor_copy(out=iotab[:], in_=iota_i[:])
        cvalb = const.tile([P, NT], BF16)
        nc.vector.tensor_copy(out=cvalb[:], in_=iC_i[:])
        i7f = const.tile([P, 7], F32)
        nc.vector.tensor_copy(out=i7f[:], in_=i7_i[:])
        thr128 = const.tile([P, 7], F32)
        nc.vector.tensor_scalar(out=thr128[:], in0=i7f[:], scalar1=128.0,
                                scalar2=None, op0=Alu.mult)
        thr16 = const.tile([P, 7], F32)
        nc.vector.tensor_scalar(out=thr16[:], in0=i7f[:], scalar1=16.0,
                                scalar2=None, op0=Alu.mult)
        # radix pass-1 threshold grid (build-time constants: lo=-16, w=0.25)
        iQf = const.tile([P, 128], F32)
        nc.vector.tensor_copy(out=iQf[:], in_=iQ_i[:])
        thr1row = const.tile([P, 128], F32)
        nc.vector.tensor_scalar(out=thr1row[:], in0=iQf[:], scalar1=32.0 / P,
                                scalar2=-16.0, op0=Alu.mult, op1=Alu.add)
        # negated per-pass threshold offsets for radix passes 2..4
        W1P = 32.0 / P
        nthrbs = []
        for p_ in range(1, RADIX_PASSES):
            w_p = W1P / (P ** p_)
            t_ = const.tile([P, 1], F32, name=f"nthrb{p_}")
            nc.vector.tensor_scalar(out=t_[:], in0=iota_f[:], scalar1=-w_p,
                                    scalar2=None, op0=Alu.mult)
            nthrbs.append((w_p, t_))
        hb_col = const.tile([P, 1], F32)
        nc.gpsimd.partition_broadcast(hb_col[:], hb_sb[:])

        scores_sb = const.tile([P, NT], F32)
        selidx_sb = const.tile([P, NSJ], I32)
        idx16_sb = const.tile([P, SEL // 16], I16)

        misc_psum_ctx = tc.tile_pool(name="misc_psum", bufs=2, space="PSUM")
        misc_psum = misc_psum_ctx.__enter__()

        # ---- phase A: router scores (fp32, exact; router bias dropped — it
        # shifts every score equally so the top-K set is unchanged).  The
        # first radix pass uses a build-time-constant threshold grid, so its
        # per-tile compare + count-matmul accumulation is folded in here. -----
        c1_psum_ctx = tc.tile_pool(name="c1_psum", bufs=1, space="PSUM")
        c1_psum = c1_psum_ctx.__enter__()
        cnt1_ps = c1_psum.tile([P, 128], F32, name="cnt1")
        nlo = const.tile([P, 1], F32, name="nlo")
        with ExitStack() as SA:
            apool = SA.enter_context(tc.tile_pool(name="apool", bufs=1))
            xs_pool = SA.enter_context(tc.tile_pool(name="xs", bufs=6))
            junk_pool = SA.enter_context(tc.tile_pool(name="junk", bufs=2))
            cmp_pool = SA.enter_context(tc.tile_pool(name="cmp", bufs=3))

            wrb = apool.tile([P, D], F32)
            for n in range(D // 512):
                pt = misc_psum.tile([P, 512], F32, name="mp")
                nc.tensor.matmul(out=pt[:], lhsT=o1x128_sb[:],
                                 rhs=wr_sb[:, n * 512:(n + 1) * 512],
                                 start=True, stop=True)
                nc.vector.tensor_copy(out=wrb[:, n * 512:(n + 1) * 512], in_=pt[:])

            x_last = None
            for t in range(NT):
                x_t = xs_pool.tile([P, D], F32)
                nc.sync.dma_start(out=x_t[:], in_=x_row[t * P:(t + 1) * P, :])
                x_last = x_t
                if t == 26:
                    nc.sync.dma_start(
                        out=scores_dg[0].rearrange("c p -> p c"),
                        in_=scores_sb[:, 0:20])
                prod = junk_pool.tile([P, D], F32, name="prod")
                nc.vector.tensor_tensor(out=prod[:], in0=x_t[:], in1=wrb[:],
                                        op=Alu.mult)
                sink = junk_pool.tile([P, D], BF16, name="sink")
                nc.scalar.activation(out=sink[:], in_=prod[:], func=Act.Identity,
                                     bias=0.0, scale=1.0,
                                     accum_out=scores_sb[:, t:t + 1])
                cmp_t = cmp_pool.tile([P, 128], F32, name="cmp")
                nc.vector.tensor_tensor(
                    out=cmp_t[:],
                    in0=scores_sb[:, t:t + 1].to_broadcast([P, 128]),
                    in1=thr1row[:], op=Alu.is_ge)
                nc.tensor.matmul(out=cnt1_ps[:], lhsT=ones128_sb[:], rhs=cmp_t[:],
                                 start=(t == 0), stop=(t == NT - 1),
                                 skip_group_check=True)


            # pass-1 finalize on every partition (count matmul used an
            # all-ones lhsT, so each partition holds the full count row):
            # nlo = -(lo1) = 16 - (sum(cnt>=K) - 1)*0.25
            selr = apool.tile([P, 128], F32, name="selr")
            nc.vector.tensor_scalar(out=selr[:], in0=cnt1_ps[:],
                                    scalar1=float(K), scalar2=None,
                                    op0=Alu.is_ge)
            s1 = apool.tile([P, 1], F32, name="s1")
            nc.vector.tensor_reduce(out=s1[:], in_=selr[:],
                                    axis=mybir.AxisListType.X, op=Alu.add)
            q1 = apool.tile([P, 1], F32, name="q1")
            nc.vector.tensor_scalar(out=q1[:], in0=s1[:], scalar1=-1.0,
                                    scalar2=-W1P, op0=Alu.add, op1=Alu.mult)
            nc.vector.tensor_scalar(out=nlo[:], in0=q1[:], scalar1=16.0,
                                    scalar2=None, op0=Alu.add)
        c1_psum_ctx.__exit__(None, None, None)

        offf_c = const.tile([P, NT], F32)
        maskf_c = const.tile([P, NT], F32)

        # ---- phases B+C+D: replicate scores, radix threshold, rank ----------
        with ExitStack() as SC:
            radix = SC.enter_context(tc.tile_pool(name="radix", bufs=2))
            rep_pool = SC.enter_context(tc.tile_pool(name="rep", bufs=1))

            # broadcast-read the spilled scores, one DMA per 1024-token group,
            # FIRST on the in-order SP queue right after the x loads (the
            # remaining const loads queue behind, they aren't needed till later)
            scores_rep = rep_pool.tile([P, L], F32)
            for gi, (glo, ghi) in enumerate(GRPS):
                n_ = (ghi - glo) * P
                if gi > 0:   # g0 was spilled inside the x stream
                    nc.sync.dma_start(
                        out=scores_dg[gi].rearrange("c p -> p c"),
                        in_=scores_sb[:, glo:ghi])
                nc.sync.dma_start(
                    out=scores_rep[:, glo * P:ghi * P],
                    in_=scores_dg[gi].rearrange("c p -> () (c p)")
                    .to_broadcast([P, n_]))

            # gate the w1 cast-loads behind the score broadcast so their DMAs
            # cannot delay it (WAW edge: the w1 DMA overwrites the gate byte)
            for kd in range(ND):
                nc.vector.tensor_copy(out=w1bf[kd][0:1, 0:1],
                                      in_=scores_rep[0:1, kd:kd + 1])

            # ---- remaining small consts on the SP queue ---------------------
            b1t_sb = cload(const, b1t, [P, NM], name="c_b1t")
            identb_sb = cload(const, identb, [P, P], BF16, name="c_idb")
            ltri_sb = cload(const, ltri, [P, P], name="c_lt")
            slt32_sb = cload(const, slt32, [NT, NT], name="c_sl")
            id32_sb = cload(const, id32, [NT, NT], name="c_id32")
            o1x128b_sb = cload(const, ones_1x128b, [1, P], BF16, name="c_o1b")
            o32x128_sb = cload(const, ones_32x128, [NT, P], name="c_o32")
            rep16_sb = cload(const, rep16, [32, P], name="c_rep16")
            ewrap_sb = cload(const, ewrap, [32, 8 * P], name="c_ew")
            b2bf_sb = const.tile([1, D], BF16)
            nc.gpsimd.dma_start(out=b2bf_sb[:], in_=b2)  # cast f32 -> bf16

            sjunk = rep_pool.tile([P, L], BF16, name="sjunk")
            NSPL = 2560          # tokens covered by broadcast group 0
            for pi, (w_p, nthrb_p) in enumerate(nthrbs):
                nthr = radix.tile([P, 1], F32, name="nthr")
                nc.vector.tensor_tensor(out=nthr[:], in0=nlo[:], in1=nthrb_p[:],
                                        op=Alu.add)
                if pi == 0:
                    # group-0 scores arrive first; count them while the tail
                    # broadcasts finish, then add the remainder
                    sgA = radix.tile([P, 1], F32, name="sgA")
                    nc.scalar.activation(out=sjunk[:, :NSPL],
                                         in_=scores_rep[:, :NSPL],
                                         func=Act.Sign, bias=nthr[:, :1],
                                         scale=1.0, accum_out=sgA[:])
                    sgB = radix.tile([P, 1], F32, name="sgB")
                    nc.scalar.activation(out=sjunk[:, NSPL:],
                                         in_=scores_rep[:, NSPL:],
                                         func=Act.Sign, bias=nthr[:, :1],
                                         scale=1.0, accum_out=sgB[:])
                    sgn = radix.tile([P, 1], F32, name="sgn")
                    nc.vector.tensor_tensor(out=sgn[:], in0=sgA[:], in1=sgB[:],
                                            op=Alu.add)
                else:
                    sgn = radix.tile([P, 1], F32, name="sgn")
                    nc.scalar.activation(out=sjunk[:], in_=scores_rep[:],
                                         func=Act.Sign, bias=nthr[:, :1],
                                         scale=1.0, accum_out=sgn[:])
                sel = radix.tile([P, 1], F32, name="sel")
                nc.vector.tensor_scalar(out=sel[:], in0=sgn[:], scalar1=0.0,
                                        scalar2=None, op0=Alu.is_ge)
                s_all = radix.tile([P, 1], F32, name="s_all")
                nc.gpsimd.partition_all_reduce(s_all[:], sel[:], channels=P,
                                               reduce_op=Red.add)
                nd = radix.tile([P, 1], F32, name="nd")
                nc.vector.tensor_scalar(out=nd[:], in0=s_all[:], scalar1=-1.0,
                                        scalar2=-w_p, op0=Alu.add, op1=Alu.mult)
                nlo2 = radix.tile([P, 1], F32, name="nlo2")
                nc.vector.tensor_tensor(out=nlo2[:], in0=nlo[:], in1=nd[:],
                                        op=Alu.add)
                nlo = nlo2

            # ---- mask + global rank (exclusive prefix of mask) --------------
            maskf = radix.tile([P, NT], F32, name="maskf")
            nc.vector.tensor_scalar(out=maskf[:], in0=scores_sb[:],
                                    scalar1=nlo[:, :1], scalar2=0.0,
                                    op0=Alu.add, op1=Alu.is_ge)
            colsum_p = misc_psum.tile([NT, 1], F32, name="mp")
            nc.tensor.matmul(out=colsum_p[:], lhsT=maskf[:], rhs=o128x1_sb[:],
                             start=True, stop=True)
            colsum = radix.tile([NT, 1], F32, name="colsum")
            nc.vector.tensor_copy(out=colsum[:], in_=colsum_p[:])
            excl_p = misc_psum.tile([NT, 1], F32, name="mp")
            nc.tensor.matmul(out=excl_p[:], lhsT=slt32_sb[:], rhs=colsum[:],
                             start=True, stop=True)
            excl = radix.tile([NT, 1], F32, name="excl")
            nc.vector.tensor_copy(out=excl[:], in_=excl_p[:])
            diag = radix.tile([NT, NT], F32, name="diag")
            nc.vector.tensor_tensor(out=diag[:], in0=id32_sb[:],
                                    in1=excl[:, :1].to_broadcast([NT, NT]),
                                    op=Alu.mult)
            rank_p = misc_psum.tile([P, NT], F32, name="mp")
            nc.tensor.matmul(out=rank_p[:], lhsT=ltri_sb[:], rhs=maskf[:],
                             start=True, stop=False, skip_group_check=True)
            nc.tensor.matmul(out=rank_p[:], lhsT=o32x128_sb[:], rhs=diag[:],
                             start=False, stop=True, skip_group_check=True)
            nc.vector.tensor_scalar(out=offf_c[:], in0=rank_p[:],
                                    scalar1=hb_col[:, :1], scalar2=None,
                                    op0=Alu.subtract)
            nc.vector.tensor_copy(out=maskf_c[:], in_=maskf[:])

        misc_psum_ctx.__exit__(None, None, None)

        # ---- w1 cast-loads on the Pool queue.  Positioned after the radix
        # all_reduces so the in-order queue starts them only ~70us in, after
        # the x-tile DMAs have drained (they'd otherwise steal DMA bandwidth
        # from the critical-path score loads). ---------------------------------
        w1bf = []
        for kd in range(ND):
            t_ = w1_pool.tile([P, DFF], BF16, name=f"w1bf_{kd}")
            nc.gpsimd.dma_start(out=t_[:], in_=w1[kd * P:(kd + 1) * P, :])
            w1bf.append(t_)

        # ---- phase E: digit split + one-hot compaction matmuls --------------
        # off in [0, SEL) for in-window selected tokens; any other off value
        # (negative rank-window miss, >=SEL, or collision of an unselected
        # token) produces no match in the lo-digit equality below, and
        # unselected tokens are additionally zeroed via tokid*mask weights.
        with ExitStack() as SE:
            ep = SE.enter_context(tc.tile_pool(name="epool", bufs=1))
            e_psum = SE.enter_context(tc.tile_pool(name="e_psum", bufs=2,
                                                   space="PSUM"))
            off = offf_c
            eq7a = ep.tile([P, NT, 7], F32, name="eq7a")
            nc.vector.tensor_tensor(
                out=eq7a[:], in0=off[:, :, None].to_broadcast([P, NT, 7]),
                in1=thr128[:, None, :].to_broadcast([P, NT, 7]), op=Alu.is_ge)
            hi128 = ep.tile([P, NT], F32, name="hi128")
            nc.vector.tensor_reduce(out=hi128[:], in_=eq7a[:],
                                    axis=mybir.AxisListType.X, op=Alu.add)
            hm = ep.tile([P, NT], F32, name="hm")
            nc.vector.tensor_scalar(out=hm[:], in0=hi128[:], scalar1=-128.0,
                                    scalar2=None, op0=Alu.mult)
            lo128 = ep.tile([P, NT], F32, name="lo128")
            nc.vector.tensor_tensor(out=lo128[:], in0=off[:], in1=hm[:],
                                    op=Alu.add)
            eq7b = ep.tile([P, NT, 7], F32, name="eq7b")
            nc.vector.tensor_tensor(
                out=eq7b[:], in0=lo128[:, :, None].to_broadcast([P, NT, 7]),
                in1=thr16[:, None, :].to_broadcast([P, NT, 7]), op=Alu.is_ge)
            mid = ep.tile([P, NT], F32, name="mid")
            nc.vector.tensor_reduce(out=mid[:], in_=eq7b[:],
                                    axis=mybir.AxisListType.X, op=Alu.add)
            hm2 = ep.tile([P, NT], F32, name="hm2")
            nc.vector.tensor_scalar(out=hm2[:], in0=mid[:], scalar1=-16.0,
                                    scalar2=None, op0=Alu.mult)
            lo16b = ep.tile([P, NT], BF16, name="lo16b")
            nc.vector.tensor_tensor(out=lo16b[:], in0=lo128[:], in1=hm2[:],
                                    op=Alu.add)
            h8 = ep.tile([P, NT], F32, name="h8")
            nc.vector.tensor_scalar(out=h8[:], in0=hi128[:], scalar1=8.0,
                                    scalar2=None, op0=Alu.mult)
            hi16b = ep.tile([P, NT], BF16, name="hi16b")
            nc.vector.tensor_tensor(out=hi16b[:], in0=h8[:], in1=mid[:],
                                    op=Alu.add)
            # token id = c*128 + p; weight the SMALL equality factors by
            # c*mask (chain C, lhsT cols 0:16) and p*mask (chain D, cols
            # 16:32), then sel16 = 128*C + D.  All factors are small exact
            # integers, so the chain runs in bf16 (1 cycle/row matmuls).
            maskb = ep.tile([P, NT], BF16, name="maskb")
            nc.vector.tensor_copy(out=maskb[:], in_=maskf_c[:])
            cwm = ep.tile([P, NT], BF16, name="cwm")
            nc.vector.tensor_tensor(out=cwm[:], in0=cvalb[:], in1=maskb[:],
                                    op=Alu.mult)
            pwm = ep.tile([P, NT], BF16, name="pwm")
            nc.vector.tensor_tensor(out=pwm[:], in0=maskb[:],
                                    in1=iotab[:, :1].to_broadcast([P, NT]),
                                    op=Alu.mult)

            eq16 = ep.tile([P, NT, 16], BF16, name="eq16")
            nc.vector.tensor_tensor(
                out=eq16[:], in0=iJ16b[:],
                in1=lo16b[:, :, None].to_broadcast([P, NT, 16]), op=Alu.is_equal)
            eqcp = ep.tile([P, NT, 32], BF16, name="eqcp")
            nc.vector.tensor_tensor(
                out=eqcp[:, :, 0:16], in0=eq16[:],
                in1=cwm[:, :, None].to_broadcast([P, NT, 16]), op=Alu.mult)
            nc.vector.tensor_tensor(
                out=eqcp[:, :, 16:32], in0=eq16[:],
                in1=pwm[:, :, None].to_broadcast([P, NT, 16]), op=Alu.mult)
            eq64 = ep.tile([P, NT, 64], BF16, name="eq64")
            nc.vector.tensor_tensor(
                out=eq64[:], in0=iK64b[:],
                in1=hi16b[:, :, None].to_broadcast([P, NT, 64]), op=Alu.is_equal)

            pCD = e_psum.tile([32, 64], F32, name="pCD")
            for c in range(NT):
                nc.tensor.matmul(out=pCD[:], lhsT=eqcp[:, c, :],
                                 rhs=eq64[:, c, :], start=(c == 0),
                                 stop=(c == NT - 1), skip_group_check=True)

            sCD = ep.tile([32, 64], F32, name="sCD")
            nc.vector.tensor_copy(out=sCD[:], in_=pCD[:])

            # scatter index layout [128, 64] (16-wrap replicated to 128);
            # lhsT folds the 128*C + D combine (rows 0:16 scaled by 128)
            rep_ps = e_psum.tile([P, 64], F32, name="rep_ps")
            nc.tensor.matmul(out=rep_ps[:], lhsT=rep16_sb[:], rhs=sCD[:],
                             start=True, stop=True)
            nc.vector.tensor_copy(out=idx16_sb[:], in_=rep_ps[:])  # f32->i16

            # gather index layout [128, 8]: selidx[p, k] = sel16[p%16, 8k+p//16]
            selps = e_psum.tile([P, NSJ], F32, name="selps")
            for g in range(8):
                nc.tensor.matmul(out=selps[:], lhsT=ewrap_sb[:, g * P:(g + 1) * P],
                                 rhs=sCD[:, g::8], start=(g == 0),
                                 stop=(g == 7), skip_group_check=True)
            nc.vector.tensor_copy(out=selidx_sb[:], in_=selps[:])  # f32->i32

        dig_ctx.__exit__(None, None, None)

        # ---- gather + transpose + MLP ---------------------------------------
        if True:
            with ExitStack() as SB:
                xt_pool = SB.enter_context(tc.tile_pool(name="xt", bufs=1))
                xsel_pool = SB.enter_context(tc.tile_pool(name="xsel", bufs=5))
                mm1_psum = SB.enter_context(tc.tile_pool(name="mm1_psum", bufs=6,
                                                         space="PSUM"))

                # xt3[p, kd, t] = x_sel[t, kd*128+p], built by the DMA-engine
                # xbar transpose (one per gathered 128-token chunk)
                xt3 = xt_pool.tile([P, ND, SEL], BF16)
                for j in range(NSJ):
                    xs = xsel_pool.tile([P, D], BF16, name="xsel")
                    nc.gpsimd.indirect_dma_start(
                        out=xs[:], out_offset=None, in_=x_row,
                        in_offset=IndirectOffsetOnAxis(ap=selidx_sb[:, j:j + 1],
                                                       axis=0))
                    nc.scalar.dma_start_transpose(
                        out=xt3[:, :, j * P:(j + 1) * P], in_=xs[:])

                # ---- mm1: ht[m, sel] = gelu(w1^T x_sel^T + b1).  The first
                # four token blocks are 128 wide so the PE starts the moment
                # each transpose lands instead of waiting for four of them;
                # the second half runs as one 512-wide block.
                for t0, tw in [(0, P), (P, P), (2 * P, P), (3 * P, P),
                               (512, 512)]:
                    for m in range(NM):
                        ph = mm1_psum.tile([P, tw], F32, name="ph")
                        for kd in range(ND):
                            nc.tensor.matmul(
                                out=ph[:],
                                lhsT=w1bf[kd][:, m * P:(m + 1) * P],
                                rhs=xt3[:, kd, t0:t0 + tw],
                                start=(kd == 0), stop=(kd == ND - 1),
                            )
                        nc.scalar.activation(
                            out=ht[:, m, t0:t0 + tw], in_=ph[:],
                            func=Act.Gelu_apprx_tanh, bias=b1t_sb[:, m:m + 1],
                            scale=1.0,
                        )

            w1_ctx.__exit__(None, None, None)  # free w1 region for w2 stream

            # ---- mm2: y[sel, D] = ht^T @ w2 + b2, then scatter-add ----------
            with ExitStack() as SY:
                y_pool = SY.enter_context(tc.tile_pool(name="y", bufs=1))
                w2_pool = SY.enter_context(tc.tile_pool(name="w2s", bufs=16))
                mm2_psum = SY.enter_context(tc.tile_pool(name="mm2_psum", bufs=8,
                                                         space="PSUM"))
                # d-half 0: kg-major accumulation (w2 tiles stream in, all 8
                # token-block psums accumulate together)
                n = 0
                y_0 = y_pool.tile([P, NSJ, 512], F32, name="y0")
                pys = [mm2_psum.tile([P, 512], F32, name="py")
                       for _ in range(NSJ)]
                w2n1 = []   # d-half-1 tiles retained for the s-major pass
                for s in range(NSJ):
                    nc.tensor.matmul(
                        out=pys[s][:], lhsT=o1x128b_sb[:],
                        rhs=b2bf_sb[:, :512],
                        start=True, stop=False, skip_group_check=True,
                    )
                for kg in range(NM // NKGRP):
                    w2t = w2_pool.tile([P, NKGRP, 512], BF16, name="w2t")
                    if kg == 0:
                        # WAW gate: keep the w2 stream off the DMA engines
                        # until the gather/transpose pipeline has fed mm1
                        nc.vector.tensor_copy(out=w2t[0:1, 0, 0:1],
                                              in_=ht[0:1, 0, 0:1])
                    src = w2[:, :512].rearrange(
                        "(g p) f -> p g f", p=P)[:, kg * NKGRP:(kg + 1) * NKGRP, :]
                    nc.gpsimd.dma_start(out=w2t[:], in_=src)
                    for ki in range(NKGRP):
                        kk = kg * NKGRP + ki
                        for s in range(NSJ):
                            nc.tensor.matmul(
                                out=pys[s][:],
                                lhsT=ht[:, kk, s * P:(s + 1) * P],
                                rhs=w2t[:, ki, :],
                                start=False, stop=(kk == NM - 1),
                                skip_group_check=True,
                            )
                # prefetch d-half-1 w2 tiles while the n=0 tail accumulates
                for kg in range(NM // NKGRP):
                    w2t = w2_pool.tile([P, NKGRP, 512], BF16, name="w2t")
                    src = w2[:, 512:].rearrange(
                        "(g p) f -> p g f", p=P)[:, kg * NKGRP:(kg + 1) * NKGRP, :]
                    nc.gpsimd.dma_start(out=w2t[:], in_=src)
                    w2n1.append(w2t)
                for s in range(NSJ):
                    nc.scalar.activation(out=y_0[:, s, :], in_=pys[s][:],
                                         func=Act.Copy, bias=0.0, scale=1.0)
                    if s % 4 == 3:
                        h = s // 4
                        nc.gpsimd.dma_scatter_add(
                            out_row[:, :512],
                            y_0[:, h * 4:(h + 1) * 4, :],
                            idx16_sb[:, h * 32:(h + 1) * 32],
                            SEL // 2,
                            SEL // 2,
                            512,
                            elem_step=D,
                        )

                # d-half 1: s-major (each token block finishes early and its
                # rows scatter while the next block accumulates)
                y_1 = y_pool.tile([P, NSJ, 512], F32, name="y1")
                for s in range(NSJ):
                    py = mm2_psum.tile([P, 512], F32, name="py")
                    nc.tensor.matmul(
                        out=py[:], lhsT=o1x128b_sb[:], rhs=b2bf_sb[:, 512:],
                        start=True, stop=False, skip_group_check=True,
                    )
                    for kk in range(NM):
                        nc.tensor.matmul(
                            out=py[:],
                            lhsT=ht[:, kk, s * P:(s + 1) * P],
                            rhs=w2n1[kk // NKGRP][:, kk % NKGRP, :],
                            start=False, stop=(kk == NM - 1),
                            skip_group_check=True,
                        )
                    nc.scalar.activation(out=y_1[:, s, :], in_=py[:],
                                         func=Act.Copy, bias=0.0, scale=1.0)
                    nc.gpsimd.dma_scatter_add(
                        out_row[:, 512:],
                        y_1[:, s:s + 1, :],
                        idx16_sb[:, s * 8:(s + 1) * 8],
                        P,
                        P,
                        512,
                        elem_step=D,
                    )

        ht_ctx.__exit__(None, None, None)

    nc.compile()
    return nc


def make_consts():
    q = np.arange(P)
    import ml_dtypes
    consts = {
        "identb": np.eye(P, dtype=ml_dtypes.bfloat16),
        "ltri128": (q[:, None] < q[None, :]).astype(np.float32),  # [q, p] = q < p
        "slt32": (np.arange(NT)[:, None] < np.arange(NT)[None, :]).astype(np.float32),
        "id32": np.eye(NT, dtype=np.float32),
        "ones_1x128": np.ones((1, P), np.float32),
        "ones_1x128b": np.ones((1, P), ml_dtypes.bfloat16),
        "ones_128x1": np.ones((P, 1), np.float32),
        "ones128": np.ones((P, P), np.float32),
        "ones_32x128": np.ones((NT, P), np.float32),
        "rep16": np.vstack([
            128.0 * (np.arange(16)[:, None] == (np.arange(P)[None, :] % 16)),
            1.0 * (np.arange(16)[:, None] == (np.arange(P)[None, :] % 16)),
        ]).astype(np.float32),
    }
    # ewrap[i, g*128 + p] = 1 iff p == g*16 + i  (16-wrap -> 128-wrap expand);
    # stacked [32, .]: rows 0:16 scaled by 128 (C chain), rows 16:32 raw (D)
    ew = np.zeros((16, 8 * P), np.float32)
    for i in range(16):
        for g in range(8):
            ew[i, g * P + g * 16 + i] = 1.0
    consts["ewrap"] = np.vstack([128.0 * ew, ew]).astype(np.float32)
    return consts


def make_in_maps(x, W1, b1, W2, b2, wr, br):
    consts = make_consts()
    x = np.ascontiguousarray(np.asarray(x, np.float32))
    in_maps = []
    for c in range(NCORES):
        b, h = divmod(c, 2)
        m = {
            "x_row": x[b],
            "w1": np.asarray(W1, np.float32),
            "w2": np.asarray(W2, np.float32),
            "wr": np.asarray(wr, np.float32).reshape(1, D),
            "b1t": np.ascontiguousarray(np.asarray(b1, np.float32).reshape(NM, P).T),
            "b2": np.asarray(b2, np.float32).reshape(1, D),
            "hbase": np.array([[h * SEL]], np.float32),
        }
        m.update(consts)
        in_maps.append(m)
    return in_maps


_NC_CACHE = None


def _get_program():
    global _NC_CACHE
    if _NC_CACHE is None:
        _NC_CACHE = build_program()
    return _NC_CACHE


def kernel(x, W1, b1, W2, b2, wr, br):
    from concourse.bass_utils import run_bass_kernel_spmd

    nc = _get_program()
    in_maps = make_in_maps(x, W1, b1, W2, b2, wr, br)
    res = run_bass_kernel_spmd(nc, in_maps, list(range(NCORES))).results
    out = np.stack(
        [res[2 * b]["out_row"] + res[2 * b + 1]["out_row"] for b in range(B)]
    )
    return out.astype(np.float32)


# revision 67
# speedup vs baseline: 2.6090x; 1.0029x over previous
"""MoD (mixture-of-depths) MLP wrapper kernel for Trainium2, 8 NeuronCores.

Sharding: core c handles batch row b = c//2 and the half of that row's
top-K tokens with global selection ranks in [h*1024, (h+1)*1024), h = c%2.
Each core computes the full row's router scores + top-K threshold locally
(no collectives), gathers exactly 1024 token rows by rank via indirect DMA,
runs the FFN in bf16 (fp32 accumulation), and scatters results back into the
pre-zeroed per-core output buffer with dma_scatter_add.  Host sums the two
buffers of each row.

Schedule: x-tile loads own the DMA engines first (weight loads are ordered
behind them); radix pass 1 folds into the score loop against a constant
threshold grid; passes 2-4 run as Sign-activation counts over a
DMA-broadcast score replica; rank compaction is a digit-decomposed one-hot
bf16 matmul whose stacked constants emit both the int32 gather and int16
scatter index layouts; gathered tokens are transposed by the DMA xbar
(dma_start_transpose); and the output scatter is dma_scatter_add (per-index
descriptors) overlapped with the tail of the second matmul.
"""

import sys

sys.path.insert(0, "/opt/trn_rl_repo")

from contextlib import ExitStack

import numpy as np

from concourse import bass, bass_isa, mybir
from concourse import bacc
import concourse.tile as tile
from concourse.bass import IndirectOffsetOnAxis

B, L, D = 4, 4096, 1024
DFF = 4 * D
K = L // 2              # 2048 selected tokens per row
NCORES = 8
P = 128
NT = L // P             # 32 token tiles per row
SEL = K // 2            # 1024 selected tokens per core
NSJ = SEL // P          # 8 selected-token blocks
ND = D // P             # 8 d chunks
NM = DFF // P           # 32 dff tiles
NKGRP = 4               # w2 k-chunks per streamed tile
RADIX_PASSES = 4

F32 = mybir.dt.float32
BF16 = mybir.dt.bfloat16
I32 = mybir.dt.int32
I16 = mybir.dt.int16
Alu = mybir.AluOpType
Act = mybir.ActivationFunctionType
Red = bass_isa.ReduceOp


def build_program():
    nc = bacc.Bacc(
        "TRN2",
        target_bir_lowering=False,
        debug=False,
        enable_asserts=False,
        num_devices=NCORES,
    )

    x_row = nc.dram_tensor("x_row", [L, D], F32, kind="ExternalInput").ap()
    w1 = nc.dram_tensor("w1", [D, DFF], F32, kind="ExternalInput").ap()
    w2 = nc.dram_tensor("w2", [DFF, D], F32, kind="ExternalInput").ap()
    wr = nc.dram_tensor("wr", [1, D], F32, kind="ExternalInput").ap()
    b1t = nc.dram_tensor("b1t", [P, NM], F32, kind="ExternalInput").ap()
    b2 = nc.dram_tensor("b2", [1, D], F32, kind="ExternalInput").ap()
    hbase = nc.dram_tensor("hbase", [1, 1], F32, kind="ExternalInput").ap()
    identb = nc.dram_tensor("identb", [P, P], BF16, kind="ExternalInput").ap()
    ltri = nc.dram_tensor("ltri128", [P, P], F32, kind="ExternalInput").ap()
    slt32 = nc.dram_tensor("slt32", [NT, NT], F32, kind="ExternalInput").ap()
    id32 = nc.dram_tensor("id32", [NT, NT], F32, kind="ExternalInput").ap()
    ones_1x128 = nc.dram_tensor("ones_1x128", [1, P], F32, kind="ExternalInput").ap()
    ones_1x128b = nc.dram_tensor("ones_1x128b", [1, P], BF16, kind="ExternalInput").ap()
    ones_128x1 = nc.dram_tensor("ones_128x1", [P, 1], F32, kind="ExternalInput").ap()
    ones128 = nc.dram_tensor("ones128", [P, P], F32, kind="ExternalInput").ap()
    ones_32x128 = nc.dram_tensor("ones_32x128", [NT, P], F32, kind="ExternalInput").ap()
    rep16 = nc.dram_tensor("rep16", [32, P], F32, kind="ExternalInput").ap()
    ewrap = nc.dram_tensor("ewrap", [32, 8 * P], F32, kind="ExternalInput").ap()

    out_row = nc.dram_tensor("out_row", [L, D], F32, kind="ExternalOutput").ap()

    GRPS = ((0, 20), (20, 30), (30, 31), (31, 32))
    scores_dg = [nc.dram_tensor(f"scores_dg{i}", [hi - lo, P], F32).ap()
                 for i, (lo, hi) in enumerate(GRPS)]

    with tile.TileContext(nc) as tc, ExitStack() as S0:
        const = S0.enter_context(tc.tile_pool(name="const", bufs=1))
        # pool stack (LIFO): const | ht | w1 | dig | ...phases
        ht_ctx = tc.tile_pool(name="ht", bufs=1)
        ht_pool = ht_ctx.__enter__()
        ht = ht_pool.tile([P, NM, SEL], BF16)
        w1_ctx = tc.tile_pool(name="w1bf", bufs=1)
        w1_pool = w1_ctx.__enter__()

        def cload(pool, ap, shape, dtype=F32, name=None):
            t = pool.tile(shape, dtype, name=name)
            nc.sync.dma_start(out=t[:], in_=ap)
            return t

        # ---- SP-queue order: wr, o1, oc, hbase FIRST (phase A needs them) ---
        wr_sb = cload(const, wr, [1, D], name="c_wr")
        o1x128_sb = cload(const, ones_1x128, [1, P], name="c_o1")
        o128x1_sb = cload(const, ones_128x1, [P, 1], name="c_oc")
        ones128_sb = cload(const, ones128, [P, P], name="c_o128")
        hb_sb = cload(const, hbase, [1, 1], name="c_hb")

        # w1 tiles exist from the start (loads are issued after the radix)
        w1bf = [w1_pool.tile([P, DFF], BF16, name=f"w1bf_{kd}")
                for kd in range(ND)]

        # ---- Pool-queue iotas (independent of SP queue) ---------------------
        # big digit-decomposition iota tables live only through phase E
        dig_ctx = tc.tile_pool(name="dig", bufs=1)
        dig = dig_ctx.__enter__()

        iota_i = const.tile([P, 1], I32)
        nc.gpsimd.iota(iota_i[:], pattern=[[1, 1]], base=0, channel_multiplier=1)
        tokid = const.tile([P, NT], I32)
        nc.gpsimd.iota(tokid[:], pattern=[[P, NT]], base=0, channel_multiplier=1)
        iC_i = const.tile([P, NT], I32)
        nc.gpsimd.iota(iC_i[:], pattern=[[1, NT]], base=0, channel_multiplier=0)
        iQ_i = const.tile([P, 128], I32)
        nc.gpsimd.iota(iQ_i[:], pattern=[[1, 128]], base=0, channel_multiplier=0)
        iK64_i = dig.tile([P, NT, 64], I16)
        nc.gpsimd.iota(iK64_i[:], pattern=[[0, NT], [1, 64]], base=0,
                       channel_multiplier=0)
        iJ16_i = dig.tile([P, NT, 16], I16)
        nc.gpsimd.iota(iJ16_i[:], pattern=[[0, NT], [1, 16]], base=0,
                       channel_multiplier=0)
        i7_i = const.tile([P, 7], I32)
        nc.gpsimd.iota(i7_i[:], pattern=[[1, 7]], base=1, channel_multiplier=0)

        iota_f = const.tile([P, 1], F32)
        nc.vector.tensor_copy(out=iota_f[:], in_=iota_i[:])
        tokidf = const.tile([P, NT], F32)
        nc.vector.tensor_copy(out=tokidf[:], in_=tokid[:])
        cvalf = const.tile([P, NT], F32)
        nc.vector.tensor_copy(out=cvalf[:], in_=iC_i[:])
        iK64b = dig.tile([P, NT, 64], BF16)
        nc.vector.tensor_copy(out=iK64b[:], in_=iK64_i[:])
        iJ16b = dig.tile([P, NT, 16], BF16)
        nc.vector.tensor_copy(out=iJ16b[:], in_=iJ16_i[:])
        iotab = const.tile([P, 1], BF16)
        nc.vector.tensor_copy(out=iotab[:], in_=iota_i[:])
        cvalb = const.tile([P, NT], BF16)
        nc.vector.tensor_copy(out=cvalb[:], in_=iC_i[:])
        i7f = const.tile([P, 7], F32)
        nc.vector.tensor_copy(out=i7f[:], in_=i7_i[:])
        thr128 = const.tile([P, 7], F32)
        nc.vector.tensor_scalar(out=thr128[:], in0=i7f[:], scalar1=128.0,
                                scalar2=None, op0=Alu.mult)
        thr16 = const.tile([P, 7], F32)
        nc.vector.tensor_scalar(out=thr16[:], in0=i7f[:], scalar1=16.0,
                                scalar2=None, op0=Alu.mult)
        # radix pass-1 threshold grid (build-time constants: lo=-16, w=0.25)
        iQf = const.tile([P, 128], F32)
        nc.vector.tensor_copy(out=iQf[:], in_=iQ_i[:])
        thr1row = const.tile([P, 128], F32)
        nc.vector.tensor_scalar(out=thr1row[:], in0=iQf[:], scalar1=32.0 / P,
                                scalar2=-16.0, op0=Alu.mult, op1=Alu.add)
        # negated per-pass threshold offsets for radix passes 2..4
        W1P = 32.0 / P
        nthrbs = []
        for p_ in range(1, RADIX_PASSES):
            w_p = W1P / (P ** p_)
            t_ = const.tile([P, 1], F32, name=f"nthrb{p_}")
            nc.vector.tensor_scalar(out=t_[:], in0=iota_f[:], scalar1=-w_p,
                                    scalar2=None, op0=Alu.mult)
            nthrbs.append((w_p, t_))
        hb_col = const.tile([P, 1], F32)
        nc.gpsimd.partition_broadcast(hb_col[:], hb_sb[:])

        scores_sb = const.tile([P, NT], F32)
        selidx_sb = const.tile([P, NSJ], I32)
        idx16_sb = const.tile([P, SEL // 16], I16)

        misc_psum_ctx = tc.tile_pool(name="misc_psum", bufs=2, space="PSUM")
        misc_psum = misc_psum_ctx.__enter__()

        # ---- phase A: router scores (fp32, exact; router bias dropped — it
        # shifts every score equally so the top-K set is unchanged).  The
        # first radix pass uses a build-time-constant threshold grid, so its
        # per-tile compare + count-matmul accumulation is folded in here. -----
        c1_psum_ctx = tc.tile_pool(name="c1_psum", bufs=1, space="PSUM")
        c1_psum = c1_psum_ctx.__enter__()
        cnt1_ps = c1_psum.tile([P, 128], F32, name="cnt1")
        nlo = const.tile([P, 1], F32, name="nlo")
        with ExitStack() as SA:
            apool = SA.enter_context(tc.tile_pool(name="apool", bufs=1))
            xs_pool = SA.enter_context(tc.tile_pool(name="xs", bufs=6))
            junk_pool = SA.enter_context(tc.tile_pool(name="junk", bufs=2))
            cmp_pool = SA.enter_context(tc.tile_pool(name="cmp", bufs=3))

            wrb = apool.tile([P, D], F32)
            for n in range(D // 512):
                pt = misc_psum.tile([P, 512], F32, name="mp")
                nc.tensor.matmul(out=pt[:], lhsT=o1x128_sb[:],
                                 rhs=wr_sb[:, n * 512:(n + 1) * 512],
                                 start=True, stop=True)
                nc.vector.tensor_copy(out=wrb[:, n * 512:(n + 1) * 512], in_=pt[:])

            x_last = None
            for t in range(NT):
                x_t = xs_pool.tile([P, D], F32)
                nc.sync.dma_start(out=x_t[:], in_=x_row[t * P:(t + 1) * P, :])
                x_last = x_t
                if t == 26:
                    nc.sync.dma_start(
                        out=scores_dg[0].rearrange("c p -> p c"),
                        in_=scores_sb[:, 0:20])
                prod = junk_pool.tile([P, D], F32, name="prod")
                nc.vector.tensor_tensor(out=prod[:], in0=x_t[:], in1=wrb[:],
                                        op=Alu.mult)
                sink = junk_pool.tile([P, D], BF16, name="sink")
                nc.scalar.activation(out=sink[:], in_=prod[:], func=Act.Identity,
                                     bias=0.0, scale=1.0,
                                     accum_out=scores_sb[:, t:t + 1])
                cmp_t = cmp_pool.tile([P, 128], F32, name="cmp")
                nc.vector.tensor_tensor(
                    out=cmp_t[:],
                    in0=scores_sb[:, t:t + 1].to_broadcast([P, 128]),
                    in1=thr1row[:], op=Alu.is_ge)
                nc.tensor.matmul(out=cnt1_ps[:], lhsT=ones128_sb[:], rhs=cmp_t[:],
                                 start=(t == 0), stop=(t == NT - 1),
                                 skip_group_check=True)


            # pass-1 finalize on every partition (count matmul used an
            # all-ones lhsT, so each partition holds the full count row):
            # nlo = -(lo1) = 16 - (sum(cnt>=K) - 1)*0.25
            selr = apool.tile([P, 128], F32, name="selr")
            nc.vector.tensor_scalar(out=selr[:], in0=cnt1_ps[:],
                                    scalar1=float(K), scalar2=None,
                                    op0=Alu.is_ge)
            s1 = apool.tile([P, 1], F32, name="s1")
            nc.vector.tensor_reduce(out=s1[:], in_=selr[:],
                                    axis=mybir.AxisListType.X, op=Alu.add)
            q1 = apool.tile([P, 1], F32, name="q1")
            nc.vector.tensor_scalar(out=q1[:], in0=s1[:], scalar1=-1.0,
                                    scalar2=-W1P, op0=Alu.add, op1=Alu.mult)
            nc.vector.tensor_scalar(out=nlo[:], in0=q1[:], scalar1=16.0,
                                    scalar2=None, op0=Alu.add)
        c1_psum_ctx.__exit__(None, None, None)

        offf_c = const.tile([P, NT], F32)
        maskf_c = const.tile([P, NT], F32)

        # ---- phases B+C+D: replicate scores, radix threshold, rank ----------
        with ExitStack() as SC:
            radix = SC.enter_context(tc.tile_pool(name="radix", bufs=2))
            rep_pool = SC.enter_context(tc.tile_pool(name="rep", bufs=1))

            # broadcast-read the spilled scores, one DMA per 1024-token group,
            # FIRST on the in-order SP queue right after the x loads (the
            # remaining const loads queue behind, they aren't needed till later)
            scores_rep = rep_pool.tile([P, L], F32)
            for gi, (glo, ghi) in enumerate(GRPS):
                n_ = (ghi - glo) * P
                if gi > 0:   # g0 was spilled inside the x stream
                    nc.sync.dma_start(
                        out=scores_dg[gi].rearrange("c p -> p c"),
                        in_=scores_sb[:, glo:ghi])
                nc.sync.dma_start(
                    out=scores_rep[:, glo * P:ghi * P],
                    in_=scores_dg[gi].rearrange("c p -> () (c p)")
                    .to_broadcast([P, n_]))

            # gate the w1 cast-loads behind the score broadcast so their DMAs
            # cannot delay it (WAW edge: the w1 DMA overwrites the gate byte)
            for kd in range(ND):
                nc.vector.tensor_copy(out=w1bf[kd][0:1, 0:1],
                                      in_=scores_rep[0:1, kd:kd + 1])

            # ---- remaining small consts on the SP queue ---------------------
            b1t_sb = cload(const, b1t, [P, NM], name="c_b1t")
            identb_sb = cload(const, identb, [P, P], BF16, name="c_idb")
            ltri_sb = cload(const, ltri, [P, P], name="c_lt")
            slt32_sb = cload(const, slt32, [NT, NT], name="c_sl")
            id32_sb = cload(const, id32, [NT, NT], name="c_id32")
            o1x128b_sb = cload(const, ones_1x128b, [1, P], BF16, name="c_o1b")
            o32x128_sb = cload(const, ones_32x128, [NT, P], name="c_o32")
            rep16_sb = cload(const, rep16, [32, P], name="c_rep16")
            ewrap_sb = cload(const, ewrap, [32, 8 * P], name="c_ew")
            b2bf_sb = const.tile([1, D], BF16)
            nc.gpsimd.dma_start(out=b2bf_sb[:], in_=b2)  # cast f32 -> bf16

            sjunk = rep_pool.tile([P, L], BF16, name="sjunk")
            NSPL = 2560          # tokens covered by broadcast group 0
            for pi, (w_p, nthrb_p) in enumerate(nthrbs):
                nthr = radix.tile([P, 1], F32, name="nthr")
                nc.vector.tensor_tensor(out=nthr[:], in0=nlo[:], in1=nthrb_p[:],
                                        op=Alu.add)
                if pi == 0:
                    # group-0 scores arrive first; count them while the tail
                    # broadcasts finish, then add the remainder
                    sgA = radix.tile([P, 1], F32, name="sgA")
                    nc.scalar.activation(out=sjunk[:, :NSPL],
                                         in_=scores_rep[:, :NSPL],
                                         func=Act.Sign, bias=nthr[:, :1],
                                         scale=1.0, accum_out=sgA[:])
                    sgB = radix.tile([P, 1], F32, name="sgB")
                    nc.scalar.activation(out=sjunk[:, NSPL:],
                                         in_=scores_rep[:, NSPL:],
                                         func=Act.Sign, bias=nthr[:, :1],
                                         scale=1.0, accum_out=sgB[:])
                    sgn = radix.tile([P, 1], F32, name="sgn")
                    nc.vector.tensor_tensor(out=sgn[:], in0=sgA[:], in1=sgB[:],
                                            op=Alu.add)
                else:
                    sgn = radix.tile([P, 1], F32, name="sgn")
                    nc.scalar.activation(out=sjunk[:], in_=scores_rep[:],
                                         func=Act.Sign, bias=nthr[:, :1],
                                         scale=1.0, accum_out=sgn[:])
                sel = radix.tile([P, 1], F32, name="sel")
                nc.vector.tensor_scalar(out=sel[:], in0=sgn[:], scalar1=0.0,
                                        scalar2=None, op0=Alu.is_ge)
                s_all = radix.tile([P, 1], F32, name="s_all")
                nc.gpsimd.partition_all_reduce(s_all[:], sel[:], channels=P,
                                               reduce_op=Red.add)
                nd = radix.tile([P, 1], F32, name="nd")
                nc.vector.tensor_scalar(out=nd[:], in0=s_all[:], scalar1=-1.0,
                                        scalar2=-w_p, op0=Alu.add, op1=Alu.mult)
                nlo2 = radix.tile([P, 1], F32, name="nlo2")
                nc.vector.tensor_tensor(out=nlo2[:], in0=nlo[:], in1=nd[:],
                                        op=Alu.add)
                nlo = nlo2

            # ---- mask + global rank (exclusive prefix of mask) --------------
            maskf = radix.tile([P, NT], F32, name="maskf")
            nc.vector.tensor_scalar(out=maskf[:], in0=scores_sb[:],
                                    scalar1=nlo[:, :1], scalar2=0.0,
                                    op0=Alu.add, op1=Alu.is_ge)
            colsum_p = misc_psum.tile([NT, 1], F32, name="mp")
            nc.tensor.matmul(out=colsum_p[:], lhsT=maskf[:], rhs=o128x1_sb[:],
                             start=True, stop=True)
            colsum = radix.tile([NT, 1], F32, name="colsum")
            nc.vector.tensor_copy(out=colsum[:], in_=colsum_p[:])
            excl_p = misc_psum.tile([NT, 1], F32, name="mp")
            nc.tensor.matmul(out=excl_p[:], lhsT=slt32_sb[:], rhs=colsum[:],
                             start=True, stop=True)
            excl = radix.tile([NT, 1], F32, name="excl")
            nc.vector.tensor_copy(out=excl[:], in_=excl_p[:])
            diag = radix.tile([NT, NT], F32, name="diag")
            nc.vector.tensor_tensor(out=diag[:], in0=id32_sb[:],
                                    in1=excl[:, :1].to_broadcast([NT, NT]),
                                    op=Alu.mult)
            rank_p = misc_psum.tile([P, NT], F32, name="mp")
            nc.tensor.matmul(out=rank_p[:], lhsT=ltri_sb[:], rhs=maskf[:],
                             start=True, stop=False, skip_group_check=True)
            nc.tensor.matmul(out=rank_p[:], lhsT=o32x128_sb[:], rhs=diag[:],
                             start=False, stop=True, skip_group_check=True)
            nc.vector.tensor_scalar(out=offf_c[:], in0=rank_p[:],
                                    scalar1=hb_col[:, :1], scalar2=None,
                                    op0=Alu.subtract)
            nc.vector.tensor_copy(out=maskf_c[:], in_=maskf[:])

        misc_psum_ctx.__exit__(None, None, None)

        # ---- w1 cast-loads on the Pool queue.  Positioned after the radix
        # all_reduces so the in-order queue starts them only ~70us in, after
        # the x-tile DMAs have drained (they'd otherwise steal DMA bandwidth
        # from the critical-path score loads). ---------------------------------
        w1bf = []
        for kd in range(ND):
            t_ = w1_pool.tile([P, DFF], BF16, name=f"w1bf_{kd}")
            nc.gpsimd.dma_start(out=t_[:], in_=w1[kd * P:(kd + 1) * P, :])
            w1bf.append(t_)

        # ---- phase E: digit split + one-hot compaction matmuls --------------
        # off in [0, SEL) for in-window selected tokens; any other off value
        # (negative rank-window miss, >=SEL, or collision of an unselected
        # token) produces no match in the lo-digit equality below, and
        # unselected tokens are additionally zeroed via tokid*mask weights.
        with ExitStack() as SE:
            ep = SE.enter_context(tc.tile_pool(name="epool", bufs=1))
            e_psum = SE.enter_context(tc.tile_pool(name="e_psum", bufs=2,
                                                   space="PSUM"))
            off = offf_c
            eq7a = ep.tile([P, NT, 7], F32, name="eq7a")
            nc.vector.tensor_tensor(
                out=eq7a[:], in0=off[:, :, None].to_broadcast([P, NT, 7]),
                in1=thr128[:, None, :].to_broadcast([P, NT, 7]), op=Alu.is_ge)
            hi128 = ep.tile([P, NT], F32, name="hi128")
            nc.vector.tensor_reduce(out=hi128[:], in_=eq7a[:],
                                    axis=mybir.AxisListType.X, op=Alu.add)
            hm = ep.tile([P, NT], F32, name="hm")
            nc.vector.tensor_scalar(out=hm[:], in0=hi128[:], scalar1=-128.0,
                                    scalar2=None, op0=Alu.mult)
            lo128 = ep.tile([P, NT], F32, name="lo128")
            nc.vector.tensor_tensor(out=lo128[:], in0=off[:], in1=hm[:],
                                    op=Alu.add)
            eq7b = ep.tile([P, NT, 7], F32, name="eq7b")
            nc.vector.tensor_tensor(
                out=eq7b[:], in0=lo128[:, :, None].to_broadcast([P, NT, 7]),
                in1=thr16[:, None, :].to_broadcast([P, NT, 7]), op=Alu.is_ge)
            mid = ep.tile([P, NT], F32, name="mid")
            nc.vector.tensor_reduce(out=mid[:], in_=eq7b[:],
                                    axis=mybir.AxisListType.X, op=Alu.add)
            hm2 = ep.tile([P, NT], F32, name="hm2")
            nc.vector.tensor_scalar(out=hm2[:], in0=mid[:], scalar1=-16.0,
                                    scalar2=None, op0=Alu.mult)
            lo16b = ep.tile([P, NT], BF16, name="lo16b")
            nc.vector.tensor_tensor(out=lo16b[:], in0=lo128[:], in1=hm2[:],
                                    op=Alu.add)
            h8 = ep.tile([P, NT], F32, name="h8")
            nc.vector.tensor_scalar(out=h8[:], in0=hi128[:], scalar1=8.0,
                                    scalar2=None, op0=Alu.mult)
            hi16b = ep.tile([P, NT], BF16, name="hi16b")
            nc.vector.tensor_tensor(out=hi16b[:], in0=h8[:], in1=mid[:],
                                    op=Alu.add)
            # token id = c*128 + p; weight the SMALL equality factors by
            # c*mask (chain C, lhsT cols 0:16) and p*mask (chain D, cols
            # 16:32), then sel16 = 128*C + D.  All factors are small exact
            # integers, so the chain runs in bf16 (1 cycle/row matmuls).
            maskb = ep.tile([P, NT], BF16, name="maskb")
            nc.vector.tensor_copy(out=maskb[:], in_=maskf_c[:])
            cwm = ep.tile([P, NT], BF16, name="cwm")
            nc.vector.tensor_tensor(out=cwm[:], in0=cvalb[:], in1=maskb[:],
                                    op=Alu.mult)
            pwm = ep.tile([P, NT], BF16, name="pwm")
            nc.vector.tensor_tensor(out=pwm[:], in0=maskb[:],
                                    in1=iotab[:, :1].to_broadcast([P, NT]),
                                    op=Alu.mult)

            # build the equality factors in half-tile chunks so the first 16
            # compaction matmuls overlap construction of the second half
            eq16 = ep.tile([P, NT, 16], BF16, name="eq16")
            eqcp = ep.tile([P, NT, 32], BF16, name="eqcp")
            eq64 = ep.tile([P, NT, 64], BF16, name="eq64")
            pCD = e_psum.tile([32, 64], F32, name="pCD")
            H = NT // 2
            for h0 in (0, H):
                sl = slice(h0, h0 + H)
                nc.vector.tensor_tensor(
                    out=eq16[:, sl, :], in0=iJ16b[:, sl, :],
                    in1=lo16b[:, sl, None].to_broadcast([P, H, 16]),
                    op=Alu.is_equal)
                nc.vector.tensor_tensor(
                    out=eqcp[:, sl, 0:16], in0=eq16[:, sl, :],
                    in1=cwm[:, sl, None].to_broadcast([P, H, 16]), op=Alu.mult)
                nc.vector.tensor_tensor(
                    out=eqcp[:, sl, 16:32], in0=eq16[:, sl, :],
                    in1=pwm[:, sl, None].to_broadcast([P, H, 16]), op=Alu.mult)
                nc.vector.tensor_tensor(
                    out=eq64[:, sl, :], in0=iK64b[:, sl, :],
                    in1=hi16b[:, sl, None].to_broadcast([P, H, 64]),
                    op=Alu.is_equal)
                for c in range(h0, h0 + H):
                    nc.tensor.matmul(out=pCD[:], lhsT=eqcp[:, c, :],
                                     rhs=eq64[:, c, :], start=(c == 0),
                                     stop=(c == NT - 1), skip_group_check=True)

            sCD = ep.tile([32, 64], F32, name="sCD")
            nc.vector.tensor_copy(out=sCD[:], in_=pCD[:])

            # scatter index layout [128, 64] (16-wrap replicated to 128);
            # lhsT folds the 128*C + D combine (rows 0:16 scaled by 128)
            rep_ps = e_psum.tile([P, 64], F32, name="rep_ps")
            nc.tensor.matmul(out=rep_ps[:], lhsT=rep16_sb[:], rhs=sCD[:],
                             start=True, stop=True)
            nc.vector.tensor_copy(out=idx16_sb[:], in_=rep_ps[:])  # f32->i16

            # gather index layout [128, 8]: selidx[p, k] = sel16[p%16, 8k+p//16]
            selps = e_psum.tile([P, NSJ], F32, name="selps")
            for g in range(8):
                nc.tensor.matmul(out=selps[:], lhsT=ewrap_sb[:, g * P:(g + 1) * P],
                                 rhs=sCD[:, g::8], start=(g == 0),
                                 stop=(g == 7), skip_group_check=True)
            nc.vector.tensor_copy(out=selidx_sb[:], in_=selps[:])  # f32->i32

        dig_ctx.__exit__(None, None, None)

        # ---- gather + transpose + MLP ---------------------------------------
        if True:
            with ExitStack() as SB:
                xt_pool = SB.enter_context(tc.tile_pool(name="xt", bufs=1))
                xsel_pool = SB.enter_context(tc.tile_pool(name="xsel", bufs=5))
                mm1_psum = SB.enter_context(tc.tile_pool(name="mm1_psum", bufs=6,
                                                         space="PSUM"))

                # xt3[p, kd, t] = x_sel[t, kd*128+p], built by the DMA-engine
                # xbar transpose (one per gathered 128-token chunk)
                xt3 = xt_pool.tile([P, ND, SEL], BF16)
                for j in range(NSJ):
                    xs = xsel_pool.tile([P, D], BF16, name="xsel")
                    nc.gpsimd.indirect_dma_start(
                        out=xs[:], out_offset=None, in_=x_row,
                        in_offset=IndirectOffsetOnAxis(ap=selidx_sb[:, j:j + 1],
                                                       axis=0))
                    nc.scalar.dma_start_transpose(
                        out=xt3[:, :, j * P:(j + 1) * P], in_=xs[:])

                # ---- mm1: ht[m, sel] = gelu(w1^T x_sel^T + b1).  The first
                # four token blocks are 128 wide so the PE starts the moment
                # each transpose lands instead of waiting for four of them;
                # the second half runs as one 512-wide block.
                for t0, tw in [(0, P), (P, P), (2 * P, P), (3 * P, P),
                               (512, 512)]:
                    for m in range(NM):
                        ph = mm1_psum.tile([P, tw], F32, name="ph")
                        for kd in range(ND):
                            nc.tensor.matmul(
                                out=ph[:],
                                lhsT=w1bf[kd][:, m * P:(m + 1) * P],
                                rhs=xt3[:, kd, t0:t0 + tw],
                                start=(kd == 0), stop=(kd == ND - 1),
                            )
                        nc.scalar.activation(
                            out=ht[:, m, t0:t0 + tw], in_=ph[:],
                            func=Act.Gelu_apprx_tanh, bias=b1t_sb[:, m:m + 1],
                            scale=1.0,
                        )

            w1_ctx.__exit__(None, None, None)  # free w1 region for w2 stream

            # ---- mm2: y[sel, D] = ht^T @ w2 + b2, then scatter-add ----------
            with ExitStack() as SY:
                y_pool = SY.enter_context(tc.tile_pool(name="y", bufs=1))
                w2_pool = SY.enter_context(tc.tile_pool(name="w2s", bufs=16))
                mm2_psum = SY.enter_context(tc.tile_pool(name="mm2_psum", bufs=8,
                                                         space="PSUM"))
                # d-half 0: kg-major accumulation (w2 tiles stream in, all 8
                # token-block psums accumulate together)
                n = 0
                y_0 = y_pool.tile([P, NSJ, 512], F32, name="y0")
                pys = [mm2_psum.tile([P, 512], F32, name="py")
                       for _ in range(NSJ)]
                w2n1 = []   # d-half-1 tiles retained for the s-major pass
                for s in range(NSJ):
                    nc.tensor.matmul(
                        out=pys[s][:], lhsT=o1x128b_sb[:],
                        rhs=b2bf_sb[:, :512],
                        start=True, stop=False, skip_group_check=True,
                    )
                for kg in range(NM // NKGRP):
                    w2t = w2_pool.tile([P, NKGRP, 512], BF16, name="w2t")
                    if kg == 0:
                        # WAW gate: keep the w2 stream off the DMA engines
                        # until the gather/transpose pipeline has fed mm1
                        nc.vector.tensor_copy(out=w2t[0:1, 0, 0:1],
                                              in_=ht[0:1, 0, 0:1])
                    src = w2[:, :512].rearrange(
                        "(g p) f -> p g f", p=P)[:, kg * NKGRP:(kg + 1) * NKGRP, :]
                    nc.gpsimd.dma_start(out=w2t[:], in_=src)
                    for ki in range(NKGRP):
                        kk = kg * NKGRP + ki
                        for s in range(NSJ):
                            nc.tensor.matmul(
                                out=pys[s][:],
                                lhsT=ht[:, kk, s * P:(s + 1) * P],
                                rhs=w2t[:, ki, :],
                                start=False, stop=(kk == NM - 1),
                                skip_group_check=True,
                            )
                # prefetch d-half-1 w2 tiles while the n=0 tail accumulates
                for kg in range(NM // NKGRP):
                    w2t = w2_pool.tile([P, NKGRP, 512], BF16, name="w2t")
                    src = w2[:, 512:].rearrange(
                        "(g p) f -> p g f", p=P)[:, kg * NKGRP:(kg + 1) * NKGRP, :]
                    nc.gpsimd.dma_start(out=w2t[:], in_=src)
                    w2n1.append(w2t)
                for s in range(NSJ):
                    nc.scalar.activation(out=y_0[:, s, :], in_=pys[s][:],
                                         func=Act.Copy, bias=0.0, scale=1.0)
                    if s % 4 == 3:
                        h = s // 4
                        nc.gpsimd.dma_scatter_add(
                            out_row[:, :512],
                            y_0[:, h * 4:(h + 1) * 4, :],
                            idx16_sb[:, h * 32:(h + 1) * 32],
                            SEL // 2,
                            SEL // 2,
                            512,
                            elem_step=D,
                        )

                # d-half 1: s-major (each token block finishes early and its
                # rows scatter while the next block accumulates)
                y_1 = y_pool.tile([P, NSJ, 512], F32, name="y1")
                for s in range(NSJ):
                    py = mm2_psum.tile([P, 512], F32, name="py")
                    nc.tensor.matmul(
                        out=py[:], lhsT=o1x128b_sb[:], rhs=b2bf_sb[:, 512:],
                        start=True, stop=False, skip_group_check=True,
                    )
                    for kk in range(NM):
                        nc.tensor.matmul(
                            out=py[:],
                            lhsT=ht[:, kk, s * P:(s + 1) * P],
                            rhs=w2n1[kk // NKGRP][:, kk % NKGRP, :],
                            start=False, stop=(kk == NM - 1),
                            skip_group_check=True,
                        )
                    nc.scalar.activation(out=y_1[:, s, :], in_=py[:],
                                         func=Act.Copy, bias=0.0, scale=1.0)
                    nc.gpsimd.dma_scatter_add(
                        out_row[:, 512:],
                        y_1[:, s:s + 1, :],
                        idx16_sb[:, s * 8:(s + 1) * 8],
                        P,
                        P,
                        512,
                        elem_step=D,
                    )

        ht_ctx.__exit__(None, None, None)

    nc.compile()
    return nc


def make_consts():
    q = np.arange(P)
    import ml_dtypes
    consts = {
        "identb": np.eye(P, dtype=ml_dtypes.bfloat16),
        "ltri128": (q[:, None] < q[None, :]).astype(np.float32),  # [q, p] = q < p
        "slt32": (np.arange(NT)[:, None] < np.arange(NT)[None, :]).astype(np.float32),
        "id32": np.eye(NT, dtype=np.float32),
        "ones_1x128": np.ones((1, P), np.float32),
        "ones_1x128b": np.ones((1, P), ml_dtypes.bfloat16),
        "ones_128x1": np.ones((P, 1), np.float32),
        "ones128": np.ones((P, P), np.float32),
        "ones_32x128": np.ones((NT, P), np.float32),
        "rep16": np.vstack([
            128.0 * (np.arange(16)[:, None] == (np.arange(P)[None, :] % 16)),
            1.0 * (np.arange(16)[:, None] == (np.arange(P)[None, :] % 16)),
        ]).astype(np.float32),
    }
    # ewrap[i, g*128 + p] = 1 iff p == g*16 + i  (16-wrap -> 128-wrap expand);
    # stacked [32, .]: rows 0:16 scaled by 128 (C chain), rows 16:32 raw (D)
    ew = np.zeros((16, 8 * P), np.float32)
    for i in range(16):
        for g in range(8):
            ew[i, g * P + g * 16 + i] = 1.0
    consts["ewrap"] = np.vstack([128.0 * ew, ew]).astype(np.float32)
    return consts


def make_in_maps(x, W1, b1, W2, b2, wr, br):
    consts = make_consts()
    x = np.ascontiguousarray(np.asarray(x, np.float32))
    in_maps = []
    for c in range(NCORES):
        b, h = divmod(c, 2)
        m = {
            "x_row": x[b],
            "w1": np.asarray(W1, np.float32),
            "w2": np.asarray(W2, np.float32),
            "wr": np.asarray(wr, np.float32).reshape(1, D),
            "b1t": np.ascontiguousarray(np.asarray(b1, np.float32).reshape(NM, P).T),
            "b2": np.asarray(b2, np.float32).reshape(1, D),
            "hbase": np.array([[h * SEL]], np.float32),
        }
        m.update(consts)
        in_maps.append(m)
    return in_maps


_NC_CACHE = None


def _get_program():
    global _NC_CACHE
    if _NC_CACHE is None:
        _NC_CACHE = build_program()
    return _NC_CACHE


def kernel(x, W1, b1, W2, b2, wr, br):
    from concourse.bass_utils import run_bass_kernel_spmd

    nc = _get_program()
    in_maps = make_in_maps(x, W1, b1, W2, b2, wr, br)
    res = run_bass_kernel_spmd(nc, in_maps, list(range(NCORES))).results
    out = np.stack(
        [res[2 * b]["out_row"] + res[2 * b + 1]["out_row"] for b in range(B)]
    )
    return out.astype(np.float32)


# revision 69
# speedup vs baseline: 2.6155x; 1.0025x over previous
"""MoD (mixture-of-depths) MLP wrapper kernel for Trainium2, 8 NeuronCores.

Sharding: core c handles batch row b = c//2 and the half of that row's
top-K tokens with global selection ranks in [h*1024, (h+1)*1024), h = c%2.
Each core computes the full row's router scores + top-K threshold locally
(no collectives), gathers exactly 1024 token rows by rank via indirect DMA,
runs the FFN in bf16 (fp32 accumulation), and scatters results back into the
pre-zeroed per-core output buffer with dma_scatter_add.  Host sums the two
buffers of each row.

Schedule: x-tile loads own the DMA engines first (weight loads are ordered
behind them); radix pass 1 folds into the score loop against a constant
threshold grid; passes 2-4 run as Sign-activation counts over a
DMA-broadcast score replica; rank compaction is a digit-decomposed one-hot
bf16 matmul whose stacked constants emit both the int32 gather and int16
scatter index layouts; gathered tokens are transposed by the DMA xbar
(dma_start_transpose); and the output scatter is dma_scatter_add (per-index
descriptors) overlapped with the tail of the second matmul.
"""

import sys

sys.path.insert(0, "/opt/trn_rl_repo")

from contextlib import ExitStack

import numpy as np

from concourse import bass, bass_isa, mybir
from concourse import bacc
import concourse.tile as tile
from concourse.bass import IndirectOffsetOnAxis

B, L, D = 4, 4096, 1024
DFF = 4 * D
K = L // 2              # 2048 selected tokens per row
NCORES = 8
P = 128
NT = L // P             # 32 token tiles per row
SEL = K // 2            # 1024 selected tokens per core
NSJ = SEL // P          # 8 selected-token blocks
ND = D // P             # 8 d chunks
NM = DFF // P           # 32 dff tiles
NKGRP = 4               # w2 k-chunks per streamed tile
RADIX_PASSES = 4

F32 = mybir.dt.float32
BF16 = mybir.dt.bfloat16
I32 = mybir.dt.int32
I16 = mybir.dt.int16
Alu = mybir.AluOpType
Act = mybir.ActivationFunctionType
Red = bass_isa.ReduceOp


def build_program():
    nc = bacc.Bacc(
        "TRN2",
        target_bir_lowering=False,
        debug=False,
        enable_asserts=False,
        num_devices=NCORES,
    )

    x_row = nc.dram_tensor("x_row", [L, D], F32, kind="ExternalInput").ap()
    w1 = nc.dram_tensor("w1", [D, DFF], F32, kind="ExternalInput").ap()
    w2 = nc.dram_tensor("w2", [DFF, D], F32, kind="ExternalInput").ap()
    wr = nc.dram_tensor("wr", [1, D], F32, kind="ExternalInput").ap()
    b1t = nc.dram_tensor("b1t", [P, NM], F32, kind="ExternalInput").ap()
    b2 = nc.dram_tensor("b2", [1, D], F32, kind="ExternalInput").ap()
    hbase = nc.dram_tensor("hbase", [1, 1], F32, kind="ExternalInput").ap()
    identb = nc.dram_tensor("identb", [P, P], BF16, kind="ExternalInput").ap()
    identf = nc.dram_tensor("identf", [P, P], F32, kind="ExternalInput").ap()
    ltri = nc.dram_tensor("ltri128", [P, P], F32, kind="ExternalInput").ap()
    slt32 = nc.dram_tensor("slt32", [NT, NT], F32, kind="ExternalInput").ap()
    id32 = nc.dram_tensor("id32", [NT, NT], F32, kind="ExternalInput").ap()
    ones_1x128 = nc.dram_tensor("ones_1x128", [1, P], F32, kind="ExternalInput").ap()
    ones_1x128b = nc.dram_tensor("ones_1x128b", [1, P], BF16, kind="ExternalInput").ap()
    ones_128x1 = nc.dram_tensor("ones_128x1", [P, 1], F32, kind="ExternalInput").ap()
    ones128 = nc.dram_tensor("ones128", [P, P], F32, kind="ExternalInput").ap()
    ones_32x128 = nc.dram_tensor("ones_32x128", [NT, P], F32, kind="ExternalInput").ap()
    rep16 = nc.dram_tensor("rep16", [32, P], F32, kind="ExternalInput").ap()
    ewrap = nc.dram_tensor("ewrap", [32, 8 * P], F32, kind="ExternalInput").ap()

    out_row = nc.dram_tensor("out_row", [L, D], F32, kind="ExternalOutput").ap()

    GRPS = ((0, 20), (20, 30))
    scores_dg = [nc.dram_tensor(f"scores_dg{i}", [hi - lo, P], F32).ap()
                 for i, (lo, hi) in enumerate(GRPS)]

    with tile.TileContext(nc) as tc, ExitStack() as S0:
        const = S0.enter_context(tc.tile_pool(name="const", bufs=1))
        # pool stack (LIFO): const | ht | w1 | dig | ...phases
        ht_ctx = tc.tile_pool(name="ht", bufs=1)
        ht_pool = ht_ctx.__enter__()
        ht = ht_pool.tile([P, NM, SEL], BF16)
        w1_ctx = tc.tile_pool(name="w1bf", bufs=1)
        w1_pool = w1_ctx.__enter__()

        def cload(pool, ap, shape, dtype=F32, name=None):
            t = pool.tile(shape, dtype, name=name)
            nc.sync.dma_start(out=t[:], in_=ap)
            return t

        # ---- SP-queue order: wr, o1, oc, hbase FIRST (phase A needs them) ---
        wr_sb = cload(const, wr, [1, D], name="c_wr")
        o1x128_sb = cload(const, ones_1x128, [1, P], name="c_o1")
        o128x1_sb = cload(const, ones_128x1, [P, 1], name="c_oc")
        ones128_sb = cload(const, ones128, [P, P], name="c_o128")
        hb_sb = cload(const, hbase, [1, 1], name="c_hb")
        identf_sb = cload(const, identf, [P, P], name="c_idf")

        # w1 tiles exist from the start (loads are issued after the radix)
        w1bf = [w1_pool.tile([P, DFF], BF16, name=f"w1bf_{kd}")
                for kd in range(ND)]

        # ---- Pool-queue iotas (independent of SP queue) ---------------------
        # big digit-decomposition iota tables live only through phase E
        dig_ctx = tc.tile_pool(name="dig", bufs=1)
        dig = dig_ctx.__enter__()

        iota_i = const.tile([P, 1], I32)
        nc.gpsimd.iota(iota_i[:], pattern=[[1, 1]], base=0, channel_multiplier=1)
        tokid = const.tile([P, NT], I32)
        nc.gpsimd.iota(tokid[:], pattern=[[P, NT]], base=0, channel_multiplier=1)
        iC_i = const.tile([P, NT], I32)
        nc.gpsimd.iota(iC_i[:], pattern=[[1, NT]], base=0, channel_multiplier=0)
        iQ_i = const.tile([P, 128], I32)
        nc.gpsimd.iota(iQ_i[:], pattern=[[1, 128]], base=0, channel_multiplier=0)
        iK64_i = dig.tile([P, NT, 64], I16)
        nc.gpsimd.iota(iK64_i[:], pattern=[[0, NT], [1, 64]], base=0,
                       channel_multiplier=0)
        iJ16_i = dig.tile([P, NT, 16], I16)
        nc.gpsimd.iota(iJ16_i[:], pattern=[[0, NT], [1, 16]], base=0,
                       channel_multiplier=0)
        i7_i = const.tile([P, 7], I32)
        nc.gpsimd.iota(i7_i[:], pattern=[[1, 7]], base=1, channel_multiplier=0)

        iota_f = const.tile([P, 1], F32)
        nc.vector.tensor_copy(out=iota_f[:], in_=iota_i[:])
        tokidf = const.tile([P, NT], F32)
        nc.vector.tensor_copy(out=tokidf[:], in_=tokid[:])
        cvalf = const.tile([P, NT], F32)
        nc.vector.tensor_copy(out=cvalf[:], in_=iC_i[:])
        iK64b = dig.tile([P, NT, 64], BF16)
        nc.vector.tensor_copy(out=iK64b[:], in_=iK64_i[:])
        iJ16b = dig.tile([P, NT, 16], BF16)
        nc.vector.tensor_copy(out=iJ16b[:], in_=iJ16_i[:])
        iotab = const.tile([P, 1], BF16)
        nc.vector.tensor_copy(out=iotab[:], in_=iota_i[:])
        cvalb = const.tile([P, NT], BF16)
        nc.vector.tensor_copy(out=cvalb[:], in_=iC_i[:])
        i7f = const.tile([P, 7], F32)
        nc.vector.tensor_copy(out=i7f[:], in_=i7_i[:])
        thr128 = const.tile([P, 7], F32)
        nc.vector.tensor_scalar(out=thr128[:], in0=i7f[:], scalar1=128.0,
                                scalar2=None, op0=Alu.mult)
        thr16 = const.tile([P, 7], F32)
        nc.vector.tensor_scalar(out=thr16[:], in0=i7f[:], scalar1=16.0,
                                scalar2=None, op0=Alu.mult)
        # radix pass-1 threshold grid (build-time constants: lo=-16, w=0.25)
        iQf = const.tile([P, 128], F32)
        nc.vector.tensor_copy(out=iQf[:], in_=iQ_i[:])
        thr1row = const.tile([P, 128], F32)
        nc.vector.tensor_scalar(out=thr1row[:], in0=iQf[:], scalar1=32.0 / P,
                                scalar2=-16.0, op0=Alu.mult, op1=Alu.add)
        # negated per-pass threshold offsets for radix passes 2..4
        W1P = 32.0 / P
        nthrbs = []
        for p_ in range(1, RADIX_PASSES):
            w_p = W1P / (P ** p_)
            t_ = const.tile([P, 1], F32, name=f"nthrb{p_}")
            nc.vector.tensor_scalar(out=t_[:], in0=iota_f[:], scalar1=-w_p,
                                    scalar2=None, op0=Alu.mult)
            nthrbs.append((w_p, t_))
        hb_col = const.tile([P, 1], F32)
        nc.gpsimd.partition_broadcast(hb_col[:], hb_sb[:])

        scores_sb = const.tile([P, NT], F32)
        selidx_sb = const.tile([P, NSJ], I32)
        idx16_sb = const.tile([P, SEL // 16], I16)

        misc_psum_ctx = tc.tile_pool(name="misc_psum", bufs=2, space="PSUM")
        misc_psum = misc_psum_ctx.__enter__()

        # ---- phase A: router scores (fp32, exact; router bias dropped — it
        # shifts every score equally so the top-K set is unchanged).  The
        # first radix pass uses a build-time-constant threshold grid, so its
        # per-tile compare + count-matmul accumulation is folded in here. -----
        c1_psum_ctx = tc.tile_pool(name="c1_psum", bufs=1, space="PSUM")
        c1_psum = c1_psum_ctx.__enter__()
        cnt1_ps = c1_psum.tile([P, 128], F32, name="cnt1")
        nlo = const.tile([P, 1], F32, name="nlo")
        with ExitStack() as SA:
            apool = SA.enter_context(tc.tile_pool(name="apool", bufs=1))
            xs_pool = SA.enter_context(tc.tile_pool(name="xs", bufs=6))
            junk_pool = SA.enter_context(tc.tile_pool(name="junk", bufs=2))
            cmp_pool = SA.enter_context(tc.tile_pool(name="cmp", bufs=3))

            wrb = apool.tile([P, D], F32)
            for n in range(D // 512):
                pt = misc_psum.tile([P, 512], F32, name="mp")
                nc.tensor.matmul(out=pt[:], lhsT=o1x128_sb[:],
                                 rhs=wr_sb[:, n * 512:(n + 1) * 512],
                                 start=True, stop=True)
                nc.vector.tensor_copy(out=wrb[:, n * 512:(n + 1) * 512], in_=pt[:])

            x_last = None
            for t in range(NT):
                x_t = xs_pool.tile([P, D], F32)
                nc.sync.dma_start(out=x_t[:], in_=x_row[t * P:(t + 1) * P, :])
                x_last = x_t
                if t == 26:
                    nc.sync.dma_start(
                        out=scores_dg[0].rearrange("c p -> p c"),
                        in_=scores_sb[:, 0:20])
                prod = junk_pool.tile([P, D], F32, name="prod")
                nc.vector.tensor_tensor(out=prod[:], in0=x_t[:], in1=wrb[:],
                                        op=Alu.mult)
                sink = junk_pool.tile([P, D], BF16, name="sink")
                nc.scalar.activation(out=sink[:], in_=prod[:], func=Act.Identity,
                                     bias=0.0, scale=1.0,
                                     accum_out=scores_sb[:, t:t + 1])
                cmp_t = cmp_pool.tile([P, 128], F32, name="cmp")
                nc.vector.tensor_tensor(
                    out=cmp_t[:],
                    in0=scores_sb[:, t:t + 1].to_broadcast([P, 128]),
                    in1=thr1row[:], op=Alu.is_ge)
                nc.tensor.matmul(out=cnt1_ps[:], lhsT=ones128_sb[:], rhs=cmp_t[:],
                                 start=(t == 0), stop=(t == NT - 1),
                                 skip_group_check=True)


            # pass-1 finalize on every partition (count matmul used an
            # all-ones lhsT, so each partition holds the full count row):
            # nlo = -(lo1) = 16 - (sum(cnt>=K) - 1)*0.25
            selr = apool.tile([P, 128], F32, name="selr")
            nc.vector.tensor_scalar(out=selr[:], in0=cnt1_ps[:],
                                    scalar1=float(K), scalar2=None,
                                    op0=Alu.is_ge)
            s1 = apool.tile([P, 1], F32, name="s1")
            nc.vector.tensor_reduce(out=s1[:], in_=selr[:],
                                    axis=mybir.AxisListType.X, op=Alu.add)
            q1 = apool.tile([P, 1], F32, name="q1")
            nc.vector.tensor_scalar(out=q1[:], in0=s1[:], scalar1=-1.0,
                                    scalar2=-W1P, op0=Alu.add, op1=Alu.mult)
            nc.vector.tensor_scalar(out=nlo[:], in0=q1[:], scalar1=16.0,
                                    scalar2=None, op0=Alu.add)
        c1_psum_ctx.__exit__(None, None, None)

        offf_c = const.tile([P, NT], F32)
        maskf_c = const.tile([P, NT], F32)

        # ---- phases B+C+D: replicate scores, radix threshold, rank ----------
        with ExitStack() as SC:
            radix = SC.enter_context(tc.tile_pool(name="radix", bufs=2))
            rep_pool = SC.enter_context(tc.tile_pool(name="rep", bufs=1))

            # broadcast-read the spilled scores, one DMA per 1024-token group,
            # FIRST on the in-order SP queue right after the x loads (the
            # remaining const loads queue behind, they aren't needed till later)
            scores_rep = rep_pool.tile([P, L], F32)
            for gi, (glo, ghi) in enumerate(GRPS):
                n_ = (ghi - glo) * P
                if gi > 0:   # g0 was spilled inside the x stream
                    nc.sync.dma_start(
                        out=scores_dg[gi].rearrange("c p -> p c"),
                        in_=scores_sb[:, glo:ghi])
                nc.sync.dma_start(
                    out=scores_rep[:, glo * P:ghi * P],
                    in_=scores_dg[gi].rearrange("c p -> () (c p)")
                    .to_broadcast([P, n_]))
            # the last two tiles gate radix pass 2: replicate them on the
            # idle PE (exact f32 transpose + all-ones broadcast matmul)
            # instead of the spill->broadcast DMA round-trip
            for ti in (30, 31):
                tp_ = misc_psum.tile([1, P], F32, name="mp")
                nc.tensor.transpose(out=tp_[:], in_=scores_sb[:, ti:ti + 1],
                                    identity=identf_sb[:])
                srow = radix.tile([1, P], F32, name="srow")
                nc.vector.tensor_copy(out=srow[:], in_=tp_[:])
                rp_ = misc_psum.tile([P, P], F32, name="mp")
                nc.tensor.matmul(out=rp_[:], lhsT=o1x128_sb[:], rhs=srow[:],
                                 start=True, stop=True)
                nc.vector.tensor_copy(out=scores_rep[:, ti * P:(ti + 1) * P],
                                      in_=rp_[:])

            # gate the w1 cast-loads behind the score broadcast so their DMAs
            # cannot delay it (WAW edge: the w1 DMA overwrites the gate byte)
            for kd in range(ND):
                nc.vector.tensor_copy(out=w1bf[kd][0:1, 0:1],
                                      in_=scores_rep[0:1, kd:kd + 1])

            # ---- remaining small consts on the SP queue ---------------------
            b1t_sb = cload(const, b1t, [P, NM], name="c_b1t")
            identb_sb = cload(const, identb, [P, P], BF16, name="c_idb")
            ltri_sb = cload(const, ltri, [P, P], name="c_lt")
            slt32_sb = cload(const, slt32, [NT, NT], name="c_sl")
            id32_sb = cload(const, id32, [NT, NT], name="c_id32")
            o1x128b_sb = cload(const, ones_1x128b, [1, P], BF16, name="c_o1b")
            o32x128_sb = cload(const, ones_32x128, [NT, P], name="c_o32")
            rep16_sb = cload(const, rep16, [32, P], name="c_rep16")
            ewrap_sb = cload(const, ewrap, [32, 8 * P], name="c_ew")
            b2bf_sb = const.tile([1, D], BF16)
            nc.gpsimd.dma_start(out=b2bf_sb[:], in_=b2)  # cast f32 -> bf16

            sjunk = rep_pool.tile([P, L], BF16, name="sjunk")
            NSPL = 2560          # tokens covered by broadcast group 0
            for pi, (w_p, nthrb_p) in enumerate(nthrbs):
                nthr = radix.tile([P, 1], F32, name="nthr")
                nc.vector.tensor_tensor(out=nthr[:], in0=nlo[:], in1=nthrb_p[:],
                                        op=Alu.add)
                if pi == 0:
                    # group-0 scores arrive first; count them while the tail
                    # broadcasts finish, then add the remainder
                    sgA = radix.tile([P, 1], F32, name="sgA")
                    nc.scalar.activation(out=sjunk[:, :NSPL],
                                         in_=scores_rep[:, :NSPL],
                                         func=Act.Sign, bias=nthr[:, :1],
                                         scale=1.0, accum_out=sgA[:])
                    sgB = radix.tile([P, 1], F32, name="sgB")
                    nc.scalar.activation(out=sjunk[:, NSPL:],
                                         in_=scores_rep[:, NSPL:],
                                         func=Act.Sign, bias=nthr[:, :1],
                                         scale=1.0, accum_out=sgB[:])
                    sgn = radix.tile([P, 1], F32, name="sgn")
                    nc.vector.tensor_tensor(out=sgn[:], in0=sgA[:], in1=sgB[:],
                                            op=Alu.add)
                else:
                    sgn = radix.tile([P, 1], F32, name="sgn")
                    nc.scalar.activation(out=sjunk[:], in_=scores_rep[:],
                                         func=Act.Sign, bias=nthr[:, :1],
                                         scale=1.0, accum_out=sgn[:])
                sel = radix.tile([P, 1], F32, name="sel")
                nc.vector.tensor_scalar(out=sel[:], in0=sgn[:], scalar1=0.0,
                                        scalar2=None, op0=Alu.is_ge)
                s_all = radix.tile([P, 1], F32, name="s_all")
                nc.gpsimd.partition_all_reduce(s_all[:], sel[:], channels=P,
                                               reduce_op=Red.add)
                nd = radix.tile([P, 1], F32, name="nd")
                nc.vector.tensor_scalar(out=nd[:], in0=s_all[:], scalar1=-1.0,
                                        scalar2=-w_p, op0=Alu.add, op1=Alu.mult)
                nlo2 = radix.tile([P, 1], F32, name="nlo2")
                nc.vector.tensor_tensor(out=nlo2[:], in0=nlo[:], in1=nd[:],
                                        op=Alu.add)
                nlo = nlo2

            # ---- mask + global rank (exclusive prefix of mask) --------------
            maskf = radix.tile([P, NT], F32, name="maskf")
            nc.vector.tensor_scalar(out=maskf[:], in0=scores_sb[:],
                                    scalar1=nlo[:, :1], scalar2=0.0,
                                    op0=Alu.add, op1=Alu.is_ge)
            colsum_p = misc_psum.tile([NT, 1], F32, name="mp")
            nc.tensor.matmul(out=colsum_p[:], lhsT=maskf[:], rhs=o128x1_sb[:],
                             start=True, stop=True)
            colsum = radix.tile([NT, 1], F32, name="colsum")
            nc.vector.tensor_copy(out=colsum[:], in_=colsum_p[:])
            excl_p = misc_psum.tile([NT, 1], F32, name="mp")
            nc.tensor.matmul(out=excl_p[:], lhsT=slt32_sb[:], rhs=colsum[:],
                             start=True, stop=True)
            excl = radix.tile([NT, 1], F32, name="excl")
            nc.vector.tensor_copy(out=excl[:], in_=excl_p[:])
            diag = radix.tile([NT, NT], F32, name="diag")
            nc.vector.tensor_tensor(out=diag[:], in0=id32_sb[:],
                                    in1=excl[:, :1].to_broadcast([NT, NT]),
                                    op=Alu.mult)
            rank_p = misc_psum.tile([P, NT], F32, name="mp")
            nc.tensor.matmul(out=rank_p[:], lhsT=ltri_sb[:], rhs=maskf[:],
                             start=True, stop=False, skip_group_check=True)
            nc.tensor.matmul(out=rank_p[:], lhsT=o32x128_sb[:], rhs=diag[:],
                             start=False, stop=True, skip_group_check=True)
            nc.vector.tensor_scalar(out=offf_c[:], in0=rank_p[:],
                                    scalar1=hb_col[:, :1], scalar2=None,
                                    op0=Alu.subtract)
            nc.vector.tensor_copy(out=maskf_c[:], in_=maskf[:])

        misc_psum_ctx.__exit__(None, None, None)

        # ---- w1 cast-loads on the Pool queue.  Positioned after the radix
        # all_reduces so the in-order queue starts them only ~70us in, after
        # the x-tile DMAs have drained (they'd otherwise steal DMA bandwidth
        # from the critical-path score loads). ---------------------------------
        w1bf = []
        for kd in range(ND):
            t_ = w1_pool.tile([P, DFF], BF16, name=f"w1bf_{kd}")
            nc.gpsimd.dma_start(out=t_[:], in_=w1[kd * P:(kd + 1) * P, :])
            w1bf.append(t_)

        # ---- phase E: digit split + one-hot compaction matmuls --------------
        # off in [0, SEL) for in-window selected tokens; any other off value
        # (negative rank-window miss, >=SEL, or collision of an unselected
        # token) produces no match in the lo-digit equality below, and
        # unselected tokens are additionally zeroed via tokid*mask weights.
        with ExitStack() as SE:
            ep = SE.enter_context(tc.tile_pool(name="epool", bufs=1))
            e_psum = SE.enter_context(tc.tile_pool(name="e_psum", bufs=2,
                                                   space="PSUM"))
            off = offf_c
            eq7a = ep.tile([P, NT, 7], F32, name="eq7a")
            nc.vector.tensor_tensor(
                out=eq7a[:], in0=off[:, :, None].to_broadcast([P, NT, 7]),
                in1=thr128[:, None, :].to_broadcast([P, NT, 7]), op=Alu.is_ge)
            hi128 = ep.tile([P, NT], F32, name="hi128")
            nc.vector.tensor_reduce(out=hi128[:], in_=eq7a[:],
                                    axis=mybir.AxisListType.X, op=Alu.add)
            hm = ep.tile([P, NT], F32, name="hm")
            nc.vector.tensor_scalar(out=hm[:], in0=hi128[:], scalar1=-128.0,
                                    scalar2=None, op0=Alu.mult)
            lo128 = ep.tile([P, NT], F32, name="lo128")
            nc.vector.tensor_tensor(out=lo128[:], in0=off[:], in1=hm[:],
                                    op=Alu.add)
            eq7b = ep.tile([P, NT, 7], F32, name="eq7b")
            nc.vector.tensor_tensor(
                out=eq7b[:], in0=lo128[:, :, None].to_broadcast([P, NT, 7]),
                in1=thr16[:, None, :].to_broadcast([P, NT, 7]), op=Alu.is_ge)
            mid = ep.tile([P, NT], F32, name="mid")
            nc.vector.tensor_reduce(out=mid[:], in_=eq7b[:],
                                    axis=mybir.AxisListType.X, op=Alu.add)
            hm2 = ep.tile([P, NT], F32, name="hm2")
            nc.vector.tensor_scalar(out=hm2[:], in0=mid[:], scalar1=-16.0,
                                    scalar2=None, op0=Alu.mult)
            lo16b = ep.tile([P, NT], BF16, name="lo16b")
            nc.vector.tensor_tensor(out=lo16b[:], in0=lo128[:], in1=hm2[:],
                                    op=Alu.add)
            h8 = ep.tile([P, NT], F32, name="h8")
            nc.vector.tensor_scalar(out=h8[:], in0=hi128[:], scalar1=8.0,
                                    scalar2=None, op0=Alu.mult)
            hi16b = ep.tile([P, NT], BF16, name="hi16b")
            nc.vector.tensor_tensor(out=hi16b[:], in0=h8[:], in1=mid[:],
                                    op=Alu.add)
            # token id = c*128 + p; weight the SMALL equality factors by
            # c*mask (chain C, lhsT cols 0:16) and p*mask (chain D, cols
            # 16:32), then sel16 = 128*C + D.  All factors are small exact
            # integers, so the chain runs in bf16 (1 cycle/row matmuls).
            maskb = ep.tile([P, NT], BF16, name="maskb")
            nc.vector.tensor_copy(out=maskb[:], in_=maskf_c[:])
            cwm = ep.tile([P, NT], BF16, name="cwm")
            nc.vector.tensor_tensor(out=cwm[:], in0=cvalb[:], in1=maskb[:],
                                    op=Alu.mult)
            pwm = ep.tile([P, NT], BF16, name="pwm")
            nc.vector.tensor_tensor(out=pwm[:], in0=maskb[:],
                                    in1=iotab[:, :1].to_broadcast([P, NT]),
                                    op=Alu.mult)

            # build the equality factors in half-tile chunks so the first 16
            # compaction matmuls overlap construction of the second half
            eq16 = ep.tile([P, NT, 16], BF16, name="eq16")
            eqcp = ep.tile([P, NT, 32], BF16, name="eqcp")
            eq64 = ep.tile([P, NT, 64], BF16, name="eq64")
            pCD = e_psum.tile([32, 64], F32, name="pCD")
            H = NT // 2
            for h0 in (0, H):
                sl = slice(h0, h0 + H)
                nc.vector.tensor_tensor(
                    out=eq16[:, sl, :], in0=iJ16b[:, sl, :],
                    in1=lo16b[:, sl, None].to_broadcast([P, H, 16]),
                    op=Alu.is_equal)
                nc.vector.tensor_tensor(
                    out=eqcp[:, sl, 0:16], in0=eq16[:, sl, :],
                    in1=cwm[:, sl, None].to_broadcast([P, H, 16]), op=Alu.mult)
                nc.vector.tensor_tensor(
                    out=eqcp[:, sl, 16:32], in0=eq16[:, sl, :],
                    in1=pwm[:, sl, None].to_broadcast([P, H, 16]), op=Alu.mult)
                nc.vector.tensor_tensor(
                    out=eq64[:, sl, :], in0=iK64b[:, sl, :],
                    in1=hi16b[:, sl, None].to_broadcast([P, H, 64]),
                    op=Alu.is_equal)
                for c in range(h0, h0 + H):
                    nc.tensor.matmul(out=pCD[:], lhsT=eqcp[:, c, :],
                                     rhs=eq64[:, c, :], start=(c == 0),
                                     stop=(c == NT - 1), skip_group_check=True)

            sCD = ep.tile([32, 64], F32, name="sCD")
            nc.vector.tensor_copy(out=sCD[:], in_=pCD[:])

            # scatter index layout [128, 64] (16-wrap replicated to 128);
            # lhsT folds the 128*C + D combine (rows 0:16 scaled by 128)
            rep_ps = e_psum.tile([P, 64], F32, name="rep_ps")
            nc.tensor.matmul(out=rep_ps[:], lhsT=rep16_sb[:], rhs=sCD[:],
                             start=True, stop=True)
            nc.vector.tensor_copy(out=idx16_sb[:], in_=rep_ps[:])  # f32->i16

            # gather index layout [128, 8]: selidx[p, k] = sel16[p%16, 8k+p//16]
            selps = e_psum.tile([P, NSJ], F32, name="selps")
            for g in range(8):
                nc.tensor.matmul(out=selps[:], lhsT=ewrap_sb[:, g * P:(g + 1) * P],
                                 rhs=sCD[:, g::8], start=(g == 0),
                                 stop=(g == 7), skip_group_check=True)
            nc.vector.tensor_copy(out=selidx_sb[:], in_=selps[:])  # f32->i32

        dig_ctx.__exit__(None, None, None)

        # ---- gather + transpose + MLP ---------------------------------------
        if True:
            with ExitStack() as SB:
                xt_pool = SB.enter_context(tc.tile_pool(name="xt", bufs=1))
                xsel_pool = SB.enter_context(tc.tile_pool(name="xsel", bufs=5))
                mm1_psum = SB.enter_context(tc.tile_pool(name="mm1_psum", bufs=6,
                                                         space="PSUM"))

                # xt3[p, kd, t] = x_sel[t, kd*128+p], built by the DMA-engine
                # xbar transpose (one per gathered 128-token chunk)
                xt3 = xt_pool.tile([P, ND, SEL], BF16)
                for j in range(NSJ):
                    xs = xsel_pool.tile([P, D], BF16, name="xsel")
                    nc.gpsimd.indirect_dma_start(
                        out=xs[:], out_offset=None, in_=x_row,
                        in_offset=IndirectOffsetOnAxis(ap=selidx_sb[:, j:j + 1],
                                                       axis=0))
                    nc.scalar.dma_start_transpose(
                        out=xt3[:, :, j * P:(j + 1) * P], in_=xs[:])

                # ---- mm1: ht[m, sel] = gelu(w1^T x_sel^T + b1).  The first
                # four token blocks are 128 wide so the PE starts the moment
                # each transpose lands instead of waiting for four of them;
                # the second half runs as one 512-wide block.
                for t0, tw in [(0, P), (P, P), (2 * P, P), (3 * P, P),
                               (512, 512)]:
                    for m in range(NM):
                        ph = mm1_psum.tile([P, tw], F32, name="ph")
                        for kd in range(ND):
                            nc.tensor.matmul(
                                out=ph[:],
                                lhsT=w1bf[kd][:, m * P:(m + 1) * P],
                                rhs=xt3[:, kd, t0:t0 + tw],
                                start=(kd == 0), stop=(kd == ND - 1),
                            )
                        nc.scalar.activation(
                            out=ht[:, m, t0:t0 + tw], in_=ph[:],
                            func=Act.Gelu_apprx_tanh, bias=b1t_sb[:, m:m + 1],
                            scale=1.0,
                        )

            w1_ctx.__exit__(None, None, None)  # free w1 region for w2 stream

            # ---- mm2: y[sel, D] = ht^T @ w2 + b2, then scatter-add ----------
            with ExitStack() as SY:
                y_pool = SY.enter_context(tc.tile_pool(name="y", bufs=1))
                w2_pool = SY.enter_context(tc.tile_pool(name="w2s", bufs=16))
                mm2_psum = SY.enter_context(tc.tile_pool(name="mm2_psum", bufs=8,
                                                         space="PSUM"))
                # d-half 0: kg-major accumulation (w2 tiles stream in, all 8
                # token-block psums accumulate together)
                n = 0
                y_0 = y_pool.tile([P, NSJ, 512], F32, name="y0")
                pys = [mm2_psum.tile([P, 512], F32, name="py")
                       for _ in range(NSJ)]
                w2n1 = []   # d-half-1 tiles retained for the s-major pass
                for s in range(NSJ):
                    nc.tensor.matmul(
                        out=pys[s][:], lhsT=o1x128b_sb[:],
                        rhs=b2bf_sb[:, :512],
                        start=True, stop=False, skip_group_check=True,
                    )
                for kg in range(NM // NKGRP):
                    w2t = w2_pool.tile([P, NKGRP, 512], BF16, name="w2t")
                    if kg == 0:
                        # WAW gate: keep the w2 stream off the DMA engines
                        # until the gather/transpose pipeline has fed mm1
                        nc.vector.tensor_copy(out=w2t[0:1, 0, 0:1],
                                              in_=ht[0:1, 0, 0:1])
                    src = w2[:, :512].rearrange(
                        "(g p) f -> p g f", p=P)[:, kg * NKGRP:(kg + 1) * NKGRP, :]
                    nc.gpsimd.dma_start(out=w2t[:], in_=src)
                    for ki in range(NKGRP):
                        kk = kg * NKGRP + ki
                        for s in range(NSJ):
                            nc.tensor.matmul(
                                out=pys[s][:],
                                lhsT=ht[:, kk, s * P:(s + 1) * P],
                                rhs=w2t[:, ki, :],
                                start=False, stop=(kk == NM - 1),
                                skip_group_check=True,
                            )
                # prefetch d-half-1 w2 tiles while the n=0 tail accumulates
                for kg in range(NM // NKGRP):
                    w2t = w2_pool.tile([P, NKGRP, 512], BF16, name="w2t")
                    src = w2[:, 512:].rearrange(
                        "(g p) f -> p g f", p=P)[:, kg * NKGRP:(kg + 1) * NKGRP, :]
                    nc.gpsimd.dma_start(out=w2t[:], in_=src)
                    w2n1.append(w2t)
                for s in range(NSJ):
                    nc.scalar.activation(out=y_0[:, s, :], in_=pys[s][:],
                                         func=Act.Copy, bias=0.0, scale=1.0)
                    if s % 4 == 3:
                        h = s // 4
                        nc.gpsimd.dma_scatter_add(
                            out_row[:, :512],
                            y_0[:, h * 4:(h + 1) * 4, :],
                            idx16_sb[:, h * 32:(h + 1) * 32],
                            SEL // 2,
                            SEL // 2,
                            512,
                            elem_step=D,
                        )

                # d-half 1: s-major (each token block finishes early and its
                # rows scatter while the next block accumulates)
                y_1 = y_pool.tile([P, NSJ, 512], F32, name="y1")
                for s in range(NSJ):
                    py = mm2_psum.tile([P, 512], F32, name="py")
                    nc.tensor.matmul(
                        out=py[:], lhsT=o1x128b_sb[:], rhs=b2bf_sb[:, 512:],
                        start=True, stop=False, skip_group_check=True,
                    )
                    for kk in range(NM):
                        nc.tensor.matmul(
                            out=py[:],
                            lhsT=ht[:, kk, s * P:(s + 1) * P],
                            rhs=w2n1[kk // NKGRP][:, kk % NKGRP, :],
                            start=False, stop=(kk == NM - 1),
                            skip_group_check=True,
                        )
                    nc.scalar.activation(out=y_1[:, s, :], in_=py[:],
                                         func=Act.Copy, bias=0.0, scale=1.0)
                    nc.gpsimd.dma_scatter_add(
                        out_row[:, 512:],
                        y_1[:, s:s + 1, :],
                        idx16_sb[:, s * 8:(s + 1) * 8],
                        P,
                        P,
                        512,
                        elem_step=D,
                    )

        ht_ctx.__exit__(None, None, None)

    nc.compile()
    return nc


def make_consts():
    q = np.arange(P)
    import ml_dtypes
    consts = {
        "identb": np.eye(P, dtype=ml_dtypes.bfloat16),
        "identf": np.eye(P, dtype=np.float32),
        "ltri128": (q[:, None] < q[None, :]).astype(np.float32),  # [q, p] = q < p
        "slt32": (np.arange(NT)[:, None] < np.arange(NT)[None, :]).astype(np.float32),
        "id32": np.eye(NT, dtype=np.float32),
        "ones_1x128": np.ones((1, P), np.float32),
        "ones_1x128b": np.ones((1, P), ml_dtypes.bfloat16),
        "ones_128x1": np.ones((P, 1), np.float32),
        "ones128": np.ones((P, P), np.float32),
        "ones_32x128": np.ones((NT, P), np.float32),
        "rep16": np.vstack([
            128.0 * (np.arange(16)[:, None] == (np.arange(P)[None, :] % 16)),
            1.0 * (np.arange(16)[:, None] == (np.arange(P)[None, :] % 16)),
        ]).astype(np.float32),
    }
    # ewrap[i, g*128 + p] = 1 iff p == g*16 + i  (16-wrap -> 128-wrap expand);
    # stacked [32, .]: rows 0:16 scaled by 128 (C chain), rows 16:32 raw (D)
    ew = np.zeros((16, 8 * P), np.float32)
    for i in range(16):
        for g in range(8):
            ew[i, g * P + g * 16 + i] = 1.0
    consts["ewrap"] = np.vstack([128.0 * ew, ew]).astype(np.float32)
    return consts


def make_in_maps(x, W1, b1, W2, b2, wr, br):
    consts = make_consts()
    x = np.ascontiguousarray(np.asarray(x, np.float32))
    in_maps = []
    for c in range(NCORES):
        b, h = divmod(c, 2)
        m = {
            "x_row": x[b],
            "w1": np.asarray(W1, np.float32),
            "w2": np.asarray(W2, np.float32),
            "wr": np.asarray(wr, np.float32).reshape(1, D),
            "b1t": np.ascontiguousarray(np.asarray(b1, np.float32).reshape(NM, P).T),
            "b2": np.asarray(b2, np.float32).reshape(1, D),
            "hbase": np.array([[h * SEL]], np.float32),
        }
        m.update(consts)
        in_maps.append(m)
    return in_maps


_NC_CACHE = None


def _get_program():
    global _NC_CACHE
    if _NC_CACHE is None:
        _NC_CACHE = build_program()
    return _NC_CACHE


def kernel(x, W1, b1, W2, b2, wr, br):
    from concourse.bass_utils import run_bass_kernel_spmd

    nc = _get_program()
    in_maps = make_in_maps(x, W1, b1, W2, b2, wr, br)
    res = run_bass_kernel_spmd(nc, in_maps, list(range(NCORES))).results
    out = np.stack(
        [res[2 * b]["out_row"] + res[2 * b + 1]["out_row"] for b in range(B)]
    )
    return out.astype(np.float32)


# revision 70
# speedup vs baseline: 2.6183x; 1.0011x over previous
"""MoD (mixture-of-depths) MLP wrapper kernel for Trainium2, 8 NeuronCores.

Sharding: core c handles batch row b = c//2 and the half of that row's
top-K tokens with global selection ranks in [h*1024, (h+1)*1024), h = c%2.
Each core computes the full row's router scores + top-K threshold locally
(no collectives), gathers exactly 1024 token rows by rank via indirect DMA,
runs the FFN in bf16 (fp32 accumulation), and scatters results back into the
pre-zeroed per-core output buffer with dma_scatter_add.  Host sums the two
buffers of each row.

Schedule: x-tile loads own the DMA engines first (weight loads are ordered
behind them); radix pass 1 folds into the score loop against a constant
threshold grid; passes 2-4 run as Sign-activation counts over a
DMA-broadcast score replica; rank compaction is a digit-decomposed one-hot
bf16 matmul whose stacked constants emit both the int32 gather and int16
scatter index layouts; gathered tokens are transposed by the DMA xbar
(dma_start_transpose); and the output scatter is dma_scatter_add (per-index
descriptors) overlapped with the tail of the second matmul.
"""

import sys

sys.path.insert(0, "/opt/trn_rl_repo")

from contextlib import ExitStack

import numpy as np

from concourse import bass, bass_isa, mybir
from concourse import bacc
import concourse.tile as tile
from concourse.bass import IndirectOffsetOnAxis

B, L, D = 4, 4096, 1024
DFF = 4 * D
K = L // 2              # 2048 selected tokens per row
NCORES = 8
P = 128
NT = L // P             # 32 token tiles per row
SEL = K // 2            # 1024 selected tokens per core
NSJ = SEL // P          # 8 selected-token blocks
ND = D // P             # 8 d chunks
NM = DFF // P           # 32 dff tiles
NKGRP = 4               # w2 k-chunks per streamed tile
RADIX_PASSES = 4

F32 = mybir.dt.float32
BF16 = mybir.dt.bfloat16
I32 = mybir.dt.int32
I16 = mybir.dt.int16
Alu = mybir.AluOpType
Act = mybir.ActivationFunctionType
Red = bass_isa.ReduceOp


def build_program():
    nc = bacc.Bacc(
        "TRN2",
        target_bir_lowering=False,
        debug=False,
        enable_asserts=False,
        num_devices=NCORES,
    )

    x_row = nc.dram_tensor("x_row", [L, D], F32, kind="ExternalInput").ap()
    w1 = nc.dram_tensor("w1", [D, DFF], F32, kind="ExternalInput").ap()
    w2 = nc.dram_tensor("w2", [DFF, D], F32, kind="ExternalInput").ap()
    wr = nc.dram_tensor("wr", [1, D], F32, kind="ExternalInput").ap()
    b1t = nc.dram_tensor("b1t", [P, NM], F32, kind="ExternalInput").ap()
    b2 = nc.dram_tensor("b2", [1, D], F32, kind="ExternalInput").ap()
    hbase = nc.dram_tensor("hbase", [1, 1], F32, kind="ExternalInput").ap()
    identb = nc.dram_tensor("identb", [P, P], BF16, kind="ExternalInput").ap()
    identf = nc.dram_tensor("identf", [P, P], F32, kind="ExternalInput").ap()
    ltri = nc.dram_tensor("ltri128", [P, P], F32, kind="ExternalInput").ap()
    slt32 = nc.dram_tensor("slt32", [NT, NT], F32, kind="ExternalInput").ap()
    id32 = nc.dram_tensor("id32", [NT, NT], F32, kind="ExternalInput").ap()
    ones_1x128 = nc.dram_tensor("ones_1x128", [1, P], F32, kind="ExternalInput").ap()
    ones_1x128b = nc.dram_tensor("ones_1x128b", [1, P], BF16, kind="ExternalInput").ap()
    ones_128x1 = nc.dram_tensor("ones_128x1", [P, 1], F32, kind="ExternalInput").ap()
    ones128 = nc.dram_tensor("ones128", [P, P], F32, kind="ExternalInput").ap()
    ones_32x128 = nc.dram_tensor("ones_32x128", [NT, P], F32, kind="ExternalInput").ap()
    rep16 = nc.dram_tensor("rep16", [32, P], F32, kind="ExternalInput").ap()
    ewrap = nc.dram_tensor("ewrap", [32, 8 * P], F32, kind="ExternalInput").ap()

    out_row = nc.dram_tensor("out_row", [L, D], F32, kind="ExternalOutput").ap()

    GRPS = ((0, 20), (20, 28))
    scores_dg = [nc.dram_tensor(f"scores_dg{i}", [hi - lo, P], F32).ap()
                 for i, (lo, hi) in enumerate(GRPS)]

    with tile.TileContext(nc) as tc, ExitStack() as S0:
        const = S0.enter_context(tc.tile_pool(name="const", bufs=1))
        # pool stack (LIFO): const | ht | w1 | dig | ...phases
        ht_ctx = tc.tile_pool(name="ht", bufs=1)
        ht_pool = ht_ctx.__enter__()
        ht = ht_pool.tile([P, NM, SEL], BF16)
        w1_ctx = tc.tile_pool(name="w1bf", bufs=1)
        w1_pool = w1_ctx.__enter__()

        def cload(pool, ap, shape, dtype=F32, name=None):
            t = pool.tile(shape, dtype, name=name)
            nc.sync.dma_start(out=t[:], in_=ap)
            return t

        # ---- SP-queue order: wr, o1, oc, hbase FIRST (phase A needs them) ---
        wr_sb = cload(const, wr, [1, D], name="c_wr")
        o1x128_sb = cload(const, ones_1x128, [1, P], name="c_o1")
        o128x1_sb = cload(const, ones_128x1, [P, 1], name="c_oc")
        ones128_sb = cload(const, ones128, [P, P], name="c_o128")
        hb_sb = cload(const, hbase, [1, 1], name="c_hb")
        identf_sb = cload(const, identf, [P, P], name="c_idf")

        # w1 tiles exist from the start (loads are issued after the radix)
        w1bf = [w1_pool.tile([P, DFF], BF16, name=f"w1bf_{kd}")
                for kd in range(ND)]

        # ---- Pool-queue iotas (independent of SP queue) ---------------------
        # big digit-decomposition iota tables live only through phase E
        dig_ctx = tc.tile_pool(name="dig", bufs=1)
        dig = dig_ctx.__enter__()

        iota_i = const.tile([P, 1], I32)
        nc.gpsimd.iota(iota_i[:], pattern=[[1, 1]], base=0, channel_multiplier=1)
        tokid = const.tile([P, NT], I32)
        nc.gpsimd.iota(tokid[:], pattern=[[P, NT]], base=0, channel_multiplier=1)
        iC_i = const.tile([P, NT], I32)
        nc.gpsimd.iota(iC_i[:], pattern=[[1, NT]], base=0, channel_multiplier=0)
        iQ_i = const.tile([P, 128], I32)
        nc.gpsimd.iota(iQ_i[:], pattern=[[1, 128]], base=0, channel_multiplier=0)
        iK64_i = dig.tile([P, NT, 64], I16)
        nc.gpsimd.iota(iK64_i[:], pattern=[[0, NT], [1, 64]], base=0,
                       channel_multiplier=0)
        iJ16_i = dig.tile([P, NT, 16], I16)
        nc.gpsimd.iota(iJ16_i[:], pattern=[[0, NT], [1, 16]], base=0,
                       channel_multiplier=0)
        i7_i = const.tile([P, 7], I32)
        nc.gpsimd.iota(i7_i[:], pattern=[[1, 7]], base=1, channel_multiplier=0)

        iota_f = const.tile([P, 1], F32)
        nc.vector.tensor_copy(out=iota_f[:], in_=iota_i[:])
        tokidf = const.tile([P, NT], F32)
        nc.vector.tensor_copy(out=tokidf[:], in_=tokid[:])
        cvalf = const.tile([P, NT], F32)
        nc.vector.tensor_copy(out=cvalf[:], in_=iC_i[:])
        iK64b = dig.tile([P, NT, 64], BF16)
        nc.vector.tensor_copy(out=iK64b[:], in_=iK64_i[:])
        iJ16b = dig.tile([P, NT, 16], BF16)
        nc.vector.tensor_copy(out=iJ16b[:], in_=iJ16_i[:])
        iotab = const.tile([P, 1], BF16)
        nc.vector.tensor_copy(out=iotab[:], in_=iota_i[:])
        cvalb = const.tile([P, NT], BF16)
        nc.vector.tensor_copy(out=cvalb[:], in_=iC_i[:])
        i7f = const.tile([P, 7], F32)
        nc.vector.tensor_copy(out=i7f[:], in_=i7_i[:])
        thr128 = const.tile([P, 7], F32)
        nc.vector.tensor_scalar(out=thr128[:], in0=i7f[:], scalar1=128.0,
                                scalar2=None, op0=Alu.mult)
        thr16 = const.tile([P, 7], F32)
        nc.vector.tensor_scalar(out=thr16[:], in0=i7f[:], scalar1=16.0,
                                scalar2=None, op0=Alu.mult)
        # radix pass-1 threshold grid (build-time constants: lo=-16, w=0.25)
        iQf = const.tile([P, 128], F32)
        nc.vector.tensor_copy(out=iQf[:], in_=iQ_i[:])
        thr1row = const.tile([P, 128], F32)
        nc.vector.tensor_scalar(out=thr1row[:], in0=iQf[:], scalar1=32.0 / P,
                                scalar2=-16.0, op0=Alu.mult, op1=Alu.add)
        # negated per-pass threshold offsets for radix passes 2..4
        W1P = 32.0 / P
        nthrbs = []
        for p_ in range(1, RADIX_PASSES):
            w_p = W1P / (P ** p_)
            t_ = const.tile([P, 1], F32, name=f"nthrb{p_}")
            nc.vector.tensor_scalar(out=t_[:], in0=iota_f[:], scalar1=-w_p,
                                    scalar2=None, op0=Alu.mult)
            nthrbs.append((w_p, t_))
        hb_col = const.tile([P, 1], F32)
        nc.gpsimd.partition_broadcast(hb_col[:], hb_sb[:])

        scores_sb = const.tile([P, NT], F32)
        selidx_sb = const.tile([P, NSJ], I32)
        idx16_sb = const.tile([P, SEL // 16], I16)

        misc_psum_ctx = tc.tile_pool(name="misc_psum", bufs=2, space="PSUM")
        misc_psum = misc_psum_ctx.__enter__()

        # ---- phase A: router scores (fp32, exact; router bias dropped — it
        # shifts every score equally so the top-K set is unchanged).  The
        # first radix pass uses a build-time-constant threshold grid, so its
        # per-tile compare + count-matmul accumulation is folded in here. -----
        c1_psum_ctx = tc.tile_pool(name="c1_psum", bufs=1, space="PSUM")
        c1_psum = c1_psum_ctx.__enter__()
        cnt1_ps = c1_psum.tile([P, 128], F32, name="cnt1")
        nlo = const.tile([P, 1], F32, name="nlo")
        with ExitStack() as SA:
            apool = SA.enter_context(tc.tile_pool(name="apool", bufs=1))
            xs_pool = SA.enter_context(tc.tile_pool(name="xs", bufs=6))
            junk_pool = SA.enter_context(tc.tile_pool(name="junk", bufs=2))
            cmp_pool = SA.enter_context(tc.tile_pool(name="cmp", bufs=3))

            wrb = apool.tile([P, D], F32)
            for n in range(D // 512):
                pt = misc_psum.tile([P, 512], F32, name="mp")
                nc.tensor.matmul(out=pt[:], lhsT=o1x128_sb[:],
                                 rhs=wr_sb[:, n * 512:(n + 1) * 512],
                                 start=True, stop=True)
                nc.vector.tensor_copy(out=wrb[:, n * 512:(n + 1) * 512], in_=pt[:])

            x_last = None
            for t in range(NT):
                x_t = xs_pool.tile([P, D], F32)
                nc.sync.dma_start(out=x_t[:], in_=x_row[t * P:(t + 1) * P, :])
                x_last = x_t
                if t == 26:
                    nc.sync.dma_start(
                        out=scores_dg[0].rearrange("c p -> p c"),
                        in_=scores_sb[:, 0:20])
                prod = junk_pool.tile([P, D], F32, name="prod")
                nc.vector.tensor_tensor(out=prod[:], in0=x_t[:], in1=wrb[:],
                                        op=Alu.mult)
                sink = junk_pool.tile([P, D], BF16, name="sink")
                nc.scalar.activation(out=sink[:], in_=prod[:], func=Act.Identity,
                                     bias=0.0, scale=1.0,
                                     accum_out=scores_sb[:, t:t + 1])
                cmp_t = cmp_pool.tile([P, 128], F32, name="cmp")
                nc.vector.tensor_tensor(
                    out=cmp_t[:],
                    in0=scores_sb[:, t:t + 1].to_broadcast([P, 128]),
                    in1=thr1row[:], op=Alu.is_ge)
                nc.tensor.matmul(out=cnt1_ps[:], lhsT=ones128_sb[:], rhs=cmp_t[:],
                                 start=(t == 0), stop=(t == NT - 1),
                                 skip_group_check=True)


            # pass-1 finalize on every partition (count matmul used an
            # all-ones lhsT, so each partition holds the full count row):
            # nlo = -(lo1) = 16 - (sum(cnt>=K) - 1)*0.25
            selr = apool.tile([P, 128], F32, name="selr")
            nc.vector.tensor_scalar(out=selr[:], in0=cnt1_ps[:],
                                    scalar1=float(K), scalar2=None,
                                    op0=Alu.is_ge)
            s1 = apool.tile([P, 1], F32, name="s1")
            nc.vector.tensor_reduce(out=s1[:], in_=selr[:],
                                    axis=mybir.AxisListType.X, op=Alu.add)
            q1 = apool.tile([P, 1], F32, name="q1")
            nc.vector.tensor_scalar(out=q1[:], in0=s1[:], scalar1=-1.0,
                                    scalar2=-W1P, op0=Alu.add, op1=Alu.mult)
            nc.vector.tensor_scalar(out=nlo[:], in0=q1[:], scalar1=16.0,
                                    scalar2=None, op0=Alu.add)
        c1_psum_ctx.__exit__(None, None, None)

        offf_c = const.tile([P, NT], F32)
        maskf_c = const.tile([P, NT], F32)

        # ---- phases B+C+D: replicate scores, radix threshold, rank ----------
        with ExitStack() as SC:
            radix = SC.enter_context(tc.tile_pool(name="radix", bufs=2))
            rep_pool = SC.enter_context(tc.tile_pool(name="rep", bufs=1))

            # broadcast-read the spilled scores, one DMA per 1024-token group,
            # FIRST on the in-order SP queue right after the x loads (the
            # remaining const loads queue behind, they aren't needed till later)
            scores_rep = rep_pool.tile([P, L], F32)
            for gi, (glo, ghi) in enumerate(GRPS):
                n_ = (ghi - glo) * P
                if gi > 0:   # g0 was spilled inside the x stream
                    nc.sync.dma_start(
                        out=scores_dg[gi].rearrange("c p -> p c"),
                        in_=scores_sb[:, glo:ghi])
                nc.sync.dma_start(
                    out=scores_rep[:, glo * P:ghi * P],
                    in_=scores_dg[gi].rearrange("c p -> () (c p)")
                    .to_broadcast([P, n_]))
            # the last two tiles gate radix pass 2: replicate them on the
            # idle PE (exact f32 transpose + all-ones broadcast matmul)
            # instead of the spill->broadcast DMA round-trip
            for ti in (28, 29, 30, 31):
                tp_ = misc_psum.tile([1, P], F32, name="mp")
                nc.tensor.transpose(out=tp_[:], in_=scores_sb[:, ti:ti + 1],
                                    identity=identf_sb[:])
                srow = radix.tile([1, P], F32, name="srow")
                nc.vector.tensor_copy(out=srow[:], in_=tp_[:])
                rp_ = misc_psum.tile([P, P], F32, name="mp")
                nc.tensor.matmul(out=rp_[:], lhsT=o1x128_sb[:], rhs=srow[:],
                                 start=True, stop=True)
                nc.vector.tensor_copy(out=scores_rep[:, ti * P:(ti + 1) * P],
                                      in_=rp_[:])

            # gate the w1 cast-loads behind the score broadcast so their DMAs
            # cannot delay it (WAW edge: the w1 DMA overwrites the gate byte)
            for kd in range(ND):
                nc.vector.tensor_copy(out=w1bf[kd][0:1, 0:1],
                                      in_=scores_rep[0:1, kd:kd + 1])

            # ---- remaining small consts on the SP queue ---------------------
            b1t_sb = cload(const, b1t, [P, NM], name="c_b1t")
            identb_sb = cload(const, identb, [P, P], BF16, name="c_idb")
            ltri_sb = cload(const, ltri, [P, P], name="c_lt")
            slt32_sb = cload(const, slt32, [NT, NT], name="c_sl")
            id32_sb = cload(const, id32, [NT, NT], name="c_id32")
            o1x128b_sb = cload(const, ones_1x128b, [1, P], BF16, name="c_o1b")
            o32x128_sb = cload(const, ones_32x128, [NT, P], name="c_o32")
            rep16_sb = cload(const, rep16, [32, P], name="c_rep16")
            ewrap_sb = cload(const, ewrap, [32, 8 * P], name="c_ew")
            b2bf_sb = const.tile([1, D], BF16)
            nc.gpsimd.dma_start(out=b2bf_sb[:], in_=b2)  # cast f32 -> bf16

            sjunk = rep_pool.tile([P, L], BF16, name="sjunk")
            NSPL = 2560          # tokens covered by broadcast group 0
            for pi, (w_p, nthrb_p) in enumerate(nthrbs):
                nthr = radix.tile([P, 1], F32, name="nthr")
                nc.vector.tensor_tensor(out=nthr[:], in0=nlo[:], in1=nthrb_p[:],
                                        op=Alu.add)
                if pi == 0:
                    # group-0 scores arrive first; count them while the tail
                    # broadcasts finish, then add the remainder
                    sgA = radix.tile([P, 1], F32, name="sgA")
                    nc.scalar.activation(out=sjunk[:, :NSPL],
                                         in_=scores_rep[:, :NSPL],
                                         func=Act.Sign, bias=nthr[:, :1],
                                         scale=1.0, accum_out=sgA[:])
                    sgB = radix.tile([P, 1], F32, name="sgB")
                    nc.scalar.activation(out=sjunk[:, NSPL:],
                                         in_=scores_rep[:, NSPL:],
                                         func=Act.Sign, bias=nthr[:, :1],
                                         scale=1.0, accum_out=sgB[:])
                    sgn = radix.tile([P, 1], F32, name="sgn")
                    nc.vector.tensor_tensor(out=sgn[:], in0=sgA[:], in1=sgB[:],
                                            op=Alu.add)
                else:
                    sgn = radix.tile([P, 1], F32, name="sgn")
                    nc.scalar.activation(out=sjunk[:], in_=scores_rep[:],
                                         func=Act.Sign, bias=nthr[:, :1],
                                         scale=1.0, accum_out=sgn[:])
                sel = radix.tile([P, 1], F32, name="sel")
                nc.vector.tensor_scalar(out=sel[:], in0=sgn[:], scalar1=0.0,
                                        scalar2=None, op0=Alu.is_ge)
                s_all = radix.tile([P, 1], F32, name="s_all")
                nc.gpsimd.partition_all_reduce(s_all[:], sel[:], channels=P,
                                               reduce_op=Red.add)
                nd = radix.tile([P, 1], F32, name="nd")
                nc.vector.tensor_scalar(out=nd[:], in0=s_all[:], scalar1=-1.0,
                                        scalar2=-w_p, op0=Alu.add, op1=Alu.mult)
                nlo2 = radix.tile([P, 1], F32, name="nlo2")
                nc.vector.tensor_tensor(out=nlo2[:], in0=nlo[:], in1=nd[:],
                                        op=Alu.add)
                nlo = nlo2

            # ---- mask + global rank (exclusive prefix of mask) --------------
            maskf = radix.tile([P, NT], F32, name="maskf")
            nc.vector.tensor_scalar(out=maskf[:], in0=scores_sb[:],
                                    scalar1=nlo[:, :1], scalar2=0.0,
                                    op0=Alu.add, op1=Alu.is_ge)
            colsum_p = misc_psum.tile([NT, 1], F32, name="mp")
            nc.tensor.matmul(out=colsum_p[:], lhsT=maskf[:], rhs=o128x1_sb[:],
                             start=True, stop=True)
            colsum = radix.tile([NT, 1], F32, name="colsum")
            nc.vector.tensor_copy(out=colsum[:], in_=colsum_p[:])
            excl_p = misc_psum.tile([NT, 1], F32, name="mp")
            nc.tensor.matmul(out=excl_p[:], lhsT=slt32_sb[:], rhs=colsum[:],
                             start=True, stop=True)
            excl = radix.tile([NT, 1], F32, name="excl")
            nc.vector.tensor_copy(out=excl[:], in_=excl_p[:])
            diag = radix.tile([NT, NT], F32, name="diag")
            nc.vector.tensor_tensor(out=diag[:], in0=id32_sb[:],
                                    in1=excl[:, :1].to_broadcast([NT, NT]),
                                    op=Alu.mult)
            rank_p = misc_psum.tile([P, NT], F32, name="mp")
            nc.tensor.matmul(out=rank_p[:], lhsT=ltri_sb[:], rhs=maskf[:],
                             start=True, stop=False, skip_group_check=True)
            nc.tensor.matmul(out=rank_p[:], lhsT=o32x128_sb[:], rhs=diag[:],
                             start=False, stop=True, skip_group_check=True)
            nc.vector.tensor_scalar(out=offf_c[:], in0=rank_p[:],
                                    scalar1=hb_col[:, :1], scalar2=None,
                                    op0=Alu.subtract)
            nc.vector.tensor_copy(out=maskf_c[:], in_=maskf[:])

        misc_psum_ctx.__exit__(None, None, None)

        # ---- w1 cast-loads on the Pool queue.  Positioned after the radix
        # all_reduces so the in-order queue starts them only ~70us in, after
        # the x-tile DMAs have drained (they'd otherwise steal DMA bandwidth
        # from the critical-path score loads). ---------------------------------
        w1bf = []
        for kd in range(ND):
            t_ = w1_pool.tile([P, DFF], BF16, name=f"w1bf_{kd}")
            nc.gpsimd.dma_start(out=t_[:], in_=w1[kd * P:(kd + 1) * P, :])
            w1bf.append(t_)

        # ---- phase E: digit split + one-hot compaction matmuls --------------
        # off in [0, SEL) for in-window selected tokens; any other off value
        # (negative rank-window miss, >=SEL, or collision of an unselected
        # token) produces no match in the lo-digit equality below, and
        # unselected tokens are additionally zeroed via tokid*mask weights.
        with ExitStack() as SE:
            ep = SE.enter_context(tc.tile_pool(name="epool", bufs=1))
            e_psum = SE.enter_context(tc.tile_pool(name="e_psum", bufs=2,
                                                   space="PSUM"))
            off = offf_c
            eq7a = ep.tile([P, NT, 7], F32, name="eq7a")
            nc.vector.tensor_tensor(
                out=eq7a[:], in0=off[:, :, None].to_broadcast([P, NT, 7]),
                in1=thr128[:, None, :].to_broadcast([P, NT, 7]), op=Alu.is_ge)
            hi128 = ep.tile([P, NT], F32, name="hi128")
            nc.vector.tensor_reduce(out=hi128[:], in_=eq7a[:],
                                    axis=mybir.AxisListType.X, op=Alu.add)
            hm = ep.tile([P, NT], F32, name="hm")
            nc.vector.tensor_scalar(out=hm[:], in0=hi128[:], scalar1=-128.0,
                                    scalar2=None, op0=Alu.mult)
            lo128 = ep.tile([P, NT], F32, name="lo128")
            nc.vector.tensor_tensor(out=lo128[:], in0=off[:], in1=hm[:],
                                    op=Alu.add)
            eq7b = ep.tile([P, NT, 7], F32, name="eq7b")
            nc.vector.tensor_tensor(
                out=eq7b[:], in0=lo128[:, :, None].to_broadcast([P, NT, 7]),
                in1=thr16[:, None, :].to_broadcast([P, NT, 7]), op=Alu.is_ge)
            mid = ep.tile([P, NT], F32, name="mid")
            nc.vector.tensor_reduce(out=mid[:], in_=eq7b[:],
                                    axis=mybir.AxisListType.X, op=Alu.add)
            hm2 = ep.tile([P, NT], F32, name="hm2")
            nc.vector.tensor_scalar(out=hm2[:], in0=mid[:], scalar1=-16.0,
                                    scalar2=None, op0=Alu.mult)
            lo16b = ep.tile([P, NT], BF16, name="lo16b")
            nc.vector.tensor_tensor(out=lo16b[:], in0=lo128[:], in1=hm2[:],
                                    op=Alu.add)
            h8 = ep.tile([P, NT], F32, name="h8")
            nc.vector.tensor_scalar(out=h8[:], in0=hi128[:], scalar1=8.0,
                                    scalar2=None, op0=Alu.mult)
            hi16b = ep.tile([P, NT], BF16, name="hi16b")
            nc.vector.tensor_tensor(out=hi16b[:], in0=h8[:], in1=mid[:],
                                    op=Alu.add)
            # token id = c*128 + p; weight the SMALL equality factors by
            # c*mask (chain C, lhsT cols 0:16) and p*mask (chain D, cols
            # 16:32), then sel16 = 128*C + D.  All factors are small exact
            # integers, so the chain runs in bf16 (1 cycle/row matmuls).
            maskb = ep.tile([P, NT], BF16, name="maskb")
            nc.vector.tensor_copy(out=maskb[:], in_=maskf_c[:])
            cwm = ep.tile([P, NT], BF16, name="cwm")
            nc.vector.tensor_tensor(out=cwm[:], in0=cvalb[:], in1=maskb[:],
                                    op=Alu.mult)
            pwm = ep.tile([P, NT], BF16, name="pwm")
            nc.vector.tensor_tensor(out=pwm[:], in0=maskb[:],
                                    in1=iotab[:, :1].to_broadcast([P, NT]),
                                    op=Alu.mult)

            # build the equality factors in half-tile chunks so the first 16
            # compaction matmuls overlap construction of the second half
            eq16 = ep.tile([P, NT, 16], BF16, name="eq16")
            eqcp = ep.tile([P, NT, 32], BF16, name="eqcp")
            eq64 = ep.tile([P, NT, 64], BF16, name="eq64")
            pCD = e_psum.tile([32, 64], F32, name="pCD")
            H = NT // 2
            for h0 in (0, H):
                sl = slice(h0, h0 + H)
                nc.vector.tensor_tensor(
                    out=eq16[:, sl, :], in0=iJ16b[:, sl, :],
                    in1=lo16b[:, sl, None].to_broadcast([P, H, 16]),
                    op=Alu.is_equal)
                nc.vector.tensor_tensor(
                    out=eqcp[:, sl, 0:16], in0=eq16[:, sl, :],
                    in1=cwm[:, sl, None].to_broadcast([P, H, 16]), op=Alu.mult)
                nc.vector.tensor_tensor(
                    out=eqcp[:, sl, 16:32], in0=eq16[:, sl, :],
                    in1=pwm[:, sl, None].to_broadcast([P, H, 16]), op=Alu.mult)
                nc.vector.tensor_tensor(
                    out=eq64[:, sl, :], in0=iK64b[:, sl, :],
                    in1=hi16b[:, sl, None].to_broadcast([P, H, 64]),
                    op=Alu.is_equal)
                for c in range(h0, h0 + H):
                    nc.tensor.matmul(out=pCD[:], lhsT=eqcp[:, c, :],
                                     rhs=eq64[:, c, :], start=(c == 0),
                                     stop=(c == NT - 1), skip_group_check=True)

            sCD = ep.tile([32, 64], F32, name="sCD")
            nc.vector.tensor_copy(out=sCD[:], in_=pCD[:])

            # scatter index layout [128, 64] (16-wrap replicated to 128);
            # lhsT folds the 128*C + D combine (rows 0:16 scaled by 128)
            rep_ps = e_psum.tile([P, 64], F32, name="rep_ps")
            nc.tensor.matmul(out=rep_ps[:], lhsT=rep16_sb[:], rhs=sCD[:],
                             start=True, stop=True)
            nc.vector.tensor_copy(out=idx16_sb[:], in_=rep_ps[:])  # f32->i16

            # gather index layout [128, 8]: selidx[p, k] = sel16[p%16, 8k+p//16]
            selps = e_psum.tile([P, NSJ], F32, name="selps")
            for g in range(8):
                nc.tensor.matmul(out=selps[:], lhsT=ewrap_sb[:, g * P:(g + 1) * P],
                                 rhs=sCD[:, g::8], start=(g == 0),
                                 stop=(g == 7), skip_group_check=True)
            nc.vector.tensor_copy(out=selidx_sb[:], in_=selps[:])  # f32->i32

        dig_ctx.__exit__(None, None, None)

        # ---- gather + transpose + MLP ---------------------------------------
        if True:
            with ExitStack() as SB:
                xt_pool = SB.enter_context(tc.tile_pool(name="xt", bufs=1))
                xsel_pool = SB.enter_context(tc.tile_pool(name="xsel", bufs=5))
                mm1_psum = SB.enter_context(tc.tile_pool(name="mm1_psum", bufs=6,
                                                         space="PSUM"))

                # xt3[p, kd, t] = x_sel[t, kd*128+p], built by the DMA-engine
                # xbar transpose (one per gathered 128-token chunk)
                xt3 = xt_pool.tile([P, ND, SEL], BF16)
                for j in range(NSJ):
                    xs = xsel_pool.tile([P, D], BF16, name="xsel")
                    nc.gpsimd.indirect_dma_start(
                        out=xs[:], out_offset=None, in_=x_row,
                        in_offset=IndirectOffsetOnAxis(ap=selidx_sb[:, j:j + 1],
                                                       axis=0))
                    nc.scalar.dma_start_transpose(
                        out=xt3[:, :, j * P:(j + 1) * P], in_=xs[:])

                # ---- mm1: ht[m, sel] = gelu(w1^T x_sel^T + b1).  The first
                # four token blocks are 128 wide so the PE starts the moment
                # each transpose lands instead of waiting for four of them;
                # the second half runs as one 512-wide block.
                for t0, tw in [(0, P), (P, P), (2 * P, P), (3 * P, P),
                               (512, 512)]:
                    for m in range(NM):
                        ph = mm1_psum.tile([P, tw], F32, name="ph")
                        for kd in range(ND):
                            nc.tensor.matmul(
                                out=ph[:],
                                lhsT=w1bf[kd][:, m * P:(m + 1) * P],
                                rhs=xt3[:, kd, t0:t0 + tw],
                                start=(kd == 0), stop=(kd == ND - 1),
                            )
                        nc.scalar.activation(
                            out=ht[:, m, t0:t0 + tw], in_=ph[:],
                            func=Act.Gelu_apprx_tanh, bias=b1t_sb[:, m:m + 1],
                            scale=1.0,
                        )

            w1_ctx.__exit__(None, None, None)  # free w1 region for w2 stream

            # ---- mm2: y[sel, D] = ht^T @ w2 + b2, then scatter-add ----------
            with ExitStack() as SY:
                y_pool = SY.enter_context(tc.tile_pool(name="y", bufs=1))
                w2_pool = SY.enter_context(tc.tile_pool(name="w2s", bufs=16))
                mm2_psum = SY.enter_context(tc.tile_pool(name="mm2_psum", bufs=8,
                                                         space="PSUM"))
                # d-half 0: kg-major accumulation (w2 tiles stream in, all 8
                # token-block psums accumulate together)
                n = 0
                y_0 = y_pool.tile([P, NSJ, 512], F32, name="y0")
                pys = [mm2_psum.tile([P, 512], F32, name="py")
                       for _ in range(NSJ)]
                w2n1 = []   # d-half-1 tiles retained for the s-major pass
                for s in range(NSJ):
                    nc.tensor.matmul(
                        out=pys[s][:], lhsT=o1x128b_sb[:],
                        rhs=b2bf_sb[:, :512],
                        start=True, stop=False, skip_group_check=True,
                    )
                for kg in range(NM // NKGRP):
                    w2t = w2_pool.tile([P, NKGRP, 512], BF16, name="w2t")
                    if kg == 0:
                        # WAW gate: keep the w2 stream off the DMA engines
                        # until the gather/transpose pipeline has fed mm1
                        nc.vector.tensor_copy(out=w2t[0:1, 0, 0:1],
                                              in_=ht[0:1, 0, 0:1])
                    src = w2[:, :512].rearrange(
                        "(g p) f -> p g f", p=P)[:, kg * NKGRP:(kg + 1) * NKGRP, :]
                    nc.gpsimd.dma_start(out=w2t[:], in_=src)
                    for ki in range(NKGRP):
                        kk = kg * NKGRP + ki
                        for s in range(NSJ):
                            nc.tensor.matmul(
                                out=pys[s][:],
                                lhsT=ht[:, kk, s * P:(s + 1) * P],
                                rhs=w2t[:, ki, :],
                                start=False, stop=(kk == NM - 1),
                                skip_group_check=True,
                            )
                # prefetch d-half-1 w2 tiles while the n=0 tail accumulates
                for kg in range(NM // NKGRP):
                    w2t = w2_pool.tile([P, NKGRP, 512], BF16, name="w2t")
                    src = w2[:, 512:].rearrange(
                        "(g p) f -> p g f", p=P)[:, kg * NKGRP:(kg + 1) * NKGRP, :]
                    nc.gpsimd.dma_start(out=w2t[:], in_=src)
                    w2n1.append(w2t)
                for s in range(NSJ):
                    nc.scalar.activation(out=y_0[:, s, :], in_=pys[s][:],
                                         func=Act.Copy, bias=0.0, scale=1.0)
                    if s % 4 == 3:
                        h = s // 4
                        nc.gpsimd.dma_scatter_add(
                            out_row[:, :512],
                            y_0[:, h * 4:(h + 1) * 4, :],
                            idx16_sb[:, h * 32:(h + 1) * 32],
                            SEL // 2,
                            SEL // 2,
                            512,
                            elem_step=D,
                        )

                # d-half 1: s-major (each token block finishes early and its
                # rows scatter while the next block accumulates)
                y_1 = y_pool.tile([P, NSJ, 512], F32, name="y1")
                for s in range(NSJ):
                    py = mm2_psum.tile([P, 512], F32, name="py")
                    nc.tensor.matmul(
                        out=py[:], lhsT=o1x128b_sb[:], rhs=b2bf_sb[:, 512:],
                        start=True, stop=False, skip_group_check=True,
                    )
                    for kk in range(NM):
                        nc.tensor.matmul(
                            out=py[:],
                            lhsT=ht[:, kk, s * P:(s + 1) * P],
                            rhs=w2n1[kk // NKGRP][:, kk % NKGRP, :],
                            start=False, stop=(kk == NM - 1),
                            skip_group_check=True,
                        )
                    nc.scalar.activation(out=y_1[:, s, :], in_=py[:],
                                         func=Act.Copy, bias=0.0, scale=1.0)
                    nc.gpsimd.dma_scatter_add(
                        out_row[:, 512:],
                        y_1[:, s:s + 1, :],
                        idx16_sb[:, s * 8:(s + 1) * 8],
                        P,
                        P,
                        512,
                        elem_step=D,
                    )

        ht_ctx.__exit__(None, None, None)

    nc.compile()
    return nc


def make_consts():
    q = np.arange(P)
    import ml_dtypes
    consts = {
        "identb": np.eye(P, dtype=ml_dtypes.bfloat16),
        "identf": np.eye(P, dtype=np.float32),
        "ltri128": (q[:, None] < q[None, :]).astype(np.float32),  # [q, p] = q < p
        "slt32": (np.arange(NT)[:, None] < np.arange(NT)[None, :]).astype(np.float32),
        "id32": np.eye(NT, dtype=np.float32),
        "ones_1x128": np.ones((1, P), np.float32),
        "ones_1x128b": np.ones((1, P), ml_dtypes.bfloat16),
        "ones_128x1": np.ones((P, 1), np.float32),
        "ones128": np.ones((P, P), np.float32),
        "ones_32x128": np.ones((NT, P), np.float32),
        "rep16": np.vstack([
            128.0 * (np.arange(16)[:, None] == (np.arange(P)[None, :] % 16)),
            1.0 * (np.arange(16)[:, None] == (np.arange(P)[None, :] % 16)),
        ]).astype(np.float32),
    }
    # ewrap[i, g*128 + p] = 1 iff p == g*16 + i  (16-wrap -> 128-wrap expand);
    # stacked [32, .]: rows 0:16 scaled by 128 (C chain), rows 16:32 raw (D)
    ew = np.zeros((16, 8 * P), np.float32)
    for i in range(16):
        for g in range(8):
            ew[i, g * P + g * 16 + i] = 1.0
    consts["ewrap"] = np.vstack([128.0 * ew, ew]).astype(np.float32)
    return consts


def make_in_maps(x, W1, b1, W2, b2, wr, br):
    consts = make_consts()
    x = np.ascontiguousarray(np.asarray(x, np.float32))
    in_maps = []
    for c in range(NCORES):
        b, h = divmod(c, 2)
        m = {
            "x_row": x[b],
            "w1": np.asarray(W1, np.float32),
            "w2": np.asarray(W2, np.float32),
            "wr": np.asarray(wr, np.float32).reshape(1, D),
            "b1t": np.ascontiguousarray(np.asarray(b1, np.float32).reshape(NM, P).T),
            "b2": np.asarray(b2, np.float32).reshape(1, D),
            "hbase": np.array([[h * SEL]], np.float32),
        }
        m.update(consts)
        in_maps.append(m)
    return in_maps


_NC_CACHE = None


def _get_program():
    global _NC_CACHE
    if _NC_CACHE is None:
        _NC_CACHE = build_program()
    return _NC_CACHE


def kernel(x, W1, b1, W2, b2, wr, br):
    from concourse.bass_utils import run_bass_kernel_spmd

    nc = _get_program()
    in_maps = make_in_maps(x, W1, b1, W2, b2, wr, br)
    res = run_bass_kernel_spmd(nc, in_maps, list(range(NCORES))).results
    out = np.stack(
        [res[2 * b]["out_row"] + res[2 * b + 1]["out_row"] for b in range(B)]
    )
    return out.astype(np.float32)
